# revision 2
# baseline (speedup 1.0000x reference)
"""Trainium2 Bass kernel for nn_Attention (MCAM + MSAM + CIAFM block).

Sharding: pure data parallelism — B=8 samples across 8 NeuronCores.
Per core: x,skip (256, 64, 64) f32 -> out (256, 64, 64) f32.

Heavy compute = four 3x3 convs (ms, fq, fuse, c3) done as 9-tap
PSUM-accumulated bf16 matmuls over zero-padded (128, 66, 66) SBUF tiles,
plus 1x1 convs (fup, c1) and a tiny NC=19 cross-attention.

Host-side preprocessing folds:
  - BN scales into conv weights (cbr -> relu6(conv(x, W*s) + t))
  - the entire MCAM front end (4 ECA conv1ds + k=3 mixer + FC) into one
    linear map  g_pre = M @ [avgpool; maxpool] + d   (M: 256x512)
"""

import os
import numpy as np
import ml_dtypes

BF = ml_dtypes.bfloat16

DIM, NCLS, SZ = 256, 19, 64
KC = 128
HW = SZ * SZ          # 4096
PD = SZ + 2           # 66
NT = 8                # 512-wide output column tiles
RT = SZ // NT         # 8 rows per tile
S_ATT = float(KC) ** -0.5


# ------------------------------------------------------------------ host prep

def _toeplitz(w, n):
    """Dense matrix of 'same'-padded 1-D cross-correlation with kernel w."""
    w = np.asarray(w, np.float64).reshape(-1)
    k = len(w)
    pad = (k - 1) // 2
    T = np.zeros((n, n), np.float64)
    for j in range(k):
        d = j - pad
        lo, hi = max(0, -d), min(n, n - d)
        idx = np.arange(lo, hi)
        T[idx, idx + d] += w[j]
    return T


def _pack_conv(w, scale=None):
    """(O, I, kh, kw) -> (128, I//128, kh*kw, O): lhsT tiles per (in-block, tap)."""
    w = np.asarray(w, np.float64)
    if scale is not None:
        w = w * np.asarray(scale, np.float64)[:, None, None, None]
    O, I, kh, kw = w.shape
    t = w.reshape(O, I, kh * kw).transpose(1, 2, 0)          # (I, taps, O)
    t = t.reshape(I // 128, 128, kh * kw, O).transpose(1, 0, 2, 3)
    return np.ascontiguousarray(t)


def _bf16(a):
    return np.ascontiguousarray(np.asarray(a, np.float32)).astype(BF)


def _f32(a):
    return np.ascontiguousarray(np.asarray(a, np.float32))


def _cols(v, nb):
    """(nb*128,) bias vector -> (128, nb): column ob = v[ob*128:(ob+1)*128]."""
    return _f32(np.asarray(v, np.float64).reshape(nb, 128).T)


def _prep(params):
    p = {k: np.asarray(v, np.float64) for k, v in params.items()}
    o = {}

    # --- fused MCAM front end: g_pre = M @ [avg; max] + d ---
    T1 = np.zeros((8 * DIM, 2 * DIM))
    b1 = np.zeros(8 * DIM)
    for br in range(2):                       # 0 = avg branch, 1 = max branch
        for kk, nm in enumerate(("mc0", "mc1", "mc2", "mc3")):
            r = (br * 4 + kk) * DIM
            T1[r:r + DIM, br * DIM:(br + 1) * DIM] = _toeplitz(p[nm + "_w"], DIM)
            b1[r:r + DIM] = p[nm + "_b"][0]
    T2 = _toeplitz(p["mcc_w"], 8 * DIM)
    bias2 = T2 @ b1 + p["mcc_b"][0]
    M = p["fc_w"] @ T2 @ T1                   # (256, 512)
    d = p["fc_w"] @ bias2 + p["fc_b"]         # (256,)
    o["wg"] = _bf16(M.T.reshape(4, 128, DIM).transpose(1, 0, 2))   # (128,4,256)
    o["dg"] = _cols(d, 2)                                          # (128,2)

    # --- mcam mid (cse1/cse2/fc1) ---
    o["wcse1"] = _bf16(_pack_conv(p["cse1_w"]))[:, :, 0, :]        # (128,2,64)
    o["bcse1"] = _f32(p["cse1_b"].reshape(64, 1))
    o["wcse2"] = _bf16(p["cse2_w"][:, :, 0, 0].T)                  # (64,19)
    o["bcse2"] = _f32(p["cse2_b"].reshape(NCLS, 1))
    o["wfc1"] = _bf16(_pack_conv(p["fc1_w"], p["fc1_s"]))[:, :, 0, :]  # (128,2,19)
    o["bfc1"] = _f32(p["fc1_t"].reshape(NCLS, 1))

    # --- conv weights, BN scale folded ---
    o["wms"] = _bf16(_pack_conv(p["ms_w"], p["ms_s"]))             # (128,2,9,256)
    o["tms"] = _cols(p["ms_t"], 2)
    o["wfq"] = _bf16(_pack_conv(p["fq_w"], p["fq_s"]))             # (128,2,9,128)
    o["tfq"] = _f32(p["fq_t"].reshape(1, 128).T)                   # (128,1)
    o["wfk"] = _bf16(_pack_conv(p["fk_w"], p["fk_s"]))[:, :, 0, :]  # (128,2,128)
    o["tfk"] = _f32(p["fk_t"].reshape(1, 128).T)
    o["wfv"] = _bf16(_pack_conv(p["fv_w"], p["fv_s"]))[:, :, 0, :]
    o["tfv"] = _f32(p["fv_t"].reshape(1, 128).T)
    o["wfup"] = _bf16(_pack_conv(p["fup_w"], p["fup_s"]))[:, 0, 0, :]  # (128,256)
    o["tfup"] = _cols(p["fup_t"], 2)
    o["wfuse"] = _bf16(_pack_conv(p["fuse_w"], p["fuse_s"]))
    o["tfuse"] = _cols(p["fuse_t"], 2)
    o["wc3"] = _bf16(_pack_conv(p["c3_w"], p["c3_s"]))
    o["tc3"] = _cols(p["c3_t"], 2)
    o["wc1"] = _bf16(_pack_conv(p["c1_w"], p["c1_s"]))[:, :, 0, :]  # (128,4,256)
    o["tc1"] = _cols(p["c1_t"], 2)

    o["ident"] = _bf16(np.eye(128))
    return o


# ------------------------------------------------------------- device program

def build_program():
    import concourse.tile as tile
    from concourse import bacc, mybir

    AF = mybir.ActivationFunctionType
    ALU = mybir.AluOpType
    F32 = mybir.dt.float32
    BF16 = mybir.dt.bfloat16

    nc = bacc.Bacc("TRN2", target_bir_lowering=False, debug=False)

    specs = [
        ("x", (DIM, HW), F32), ("skip", (DIM, HW), F32),
        ("wg", (128, 4, DIM), BF16), ("dg", (128, 2), F32),
        ("wcse1", (128, 2, 64), BF16), ("bcse1", (64, 1), F32),
        ("wcse2", (64, NCLS), BF16), ("bcse2", (NCLS, 1), F32),
        ("wfc1", (128, 2, NCLS), BF16), ("bfc1", (NCLS, 1), F32),
        ("wms", (128, 2, 9, DIM), BF16), ("tms", (128, 2), F32),
        ("wfq", (128, 2, 9, KC), BF16), ("tfq", (128, 1), F32),
        ("wfk", (128, 2, KC), BF16), ("tfk", (128, 1), F32),
        ("wfv", (128, 2, KC), BF16), ("tfv", (128, 1), F32),
        ("wfup", (128, DIM), BF16), ("tfup", (128, 2), F32),
        ("wfuse", (128, 2, 9, DIM), BF16), ("tfuse", (128, 2), F32),
        ("wc3", (128, 2, 9, DIM), BF16), ("tc3", (128, 2), F32),
        ("wc1", (128, 4, DIM), BF16), ("tc1", (128, 2), F32),
        ("ident", (128, 128), BF16),
    ]
    dram = {n: nc.dram_tensor(n, list(s), dt, kind="ExternalInput")
            for n, s, dt in specs}
    out_d = nc.dram_tensor("out", [DIM, HW], F32, kind="ExternalOutput")

    with tile.TileContext(nc) as tc:
        with tc.tile_pool(name="cst", bufs=1) as cst, \
             tc.tile_pool(name="glob", bufs=1) as glob, \
             tc.tile_pool(name="psp", bufs=6, space="PSUM") as psp, \
             tc.tile_pool(name="pss", bufs=2, space="PSUM") as pss:

            # ---- constants into SBUF ----
            W = {}
            for n, s, dt in specs:
                if n in ("x", "skip"):
                    continue
                t = cst.tile(list(s), dt, tag=n, name=f"c_{n}")
                nc.sync.dma_start(out=t, in_=dram[n].ap())
                W[n] = t

            def pst(name, nt=1):
                return psp.tile([128, 512], F32, tag="ps", name=name)

            def psq(name, shape, dt=None):
                return pss.tile(list(shape), dt or F32, tag="pss", name=name)

            # ---- padded conv-input buffers ----
            def padded(tagname):
                ts = [glob.tile([128, PD, PD], BF16, tag=f"{tagname}{i}",
                                name=f"{tagname}{i}") for i in range(2)]
                for t in ts:
                    nc.gpsimd.memset(t[:, 0, :], 0.0)
                    nc.gpsimd.memset(t[:, PD - 1, :], 0.0)
                    nc.gpsimd.memset(t[:, 1:PD - 1, 0], 0.0)
                    nc.gpsimd.memset(t[:, 1:PD - 1, PD - 1], 0.0)
                return ts

            P1 = padded("P1")   # msam y  (ms conv input)
            P2 = padded("P2")   # sa      (fq / c3 / fuse-add input)
            P3 = padded("P3")   # fup_out + sa (fuse conv input)

            def interior(P, nt=None):
                if nt is None:
                    return P[:, 1:1 + SZ, 1:1 + SZ]
                return P[:, 1 + nt * RT:1 + (nt + 1) * RT, 1:1 + SZ]

            # small cross-phase tensors
            proxy_b = [glob.tile([128, NCLS], BF16, tag=f"proxy{i}",
                                 name=f"proxy{i}") for i in range(2)]
            k_b = glob.tile([128, NCLS], BF16, tag="k_b", name="k_b")
            vT_b = glob.tile([NCLS, 128], BF16, tag="vT_b", name="vT_b")

            relu6 = lambda ap: nc.vector.tensor_scalar_min(ap, ap, 6.0)

            # =============== early phase: inputs, MCAM, MSAM pre-conv =======
            with tc.tile_pool(name="early", bufs=1) as early:
                xx = [early.tile([128, HW], F32, tag=f"xx{i}", name=f"xx{i}")
                      for i in range(2)]
                for ib in range(2):
                    for c in range(NT):
                        sl = slice(c * 512, (c + 1) * 512)
                        xt = early.tile([128, 512], F32, tag="xt", bufs=3,
                                        name=f"xt{ib}_{c}")
                        st = early.tile([128, 512], F32, tag="st", bufs=3,
                                        name=f"st{ib}_{c}")
                        nc.sync.dma_start(out=xt, in_=dram["x"].ap()[ib * 128:(ib + 1) * 128, sl])
                        nc.sync.dma_start(out=st, in_=dram["skip"].ap()[ib * 128:(ib + 1) * 128, sl])
                        nc.vector.tensor_add(xx[ib][:, sl], xt, st)

                # ---- MCAM: pooled stats -> g ----
                avg_b, max_b = [], []
                for ib in range(2):
                    ssum = early.tile([128, 1], F32, tag=f"ssum{ib}", name=f"ssum{ib}")
                    smax = early.tile([128, 1], F32, tag=f"smax{ib}", name=f"smax{ib}")
                    nc.vector.tensor_reduce(out=ssum, in_=xx[ib][:],
                                            axis=mybir.AxisListType.X, op=ALU.add)
                    nc.vector.tensor_reduce(out=smax, in_=xx[ib][:],
                                            axis=mybir.AxisListType.X, op=ALU.max)
                    ab = early.tile([128, 1], BF16, tag=f"ab{ib}", name=f"ab{ib}")
                    mb = early.tile([128, 1], BF16, tag=f"mb{ib}", name=f"mb{ib}")
                    nc.scalar.activation(out=ab, in_=ssum, func=AF.Copy, scale=1.0 / HW)
                    nc.scalar.activation(out=mb, in_=smax, func=AF.Copy)
                    avg_b.append(ab)
                    max_b.append(mb)
                vblocks = [avg_b[0], avg_b[1], max_b[0], max_b[1]]

                g_colb = []     # g as bf16 columns per out-block
                g_row = early.tile([1, DIM], F32, tag="g_row", name="g_row")
                for ob in range(2):
                    pg = psq(f"psg{ob}", (128, 1))
                    for j in range(4):
                        nc.tensor.matmul(pg[:], lhsT=W["wg"][:, j, ob * 128:(ob + 1) * 128],
                                         rhs=vblocks[j], start=(j == 0), stop=(j == 3))
                    gc = early.tile([128, 1], BF16, tag=f"gc{ob}", name=f"gc{ob}")
                    nc.scalar.activation(out=gc, in_=pg[:], func=AF.Sigmoid,
                                         bias=W["dg"][:, ob:ob + 1], scale=1.0)
                    g_colb.append(gc)
                    # row copy of g for the class-matrix outer product
                    pr = psq(f"psgr{ob}", (1, 128), BF16)
                    nc.tensor.transpose(pr[:], gc[:], W["ident"][:, :])
                    nc.scalar.activation(out=g_row[:, ob * 128:(ob + 1) * 128],
                                         in_=pr[:], func=AF.Copy)

                # h = relu(cse1 @ g + b) ; y1 = sigmoid(cse2 @ h + b)
                ph = psq("psh", (64, 1))
                for ib in range(2):
                    nc.tensor.matmul(ph[:], lhsT=W["wcse1"][:, ib, :], rhs=g_colb[ib],
                                     start=(ib == 0), stop=(ib == 1))
                h_b = early.tile([64, 1], BF16, tag="h_b", name="h_b")
                nc.scalar.activation(out=h_b, in_=ph[:], func=AF.Relu,
                                     bias=W["bcse1"][:, 0:1], scale=1.0)
                py = psq("psy", (NCLS, 1))
                nc.tensor.matmul(py[:], lhsT=W["wcse2"][:, :], rhs=h_b,
                                 start=True, stop=True)
                y1c = early.tile([NCLS, 1], BF16, tag="y1c", name="y1c")
                nc.scalar.activation(out=y1c, in_=py[:], func=AF.Sigmoid,
                                     bias=W["bcse2"][:, 0:1], scale=1.0)
                pf = psq("psf", (NCLS, 1))
                for ib in range(2):
                    nc.tensor.matmul(pf[:], lhsT=W["wfc1"][:, ib, :], rhs=g_colb[ib],
                                     start=(ib == 0), stop=(ib == 1))
                f1c = early.tile([NCLS, 1], BF16, tag="f1c", name="f1c")
                nc.scalar.activation(out=f1c, in_=pf[:], func=AF.Identity,
                                     bias=W["bfc1"][:, 0:1], scale=1.0)

                # rows: f1r, y1r (1, 19)
                def to_row(src, nm):
                    pr = psq(f"pr_{nm}", (1, NCLS), BF16)
                    nc.tensor.transpose(pr[:], src[:], W["ident"][0:NCLS, 0:NCLS])
                    r = early.tile([1, NCLS], F32, tag=f"row_{nm}", name=f"row_{nm}")
                    nc.scalar.activation(out=r, in_=pr[:], func=AF.Copy)
                    return r
                f1r = to_row(f1c, "f1")
                y1r = to_row(y1c, "y1")

                # softmax(f1r) + y1r -> class_feat row, then back to column
                nm1 = early.tile([1, 1], F32, tag="nm1", name="nm1")
                nc.vector.tensor_reduce(out=nm1, in_=f1r[:], axis=mybir.AxisListType.X,
                                        op=ALU.max, negate=True)
                s1 = early.tile([1, 1], F32, tag="s1", name="s1")
                e1 = early.tile([1, NCLS], F32, tag="e1", name="e1")
                nc.scalar.activation(out=e1, in_=f1r[:], func=AF.Exp,
                                     bias=nm1[:, 0:1], scale=1.0, accum_out=s1[:, 0:1])
                r1 = early.tile([1, 1], F32, tag="r1", name="r1")
                nc.vector.reciprocal(r1, s1)
                nc.vector.tensor_scalar(out=e1, in0=e1, scalar1=r1[:, 0:1],
                                        scalar2=None, op0=ALU.mult)
                cfr = early.tile([1, NCLS], BF16, tag="cfr", name="cfr")
                nc.vector.tensor_add(cfr, e1, y1r)
                pc = psq("pc_cf", (NCLS, 1), BF16)
                nc.tensor.transpose(pc[:], cfr[:], W["ident"][0:1, 0:1])
                cfc = early.tile([NCLS, 1], F32, tag="cfc", name="cfc")
                nc.scalar.activation(out=cfc, in_=pc[:], func=AF.Copy)

                # class matrix cm^T (19, 256) = cf ⊗ g ; double softmax
                g19 = early.tile([NCLS, DIM], F32, tag="g19", name="g19")
                nc.gpsimd.partition_broadcast(g19[:], g_row[:])
                cmT = early.tile([NCLS, DIM], F32, tag="cmT", name="cmT")
                nc.vector.tensor_scalar(out=cmT, in0=g19, scalar1=cfc[:, 0:1],
                                        scalar2=None, op0=ALU.mult)
                nm2 = early.tile([NCLS, 1], F32, tag="nm2", name="nm2")
                nc.vector.tensor_reduce(out=nm2, in_=cmT[:], axis=mybir.AxisListType.X,
                                        op=ALU.max, negate=True)
                s2 = early.tile([NCLS, 1], F32, tag="s2", name="s2")
                cme = early.tile([NCLS, DIM], BF16, tag="cme", name="cme")
                nc.scalar.activation(out=cme, in_=cmT[:], func=AF.Exp,
                                     bias=nm2[:, 0:1], scale=1.0, accum_out=s2[:, 0:1])
                r2 = early.tile([NCLS, 1], F32, tag="r2", name="r2")
                nc.vector.reciprocal(r2, s2)
                nc.vector.tensor_scalar(out=cme, in0=cme, scalar1=r2[:, 0:1],
                                        scalar2=None, op0=ALU.mult)

                # transpose halves -> (128, 19) x2, softmax over classes
                for ib in range(2):
                    pt = psq(f"pt{ib}", (128, NCLS), BF16)
                    nc.tensor.transpose(pt[:], cme[:, ib * 128:(ib + 1) * 128],
                                        W["ident"][0:NCLS, 0:NCLS])
                    pp = early.tile([128, NCLS], F32, tag=f"pp{ib}", name=f"pp{ib}")
                    nc.scalar.activation(out=pp, in_=pt[:], func=AF.Copy)
                    nm3 = early.tile([128, 1], F32, tag=f"nm3_{ib}", name=f"nm3_{ib}")
                    nc.vector.tensor_reduce(out=nm3, in_=pp[:], axis=mybir.AxisListType.X,
                                            op=ALU.max, negate=True)
                    s3 = early.tile([128, 1], F32, tag=f"s3_{ib}", name=f"s3_{ib}")
                    nc.scalar.activation(out=proxy_b[ib], in_=pp[:], func=AF.Exp,
                                         bias=nm3[:, 0:1], scale=1.0, accum_out=s3[:, 0:1])
                    r3 = early.tile([128, 1], F32, tag=f"r3_{ib}", name=f"r3_{ib}")
                    nc.vector.reciprocal(r3, s3)
                    nc.vector.tensor_scalar(out=proxy_b[ib], in0=proxy_b[ib],
                                            scalar1=r3[:, 0:1], scalar2=None, op0=ALU.mult)

                # k / v 1x1 convs on proxy (+ v transpose)
                for wname, tname, dst in (("wfk", "tfk", k_b), ("wfv", "tfv", None)):
                    pkv = psq(f"pkv_{wname}", (128, NCLS))
                    for ib in range(2):
                        nc.tensor.matmul(pkv[:], lhsT=W[wname][:, ib, :],
                                         rhs=proxy_b[ib], start=(ib == 0), stop=(ib == 1))
                    if dst is None:
                        v_b = early.tile([128, NCLS], BF16, tag="v_b", name="v_b")
                        dst = v_b
                    nc.scalar.activation(out=dst, in_=pkv[:], func=AF.Relu,
                                         bias=W[tname][:, 0:1], scale=1.0)
                    relu6(dst[:])
                pvT = psq("pvT", (NCLS, 128), BF16)
                nc.tensor.transpose(pvT[:], v_b[:], W["ident"][:, :])
                nc.scalar.activation(out=vT_b, in_=pvT[:], func=AF.Copy)

                # ---- MSAM: qk outer product, softmax, y -> P1 ----
                for ib in range(2):
                    x3 = xx[ib][:].rearrange("p (h w) -> p h w", w=SZ)
                    x3t = xx[ib][:].rearrange("p (h w) -> p w h", w=SZ)
                    xw = early.tile([128, SZ], F32, tag=f"xw{ib}", name=f"xw{ib}")
                    xh = early.tile([128, SZ], F32, tag=f"xh{ib}", name=f"xh{ib}")
                    nc.vector.tensor_reduce(out=xw, in_=x3, axis=mybir.AxisListType.X, op=ALU.add)
                    nc.vector.tensor_reduce(out=xh, in_=x3t, axis=mybir.AxisListType.X, op=ALU.add)
                    qk = early.tile([128, SZ, SZ], BF16, tag="qk", name=f"qk{ib}")
                    nc.vector.scalar_tensor_tensor(
                        out=qk, in0=xw[:].unsqueeze(2).broadcast_to([128, SZ, SZ]),
                        scalar=1.0 / (HW * 1.0), in1=xh[:].unsqueeze(1).broadcast_to([128, SZ, SZ]),
                        op0=ALU.mult, op1=ALU.mult)
                    nmq = early.tile([128, 1], F32, tag=f"nmq{ib}", name=f"nmq{ib}")
                    nc.vector.tensor_reduce(out=nmq, in_=qk[:], axis=mybir.AxisListType.XY,
                                            op=ALU.max, negate=True)
                    sq = early.tile([128, 1], F32, tag=f"sq{ib}", name=f"sq{ib}")
                    Ee = early.tile([128, SZ, SZ], F32, tag="Ee", name=f"Ee{ib}")
                    nc.scalar.activation(out=Ee, in_=qk[:], func=AF.Exp,
                                         bias=nmq[:, 0:1], scale=1.0, accum_out=sq[:, 0:1])
                    rq = early.tile([128, 1], F32, tag=f"rq{ib}", name=f"rq{ib}")
                    nc.vector.reciprocal(rq, sq)
                    # Ee <- attn + 1  (f32: keeps the tiny attn correction)
                    nc.vector.tensor_scalar(out=Ee, in0=Ee, scalar1=rq[:, 0:1],
                                            scalar2=1.0, op0=ALU.mult, op1=ALU.add)
                    nc.vector.tensor_tensor(out=interior(P1[ib]), in0=Ee, in1=x3,
                                            op=ALU.mult)

            # =============== late phase: convs + attention ==================
            with tc.tile_pool(name="late", bufs=1) as late, \
                 tc.tile_pool(name="stg", bufs=1) as stg:

                def conv3x3(Pin, wname, tname, n_ob, emit):
                    for ob in range(n_ob):
                        for nt in range(NT):
                            ps = pst(f"ps_{wname}_{ob}_{nt}")
                            y0 = nt * RT
                            kk = 0
                            for ib in range(2):
                                for t9 in range(9):
                                    ky, kx = divmod(t9, 3)
                                    nc.tensor.matmul(
                                        ps[:],
                                        lhsT=W[wname][:, ib, t9, ob * 128:(ob + 1) * 128],
                                        rhs=Pin[ib][:, y0 + ky:y0 + ky + RT, kx:kx + SZ],
                                        start=(kk == 0), stop=(kk == 17))
                                    kk += 1
                            emit(ob, nt, ps)

                # ms conv: P1 -> sa in P2
                def emit_ms(ob, nt, ps):
                    pv = interior(P2[ob], nt)
                    nc.scalar.activation(out=pv, in_=ps[:], func=AF.Relu,
                                         bias=W["tms"][:, ob:ob + 1], scale=1.0)
                    relu6(pv)
                conv3x3(P1, "wms", "tms", 2, emit_ms)

                # fq conv: P2 -> q (natural bf16)
                q_b = late.tile([128, HW], BF16, tag="q_b", name="q_b")
                def emit_fq(ob, nt, ps):
                    pv = q_b[:, nt * 512:(nt + 1) * 512]
                    nc.scalar.activation(out=pv, in_=ps[:], func=AF.Relu,
                                         bias=W["tfq"][:, 0:1], scale=1.0)
                    relu6(pv)
                conv3x3(P2, "wfq", "tfq", 1, emit_fq)

                # attention mm1: (19, 4096) = k^T q
                mm_b = late.tile([NCLS, HW], BF16, tag="mm_b", name="mm_b")
                for nt in range(NT):
                    pm = psp.tile([NCLS, 512], F32, tag="ps", name=f"pmm{nt}")
                    nc.tensor.matmul(pm[:], lhsT=k_b[:], rhs=q_b[:, nt * 512:(nt + 1) * 512],
                                     start=True, stop=True)
                    nc.scalar.activation(out=mm_b[:, nt * 512:(nt + 1) * 512],
                                         in_=pm[:], func=AF.Copy)

                # c3 conv traced now so PE stays busy during the softmax
                sp = [late.tile([128, HW], BF16, tag=f"sp{i}", name=f"sp{i}")
                      for i in range(2)]
                def emit_c3(ob, nt, ps):
                    pv = sp[ob][:, nt * 512:(nt + 1) * 512]
                    nc.scalar.activation(out=pv, in_=ps[:], func=AF.Relu,
                                         bias=W["tc3"][:, ob:ob + 1], scale=1.0)
                    relu6(pv)
                conv3x3(P2, "wc3", "tc3", 2, emit_c3)

                # softmax over hw (free axis) rows of mm
                nma = late.tile([NCLS, 1], F32, tag="nma", name="nma")
                nc.vector.tensor_reduce(out=nma, in_=mm_b[:], axis=mybir.AxisListType.X,
                                        op=ALU.max, negate=True)
                nmas = late.tile([NCLS, 1], F32, tag="nmas", name="nmas")
                nc.vector.tensor_scalar(out=nmas, in0=nma, scalar1=S_ATT,
                                        scalar2=None, op0=ALU.mult)
                sa_s = late.tile([NCLS, 1], F32, tag="sa_s", name="sa_s")
                A_b = late.tile([NCLS, HW], BF16, tag="A_b", name="A_b")
                nc.scalar.activation(out=A_b, in_=mm_b[:], func=AF.Exp,
                                     bias=nmas[:, 0:1], scale=S_ATT, accum_out=sa_s[:, 0:1])
                ra = late.tile([NCLS, 1], F32, tag="ra", name="ra")
                nc.vector.reciprocal(ra, sa_s)
                nc.vector.tensor_scalar(out=A_b, in0=A_b, scalar1=ra[:, 0:1],
                                        scalar2=None, op0=ALU.mult)

                # mm2 + fup + add sa -> P3
                ctx_b = late.tile([128, HW], BF16, tag="ctx_b", name="ctx_b")
                for nt in range(NT):
                    pc2 = pst(f"pctx{nt}")
                    nc.tensor.matmul(pc2[:], lhsT=vT_b[:], rhs=A_b[:, nt * 512:(nt + 1) * 512],
                                     start=True, stop=True)
                    nc.scalar.activation(out=ctx_b[:, nt * 512:(nt + 1) * 512],
                                         in_=pc2[:], func=AF.Copy)
                for ob in range(2):
                    for nt in range(NT):
                        pu = pst(f"pfup{ob}_{nt}")
                        nc.tensor.matmul(pu[:], lhsT=W["wfup"][:, ob * 128:(ob + 1) * 128],
                                         rhs=ctx_b[:, nt * 512:(nt + 1) * 512],
                                         start=True, stop=True)
                        fs = stg.tile([128, 512], BF16, tag="fs", bufs=3,
                                      name=f"fs{ob}_{nt}")
                        nc.scalar.activation(out=fs, in_=pu[:], func=AF.Relu,
                                             bias=W["tfup"][:, ob:ob + 1], scale=1.0)
                        nc.vector.scalar_tensor_tensor(
                            out=interior(P3[ob], nt), in0=fs, scalar=6.0,
                            in1=interior(P2[ob], nt), op0=ALU.min, op1=ALU.add)

                # fuse conv: P3 -> ctxf
                ctxf = [late.tile([128, HW], BF16, tag=f"ctxf{i}", name=f"ctxf{i}")
                        for i in range(2)]
                def emit_fuse(ob, nt, ps):
                    pv = ctxf[ob][:, nt * 512:(nt + 1) * 512]
                    nc.scalar.activation(out=pv, in_=ps[:], func=AF.Relu,
                                         bias=W["tfuse"][:, ob:ob + 1], scale=1.0)
                    relu6(pv)
                conv3x3(P3, "wfuse", "tfuse", 2, emit_fuse)

                # c1 1x1 over concat([ctxf, sp]) -> out
                cat = [ctxf[0], ctxf[1], sp[0], sp[1]]
                for ob in range(2):
                    for nt in range(NT):
                        ps = pst(f"pc1_{ob}_{nt}")
                        for j in range(4):
                            nc.tensor.matmul(ps[:], lhsT=W["wc1"][:, j, ob * 128:(ob + 1) * 128],
                                             rhs=cat[j][:, nt * 512:(nt + 1) * 512],
                                             start=(j == 0), stop=(j == 3))
                        og = stg.tile([128, 512], F32, tag="og", bufs=4,
                                      name=f"og{ob}_{nt}")
                        nc.scalar.activation(out=og, in_=ps[:], func=AF.Relu,
                                             bias=W["tc1"][:, ob:ob + 1], scale=1.0)
                        relu6(og[:])
                        nc.sync.dma_start(
                            out=out_d.ap()[ob * 128:(ob + 1) * 128, nt * 512:(nt + 1) * 512],
                            in_=og)

    nc.compile()
    return nc


# ------------------------------------------------------------------- wrapper

_CACHE = {}


def kernel(x, skip, params):
    from concourse import bass_utils

    x = np.asarray(x, np.float32)
    skip = np.asarray(skip, np.float32)
    B = x.shape[0]
    packed = _prep(params)

    if "nc" not in _CACHE:
        _CACHE["nc"] = build_program()
    nc = _CACHE["nc"]

    in_maps = []
    for i in range(B):
        m = dict(packed)
        m["x"] = np.ascontiguousarray(x[i].reshape(DIM, HW))
        m["skip"] = np.ascontiguousarray(skip[i].reshape(DIM, HW))
        in_maps.append(m)

    res = bass_utils.run_bass_kernel_spmd(
        nc, in_maps, core_ids=list(range(B)),
        trace=bool(int(os.environ.get("KBENCH_TRACE", "0"))))
    _CACHE["last_result"] = res
    out = np.stack([r["out"].reshape(DIM, SZ, SZ) for r in res.results])
    return out.astype(np.float32)


# revision 10
# speedup vs baseline: 2.8137x; 2.8137x over previous
"""Trainium2 Bass kernel for nn_Attention (MCAM + MSAM + CIAFM block).

Sharding: pure data parallelism — B=8 samples across 8 NeuronCores.
Per core: x,skip (256, 64, 64) f32 -> out (256, 64, 64) f32.

Heavy compute = four 3x3 convs (ms, fq, fuse, c3) done as 9-tap
PSUM-accumulated bf16 matmuls over zero-padded (128, 66, 66) SBUF tiles,
plus 1x1 convs (fup, c1) and a tiny NC=19 cross-attention.

Host-side preprocessing folds:
  - BN scales into conv weights (cbr -> relu6(conv(x, W*s) + t))
  - the entire MCAM front end (4 ECA conv1ds + k=3 mixer + FC) into one
    linear map  g_pre = M @ [avgpool; maxpool] + d   (M: 256x512)

Schedule: inputs stream in per 512-column chunk (adds on GPSIMD, partial
pool stats on DVE during the DMA window); MSAM softmax+modulation feeds
the ms conv per chunk; the ms conv is split by input channel block so its
block-0 matmuls start before block-1's modulated input is finished. The
serial MCAM chain (tiny) is traced between ms and fq so it never blocks
the PE FIFO; c3 is traced before the attention softmax for the same
reason.
"""

import os
import numpy as np
import ml_dtypes

BF = ml_dtypes.bfloat16

DIM, NCLS, SZ = 256, 19, 64
KC = 128
HW = SZ * SZ          # 4096
PD = SZ + 2           # 66
NT = 8                # 512-wide output column tiles
RT = SZ // NT         # 8 rows per tile
S_ATT = float(KC) ** -0.5


# ------------------------------------------------------------------ host prep

def _toeplitz(w, n):
    """Dense matrix of 'same'-padded 1-D cross-correlation with kernel w."""
    w = np.asarray(w, np.float64).reshape(-1)
    k = len(w)
    pad = (k - 1) // 2
    T = np.zeros((n, n), np.float64)
    for j in range(k):
        d = j - pad
        lo, hi = max(0, -d), min(n, n - d)
        idx = np.arange(lo, hi)
        T[idx, idx + d] += w[j]
    return T


def _pack_conv(w, scale=None):
    """(O, I, kh, kw) -> (128, I//128, kh*kw, O): lhsT tiles per (in-block, tap)."""
    w = np.asarray(w, np.float64)
    if scale is not None:
        w = w * np.asarray(scale, np.float64)[:, None, None, None]
    O, I, kh, kw = w.shape
    t = w.reshape(O, I, kh * kw).transpose(1, 2, 0)          # (I, taps, O)
    t = t.reshape(I // 128, 128, kh * kw, O).transpose(1, 0, 2, 3)
    return np.ascontiguousarray(t)


def _bf16(a):
    return np.ascontiguousarray(np.asarray(a, np.float32)).astype(BF)


def _f32(a):
    return np.ascontiguousarray(np.asarray(a, np.float32))


def _cols(v, nb):
    """(nb*128,) bias vector -> (128, nb): column ob = v[ob*128:(ob+1)*128]."""
    return _f32(np.asarray(v, np.float64).reshape(nb, 128).T)


def _prep(params):
    p = {k: np.asarray(v, np.float64) for k, v in params.items()}
    o = {}

    # --- fused MCAM front end: g_pre = M @ [avg; max] + d ---
    T1 = np.zeros((8 * DIM, 2 * DIM))
    b1 = np.zeros(8 * DIM)
    for br in range(2):                       # 0 = avg branch, 1 = max branch
        for kk, nm in enumerate(("mc0", "mc1", "mc2", "mc3")):
            r = (br * 4 + kk) * DIM
            T1[r:r + DIM, br * DIM:(br + 1) * DIM] = _toeplitz(p[nm + "_w"], DIM)
            b1[r:r + DIM] = p[nm + "_b"][0]
    T2 = _toeplitz(p["mcc_w"], 8 * DIM)
    bias2 = T2 @ b1 + p["mcc_b"][0]
    M = p["fc_w"] @ T2 @ T1                   # (256, 512)
    d = p["fc_w"] @ bias2 + p["fc_b"]         # (256,)
    o["wg"] = _bf16(M.T.reshape(4, 128, DIM).transpose(1, 0, 2))   # (128,4,256)
    o["dg"] = _cols(d, 2)                                          # (128,2)

    # --- mcam mid (cse1/cse2/fc1) ---
    o["wcse1"] = _bf16(_pack_conv(p["cse1_w"]))[:, :, 0, :]        # (128,2,64)
    o["bcse1"] = _f32(p["cse1_b"].reshape(64, 1))
    o["wcse2"] = _bf16(p["cse2_w"][:, :, 0, 0].T)                  # (64,19)
    o["bcse2"] = _f32(p["cse2_b"].reshape(NCLS, 1))
    o["wfc1"] = _bf16(_pack_conv(p["fc1_w"], p["fc1_s"]))[:, :, 0, :]  # (128,2,19)
    o["bfc1"] = _f32(p["fc1_t"].reshape(NCLS, 1))

    # --- conv weights, BN scale folded ---
    o["wms"] = _bf16(_pack_conv(p["ms_w"], p["ms_s"]))             # (128,2,9,256)
    o["tms"] = _cols(p["ms_t"], 2)
    o["wfq"] = _bf16(_pack_conv(p["fq_w"], p["fq_s"]))             # (128,2,9,128)
    o["tfq"] = _f32(p["fq_t"].reshape(1, 128).T)                   # (128,1)
    o["wfk"] = _bf16(_pack_conv(p["fk_w"], p["fk_s"]))[:, :, 0, :]  # (128,2,128)
    o["tfk"] = _f32(p["fk_t"].reshape(1, 128).T)
    o["wfv"] = _bf16(_pack_conv(p["fv_w"], p["fv_s"]))[:, :, 0, :]
    o["tfv"] = _f32(p["fv_t"].reshape(1, 128).T)
    o["wfup"] = _bf16(_pack_conv(p["fup_w"], p["fup_s"]))[:, 0, 0, :]  # (128,256)
    o["tfup"] = _cols(p["fup_t"], 2)
    o["wfuse"] = _bf16(_pack_conv(p["fuse_w"], p["fuse_s"]))
    o["tfuse"] = _cols(p["fuse_t"], 2)
    o["wc3"] = _bf16(_pack_conv(p["c3_w"], p["c3_s"]))
    o["tc3"] = _cols(p["c3_t"], 2)
    o["wc1"] = _bf16(_pack_conv(p["c1_w"], p["c1_s"]))[:, :, 0, :]  # (128,4,256)
    o["tc1"] = _cols(p["c1_t"], 2)

    o["ident"] = _bf16(np.eye(128))
    return o


# ------------------------------------------------------------- device program

def build_program():
    import concourse.tile as tile
    from concourse import bacc, mybir

    AF = mybir.ActivationFunctionType
    ALU = mybir.AluOpType
    F32 = mybir.dt.float32
    BF16 = mybir.dt.bfloat16

    nc = bacc.Bacc("TRN2", target_bir_lowering=False, debug=False)

    specs = [
        ("x", (DIM, HW), F32), ("skip", (DIM, HW), F32),
        ("wms", (128, 2, 9, DIM), BF16), ("tms", (128, 2), F32),
        ("wfq", (128, 2, 9, KC), BF16), ("tfq", (128, 1), F32),
        ("wg", (128, 4, DIM), BF16), ("dg", (128, 2), F32),
        ("wcse1", (128, 2, 64), BF16), ("bcse1", (64, 1), F32),
        ("wcse2", (64, NCLS), BF16), ("bcse2", (NCLS, 1), F32),
        ("wfc1", (128, 2, NCLS), BF16), ("bfc1", (NCLS, 1), F32),
        ("wfk", (128, 2, KC), BF16), ("tfk", (128, 1), F32),
        ("wfv", (128, 2, KC), BF16), ("tfv", (128, 1), F32),
        ("wfup", (128, DIM), BF16), ("tfup", (128, 2), F32),
        ("wfuse", (128, 2, 9, DIM), BF16), ("tfuse", (128, 2), F32),
        ("wc3", (128, 2, 9, DIM), BF16), ("tc3", (128, 2), F32),
        ("wc1", (128, 4, DIM), BF16), ("tc1", (128, 2), F32),
        ("ident", (128, 128), BF16),
    ]
    dram = {n: nc.dram_tensor(n, list(s), dt, kind="ExternalInput")
            for n, s, dt in specs}
    out_d = nc.dram_tensor("out", [DIM, HW], F32, kind="ExternalOutput")

    with tile.TileContext(nc) as tc:
        with tc.tile_pool(name="cst", bufs=1) as cst, \
             tc.tile_pool(name="glob", bufs=1) as glob:

            # ---- padded conv-input buffers ----
            def padded(tagname):
                ts = [glob.tile([128, PD, PD], BF16, tag=f"{tagname}{i}",
                                name=f"{tagname}{i}") for i in range(2)]
                for t in ts:
                    nc.gpsimd.memset(t[:, 0, :], 0.0)
                    nc.gpsimd.memset(t[:, PD - 1, :], 0.0)
                    nc.gpsimd.memset(t[:, 1:PD - 1, 0], 0.0)
                    nc.gpsimd.memset(t[:, 1:PD - 1, PD - 1], 0.0)
                return ts

            P1 = padded("P1")   # msam y  (ms conv input)
            P2 = padded("P2")   # sa      (fq / c3 / fuse-add input)
            P3 = padded("P3")   # fup_out + sa (fuse conv input)

            def interior(P, nt=None):
                if nt is None:
                    return P[:, 1:1 + SZ, 1:1 + SZ]
                return P[:, 1 + nt * RT:1 + (nt + 1) * RT, 1:1 + SZ]

            # small tensors that cross the early/late phase boundary
            def gt(shape, dt, nm):
                return glob.tile(list(shape), dt, tag=nm, name=nm)

            proxy_b = [gt((128, NCLS), BF16, f"proxy{i}") for i in range(2)]
            k_b = gt((128, NCLS), BF16, "k_b")
            vT_b = gt((NCLS, 128), BF16, "vT_b")
            avg_b = [gt((128, 1), BF16, f"ab{i}") for i in range(2)]
            max_b = [gt((128, 1), BF16, f"mb{i}") for i in range(2)]

            relu6 = lambda ap: nc.vector.tensor_scalar_min(ap, ap, 6.0)

            # ============ early phase: stream inputs, MSAM -> P1 ============
            W = {}

            def load_const(names):
                for n, s, dt in specs:
                    if n in ("x", "skip") or n in W or n not in names:
                        continue
                    t = cst.tile(list(s), dt, tag=n, name=f"c_{n}")
                    nc.sync.dma_start(out=t, in_=dram[n].ap())
                    W[n] = t

            # ms-conv weights must beat the input stream to the DMA queues
            load_const({"wms", "tms"})

            with tc.tile_pool(name="early", bufs=1) as early:
                xx = [early.tile([128, HW], F32, tag=f"xx{i}", name=f"xx{i}")
                      for i in range(2)]
                xw = [early.tile([128, SZ], F32, tag=f"xw{i}", name=f"xw{i}")
                      for i in range(2)]
                xhs = [early.tile([128, SZ], F32, tag=f"xhs{i}", name=f"xhs{i}")
                       for i in range(2)]
                xhp_all = [early.tile([128, SZ, NT], F32, tag=f"xhp{i}", name=f"xhp{i}")
                           for i in range(2)]
                xmaxp = [early.tile([128, NT], F32, tag=f"xmaxp{i}", name=f"xmaxp{i}")
                         for i in range(2)]

                def load_block(ib):
                    x3 = xx[ib][:].rearrange("p (h w) -> p h w", w=SZ)
                    for c in range(NT):
                        sl = slice(c * 512, (c + 1) * 512)
                        xt = early.tile([128, 512], F32, tag="xt", bufs=4,
                                        name=f"xt{ib}_{c}")
                        st = early.tile([128, 512], F32, tag="st", bufs=4,
                                        name=f"st{ib}_{c}")
                        nc.sync.dma_start(out=xt, in_=dram["x"].ap()[ib * 128:(ib + 1) * 128, sl])
                        nc.sync.dma_start(out=st, in_=dram["skip"].ap()[ib * 128:(ib + 1) * 128, sl])
                        nc.gpsimd.tensor_add(xx[ib][:, sl], xt, st)
                        ch3 = x3[:, c * RT:(c + 1) * RT, :]
                        nc.vector.tensor_reduce(out=xw[ib][:, c * RT:(c + 1) * RT],
                                                in_=ch3, axis=mybir.AxisListType.X,
                                                op=ALU.add)
                        # column-sum partial for x_h: reduce the chunk's 8 rows
                        ch3t = xx[ib][:, sl].rearrange("p (h w) -> p w h", w=SZ)
                        nc.vector.tensor_reduce(out=xhp_all[ib][:, :, c], in_=ch3t,
                                                axis=mybir.AxisListType.X, op=ALU.add)

                def msam_block(ib):
                    x3 = xx[ib][:].rearrange("p (h w) -> p h w", w=SZ)
                    xh = xhs[ib]
                    nc.vector.tensor_reduce(out=xh, in_=xhp_all[ib][:], 
                                            axis=mybir.AxisListType.X, op=ALU.add)
                    qk = early.tile([128, SZ, SZ], BF16, tag="qk", name=f"qk{ib}")
                    Ee = early.tile([128, SZ, SZ], F32, tag="Ee", name=f"Ee{ib}")
                    sqh = [early.tile([128, 1], F32, tag=f"sqh{h}", name=f"sq{ib}_{h}")
                           for h in range(2)]
                    for h in range(2):
                        rows = slice(h * (SZ // 2), (h + 1) * (SZ // 2))
                        nc.vector.scalar_tensor_tensor(
                            out=qk[:, rows, :],
                            in0=xw[ib][:, rows].unsqueeze(2).broadcast_to([128, SZ // 2, SZ]),
                            scalar=1.0 / (HW * 1.0),
                            in1=xh[:].unsqueeze(1).broadcast_to([128, SZ // 2, SZ]),
                            op0=ALU.mult, op1=ALU.mult)
                        nc.scalar.activation(out=Ee[:, rows, :], in_=qk[:, rows, :],
                                             func=AF.Exp, bias=0.0, scale=1.0,
                                             accum_out=sqh[h][:, 0:1])
                    sq = early.tile([128, 1], F32, tag=f"sq{ib}", name=f"sq{ib}")
                    nc.vector.tensor_add(sq, sqh[0], sqh[1])
                    rq = early.tile([128, 1], F32, tag=f"rq{ib}", name=f"rq{ib}")
                    nc.vector.reciprocal(rq, sq)
                    for c in range(NT):
                        Ech = Ee[:, c * RT:(c + 1) * RT, :]
                        nc.vector.tensor_scalar(out=Ech, in0=Ech, scalar1=rq[:, 0:1],
                                                scalar2=1.0, op0=ALU.mult, op1=ALU.add)
                        nc.gpsimd.tensor_tensor(out=interior(P1[ib], c), in0=Ech,
                                                in1=x3[:, c * RT:(c + 1) * RT, :],
                                                op=ALU.mult)

                def stats_block(ib):
                    x3 = xx[ib][:].rearrange("p (h w) -> p h w", w=SZ)
                    for c in range(NT):
                        nc.vector.tensor_reduce(out=xmaxp[ib][:, c:c + 1],
                                                in_=x3[:, c * RT:(c + 1) * RT, :],
                                                axis=mybir.AxisListType.XY, op=ALU.max)
                    ssum = early.tile([128, 1], F32, tag=f"ssum{ib}", name=f"ssum{ib}")
                    smax = early.tile([128, 1], F32, tag=f"smax{ib}", name=f"smax{ib}")
                    nc.vector.tensor_reduce(out=ssum, in_=xw[ib][:],
                                            axis=mybir.AxisListType.X, op=ALU.add)
                    nc.vector.tensor_reduce(out=smax, in_=xmaxp[ib][:],
                                            axis=mybir.AxisListType.X, op=ALU.max)
                    nc.scalar.activation(out=avg_b[ib], in_=ssum, func=AF.Copy,
                                         scale=1.0 / HW)
                    nc.scalar.activation(out=max_b[ib], in_=smax, func=AF.Copy)

                load_block(0)
                msam_block(0)
                load_block(1)
                msam_block(1)
                stats_block(0)
                stats_block(1)

                # remaining constants (queued behind the input loads)
                load_const({n for n, _, _ in specs})

            # =============== late phase: convs + mcam + attention ===========
            with tc.tile_pool(name="late", bufs=1) as late, \
                 tc.tile_pool(name="stg", bufs=1) as stg, \
                 tc.tile_pool(name="psp", bufs=8, space="PSUM") as psp:

                def pst(name):
                    return psp.tile([128, 512], F32, tag="ps", name=name)

                def psq(name, shape, dt=None):
                    return psp.tile(list(shape), dt or F32, tag="ps", name=name)

                def conv3x3(Pin, wname, n_ob, emit, ib_split=False, filler=None):
                    for ob in range(n_ob):
                        pslist = [pst(f"ps_{wname}_{ob}_{nt}") for nt in range(NT)]
                        ib_order = ((0, 1),) if not ib_split else ((0,), (1,))
                        for ibs in ib_order:
                            for nt in range(NT):
                                y0 = nt * RT
                                for ib in ibs:
                                    for t9 in range(9):
                                        ky, kx = divmod(t9, 3)
                                        nc.tensor.matmul(
                                            pslist[nt][:],
                                            lhsT=W[wname][:, ib, t9, ob * 128:(ob + 1) * 128],
                                            rhs=Pin[ib][:, y0 + ky:y0 + ky + RT, kx:kx + SZ],
                                            start=(ib == 0 and t9 == 0),
                                            stop=(ib == 1 and t9 == 8))
                                if ibs[-1] == 1:
                                    emit(ob, nt, pslist[nt])
                                if filler is not None:
                                    for step in (filler.pop(0) for _ in range(2) if filler):
                                        step()

                def emit_ms(ob, nt, ps):
                    pv = interior(P2[ob], nt)
                    nc.scalar.activation(out=pv, in_=ps[:], func=AF.Relu,
                                         bias=W["tms"][:, ob:ob + 1], scale=1.0)
                    relu6(pv)
                conv3x3(P1, "wms", 2, emit_ms, ib_split=True)

                # ---- MCAM chain as filler steps (interleaved into fq conv) ----
                vblocks = [avg_b[0], avg_b[1], max_b[0], max_b[1]]
                g_colb = [late.tile([128, 1], BF16, tag=f"gc{i}", name=f"gc{i}")
                          for i in range(2)]
                g_row = late.tile([1, DIM], F32, tag="g_row", name="g_row")
                h_b = late.tile([64, 1], BF16, tag="h_b", name="h_b")
                y1c = late.tile([NCLS, 1], BF16, tag="y1c", name="y1c")
                f1c = late.tile([NCLS, 1], BF16, tag="f1c", name="f1c")
                rowt = {nm: late.tile([1, NCLS], F32, tag=f"row_{nm}", name=f"row_{nm}")
                        for nm in ("f1", "y1")}
                cfr = late.tile([1, NCLS], BF16, tag="cfr", name="cfr")
                cfc = late.tile([NCLS, 1], F32, tag="cfc", name="cfc")
                g19 = late.tile([NCLS, DIM], F32, tag="g19", name="g19")
                cmT = late.tile([NCLS, DIM], F32, tag="cmT", name="cmT")
                cme = late.tile([NCLS, DIM], BF16, tag="cme", name="cme")
                v_b = late.tile([128, NCLS], BF16, tag="v_b", name="v_b")
                steps = []

                def s_g(ob):
                    def f():
                        pg = psq(f"psg{ob}", (128, 1))
                        for j in range(4):
                            nc.tensor.matmul(pg[:], lhsT=W["wg"][:, j, ob * 128:(ob + 1) * 128],
                                             rhs=vblocks[j], start=(j == 0), stop=(j == 3))
                        nc.scalar.activation(out=g_colb[ob], in_=pg[:], func=AF.Sigmoid,
                                             bias=W["dg"][:, ob:ob + 1], scale=1.0)
                        pr = psq(f"psgr{ob}", (1, 128), BF16)
                        nc.tensor.transpose(pr[:], g_colb[ob][:], W["ident"][:, :])
                        nc.scalar.activation(out=g_row[:, ob * 128:(ob + 1) * 128],
                                             in_=pr[:], func=AF.Copy)
                    return f
                steps += [s_g(0), s_g(1)]

                def s_h():
                    ph = psq("psh", (64, 1))
                    for ib in range(2):
                        nc.tensor.matmul(ph[:], lhsT=W["wcse1"][:, ib, :], rhs=g_colb[ib],
                                         start=(ib == 0), stop=(ib == 1))
                    nc.scalar.activation(out=h_b, in_=ph[:], func=AF.Relu,
                                         bias=W["bcse1"][:, 0:1], scale=1.0)
                steps.append(s_h)

                def s_y1():
                    py = psq("psy", (NCLS, 1))
                    nc.tensor.matmul(py[:], lhsT=W["wcse2"][:, :], rhs=h_b,
                                     start=True, stop=True)
                    nc.scalar.activation(out=y1c, in_=py[:], func=AF.Sigmoid,
                                         bias=W["bcse2"][:, 0:1], scale=1.0)
                steps.append(s_y1)

                def s_f1():
                    pf = psq("psf", (NCLS, 1))
                    for ib in range(2):
                        nc.tensor.matmul(pf[:], lhsT=W["wfc1"][:, ib, :], rhs=g_colb[ib],
                                         start=(ib == 0), stop=(ib == 1))
                    nc.scalar.activation(out=f1c, in_=pf[:], func=AF.Identity,
                                         bias=W["bfc1"][:, 0:1], scale=1.0)
                steps.append(s_f1)

                def s_row(nm, src_t):
                    def f():
                        pr = psq(f"pr_{nm}", (1, NCLS), BF16)
                        nc.tensor.transpose(pr[:], src_t[:], W["ident"][0:NCLS, 0:NCLS])
                        nc.scalar.activation(out=rowt[nm], in_=pr[:], func=AF.Copy)
                    return f
                steps += [s_row("f1", f1c), s_row("y1", y1c)]

                def s_sm1():
                    f1r, y1r = rowt["f1"], rowt["y1"]
                    s1 = late.tile([1, 1], F32, tag="s1", name="s1")
                    e1 = late.tile([1, NCLS], F32, tag="e1", name="e1")
                    nc.scalar.activation(out=e1, in_=f1r[:], func=AF.Exp,
                                         bias=0.0, scale=1.0, accum_out=s1[:, 0:1])
                    r1 = late.tile([1, 1], F32, tag="r1", name="r1")
                    nc.vector.reciprocal(r1, s1)
                    nc.vector.tensor_scalar(out=e1, in0=e1, scalar1=r1[:, 0:1],
                                            scalar2=None, op0=ALU.mult)
                    nc.vector.tensor_add(cfr, e1, y1r)
                steps.append(s_sm1)

                def s_cfc():
                    pcx = psq("pc_cf", (NCLS, 1), BF16)
                    nc.tensor.transpose(pcx[:], cfr[:], W["ident"][0:1, 0:1])
                    nc.scalar.activation(out=cfc, in_=pcx[:], func=AF.Copy)
                    nc.gpsimd.partition_broadcast(g19[:], g_row[:])
                steps.append(s_cfc)

                def s_cm():
                    nc.vector.tensor_scalar(out=cmT, in0=g19, scalar1=cfc[:, 0:1],
                                            scalar2=None, op0=ALU.mult)
                    s2 = late.tile([NCLS, 1], F32, tag="s2", name="s2")
                    nc.scalar.activation(out=cme, in_=cmT[:], func=AF.Exp,
                                         bias=0.0, scale=1.0, accum_out=s2[:, 0:1])
                    r2 = late.tile([NCLS, 1], F32, tag="r2", name="r2")
                    nc.vector.reciprocal(r2, s2)
                    nc.vector.tensor_scalar(out=cme, in0=cme, scalar1=r2[:, 0:1],
                                            scalar2=None, op0=ALU.mult)
                steps.append(s_cm)

                def s_half(ib):
                    def f():
                        pt = psq(f"pt{ib}", (128, NCLS), BF16)
                        nc.tensor.transpose(pt[:], cme[:, ib * 128:(ib + 1) * 128],
                                            W["ident"][0:NCLS, 0:NCLS])
                        pp = late.tile([128, NCLS], F32, tag=f"pp{ib}", name=f"pp{ib}")
                        nc.scalar.activation(out=pp, in_=pt[:], func=AF.Copy)
                        s3 = late.tile([128, 1], F32, tag=f"s3_{ib}", name=f"s3_{ib}")
                        nc.scalar.activation(out=proxy_b[ib], in_=pp[:], func=AF.Exp,
                                             bias=0.0, scale=1.0, accum_out=s3[:, 0:1])
                        r3 = late.tile([128, 1], F32, tag=f"r3_{ib}", name=f"r3_{ib}")
                        nc.vector.reciprocal(r3, s3)
                        nc.vector.tensor_scalar(out=proxy_b[ib], in0=proxy_b[ib],
                                                scalar1=r3[:, 0:1], scalar2=None, op0=ALU.mult)
                    return f
                steps += [s_half(0), s_half(1)]

                def s_kv(wname, tname, dst):
                    def f():
                        pkv = psq(f"pkv_{wname}", (128, NCLS))
                        for ib in range(2):
                            nc.tensor.matmul(pkv[:], lhsT=W[wname][:, ib, :],
                                             rhs=proxy_b[ib], start=(ib == 0), stop=(ib == 1))
                        nc.scalar.activation(out=dst, in_=pkv[:], func=AF.Relu,
                                             bias=W[tname][:, 0:1], scale=1.0)
                        relu6(dst[:])
                    return f
                steps += [s_kv("wfk", "tfk", k_b), s_kv("wfv", "tfv", v_b)]

                def s_vT():
                    pvT = psq("pvT", (NCLS, 128), BF16)
                    nc.tensor.transpose(pvT[:], v_b[:], W["ident"][:, :])
                    nc.scalar.activation(out=vT_b, in_=pvT[:], func=AF.Copy)
                steps.append(s_vT)

                # ---- fq conv: P2 -> q ----
                q_b = late.tile([128, HW], BF16, tag="q_b", name="q_b")
                def emit_fq(ob, nt, ps):
                    pv = q_b[:, nt * 512:(nt + 1) * 512]
                    nc.scalar.activation(out=pv, in_=ps[:], func=AF.Relu,
                                         bias=W["tfq"][:, 0:1], scale=1.0)
                    relu6(pv)
                conv3x3(P2, "wfq", 1, emit_fq, filler=steps)
                while steps:
                    steps.pop(0)()

                # attention mm1: (19, 4096) = k^T q
                mm_b = late.tile([NCLS, HW], BF16, tag="mm_b", name="mm_b")
                for nt in range(NT):
                    pm = psp.tile([NCLS, 512], F32, tag="ps", name=f"pmm{nt}")
                    nc.tensor.matmul(pm[:], lhsT=k_b[:], rhs=q_b[:, nt * 512:(nt + 1) * 512],
                                     start=True, stop=True)
                    nc.scalar.activation(out=mm_b[:, nt * 512:(nt + 1) * 512],
                                         in_=pm[:], func=AF.Copy)

                # c3 conv traced now so PE stays busy during the softmax
                sp = [late.tile([128, HW], BF16, tag=f"sp{i}", name=f"sp{i}")
                      for i in range(2)]
                def emit_c3(ob, nt, ps):
                    pv = sp[ob][:, nt * 512:(nt + 1) * 512]
                    nc.scalar.activation(out=pv, in_=ps[:], func=AF.Relu,
                                         bias=W["tc3"][:, ob:ob + 1], scale=1.0)
                    relu6(pv)
                conv3x3(P2, "wc3", 2, emit_c3)

                # softmax over hw rows of mm (inputs are bounded: no max-sub needed)
                sa_s = late.tile([NCLS, 1], F32, tag="sa_s", name="sa_s")
                A_b = late.tile([NCLS, HW], BF16, tag="A_b", name="A_b")
                nc.scalar.activation(out=A_b, in_=mm_b[:], func=AF.Exp,
                                     bias=0.0, scale=S_ATT, accum_out=sa_s[:, 0:1])
                ra = late.tile([NCLS, 1], F32, tag="ra", name="ra")
                nc.vector.reciprocal(ra, sa_s)
                nc.vector.tensor_scalar(out=A_b, in0=A_b, scalar1=ra[:, 0:1],
                                        scalar2=None, op0=ALU.mult)

                # mm2 + fup + add sa -> P3
                ctx_b = late.tile([128, HW], BF16, tag="ctx_b", name="ctx_b")
                for nt in range(NT):
                    pc2 = pst(f"pctx{nt}")
                    nc.tensor.matmul(pc2[:], lhsT=vT_b[:], rhs=A_b[:, nt * 512:(nt + 1) * 512],
                                     start=True, stop=True)
                    nc.scalar.activation(out=ctx_b[:, nt * 512:(nt + 1) * 512],
                                         in_=pc2[:], func=AF.Copy)
                for ob in range(2):
                    for nt in range(NT):
                        pu = pst(f"pfup{ob}_{nt}")
                        nc.tensor.matmul(pu[:], lhsT=W["wfup"][:, ob * 128:(ob + 1) * 128],
                                         rhs=ctx_b[:, nt * 512:(nt + 1) * 512],
                                         start=True, stop=True)
                        fs = stg.tile([128, 512], BF16, tag="fs", bufs=3,
                                      name=f"fs{ob}_{nt}")
                        nc.scalar.activation(out=fs, in_=pu[:], func=AF.Relu,
                                             bias=W["tfup"][:, ob:ob + 1], scale=1.0)
                        nc.vector.scalar_tensor_tensor(
                            out=interior(P3[ob], nt), in0=fs, scalar=6.0,
                            in1=interior(P2[ob], nt), op0=ALU.min, op1=ALU.add)

                # fuse conv: P3 -> ctxf
                ctxf = [late.tile([128, HW], BF16, tag=f"ctxf{i}", name=f"ctxf{i}")
                        for i in range(2)]
                def emit_fuse(ob, nt, ps):
                    pv = ctxf[ob][:, nt * 512:(nt + 1) * 512]
                    nc.scalar.activation(out=pv, in_=ps[:], func=AF.Relu,
                                         bias=W["tfuse"][:, ob:ob + 1], scale=1.0)
                    relu6(pv)
                conv3x3(P3, "wfuse", 2, emit_fuse)

                # c1 1x1 over concat([ctxf, sp]) -> out
                cat = [ctxf[0], ctxf[1], sp[0], sp[1]]
                for ob in range(2):
                    for nt in range(NT):
                        ps = pst(f"pc1_{ob}_{nt}")
                        for j in range(4):
                            nc.tensor.matmul(ps[:], lhsT=W["wc1"][:, j, ob * 128:(ob + 1) * 128],
                                             rhs=cat[j][:, nt * 512:(nt + 1) * 512],
                                             start=(j == 0), stop=(j == 3))
                        og = stg.tile([128, 512], F32, tag="og", bufs=4,
                                      name=f"og{ob}_{nt}")
                        nc.scalar.activation(out=og, in_=ps[:], func=AF.Relu,
                                             bias=W["tc1"][:, ob:ob + 1], scale=1.0)
                        relu6(og[:])
                        nc.sync.dma_start(
                            out=out_d.ap()[ob * 128:(ob + 1) * 128, nt * 512:(nt + 1) * 512],
                            in_=og)

    nc.compile()
    return nc


# ------------------------------------------------------------------- wrapper

_CACHE = {}


def kernel(x, skip, params):
    from concourse import bass_utils

    x = np.asarray(x, np.float32)
    skip = np.asarray(skip, np.float32)
    B = x.shape[0]
    packed = _prep(params)

    if "nc" not in _CACHE:
        _CACHE["nc"] = build_program()
    nc = _CACHE["nc"]

    in_maps = []
    for i in range(B):
        m = dict(packed)
        m["x"] = np.ascontiguousarray(x[i].reshape(DIM, HW))
        m["skip"] = np.ascontiguousarray(skip[i].reshape(DIM, HW))
        in_maps.append(m)

    res = bass_utils.run_bass_kernel_spmd(
        nc, in_maps, core_ids=list(range(B)),
        trace=bool(int(os.environ.get("KBENCH_TRACE", "0"))))
    _CACHE["last_result"] = res
    out = np.stack([r["out"].reshape(DIM, SZ, SZ) for r in res.results])
    return out.astype(np.float32)


# revision 18
# speedup vs baseline: 17.0282x; 6.0519x over previous
"""Trainium2 Bass kernel for nn_Attention (MCAM + MSAM + CIAFM block).

Sharding: pure data parallelism — B=8 samples across 8 NeuronCores.
Per core: x,skip (256, 64, 64) f32 -> out (256, 64, 64) f32.

Heavy compute = four 3x3 convs (ms, fq, fuse, c3) done as 9-tap
PSUM-accumulated bf16 matmuls over zero-padded (128, 66, 66) SBUF tiles,
plus 1x1 convs (fup, c1) and a tiny NC=19 cross-attention.

Host-side preprocessing folds:
  - BN scales into conv weights (cbr -> relu6(conv(x, W*s) + t))
  - the entire MCAM front end (4 ECA conv1ds + k=3 mixer + FC) into one
    linear map  g_pre = M @ [avgpool; maxpool] + d   (M: 256x512)

Schedule: inputs stream in per 512-column chunk (adds on GPSIMD, partial
pool stats on DVE during the DMA window); MSAM softmax+modulation feeds
the ms conv per chunk; the ms conv is split by input channel block so its
block-0 matmuls start before block-1's modulated input is finished. The
serial MCAM chain (tiny) is traced between ms and fq so it never blocks
the PE FIFO; c3 is traced before the attention softmax for the same
reason.
"""

import os
import numpy as np
import ml_dtypes

BF = ml_dtypes.bfloat16

DIM, NCLS, SZ = 256, 19, 64
KC = 128
HW = SZ * SZ          # 4096
PD = SZ + 2           # 66
NT = 8                # 512-wide output column tiles
RT = SZ // NT         # 8 rows per tile
S_ATT = float(KC) ** -0.5


# ------------------------------------------------------------------ host prep

def _toeplitz(w, n):
    """Dense matrix of 'same'-padded 1-D cross-correlation with kernel w."""
    w = np.asarray(w, np.float64).reshape(-1)
    k = len(w)
    pad = (k - 1) // 2
    T = np.zeros((n, n), np.float64)
    for j in range(k):
        d = j - pad
        lo, hi = max(0, -d), min(n, n - d)
        idx = np.arange(lo, hi)
        T[idx, idx + d] += w[j]
    return T


def _pack_conv(w, scale=None):
    """(O, I, kh, kw) -> (128, I//128, kh*kw, O): lhsT tiles per (in-block, tap)."""
    w = np.asarray(w, np.float64)
    if scale is not None:
        w = w * np.asarray(scale, np.float64)[:, None, None, None]
    O, I, kh, kw = w.shape
    t = w.reshape(O, I, kh * kw).transpose(1, 2, 0)          # (I, taps, O)
    t = t.reshape(I // 128, 128, kh * kw, O).transpose(1, 0, 2, 3)
    return np.ascontiguousarray(t)


def _bf16(a):
    return np.ascontiguousarray(np.asarray(a, np.float32)).astype(BF)


def _f32(a):
    return np.ascontiguousarray(np.asarray(a, np.float32))


def _cols(v, nb):
    """(nb*128,) bias vector -> (128, nb): column ob = v[ob*128:(ob+1)*128]."""
    return _f32(np.asarray(v, np.float64).reshape(nb, 128).T)


def _prep(params):
    p = {k: np.asarray(v, np.float64) for k, v in params.items()}
    o = {}

    # --- fused MCAM front end: g_pre = M @ [avg; max] + d ---
    T1 = np.zeros((8 * DIM, 2 * DIM))
    b1 = np.zeros(8 * DIM)
    for br in range(2):                       # 0 = avg branch, 1 = max branch
        for kk, nm in enumerate(("mc0", "mc1", "mc2", "mc3")):
            r = (br * 4 + kk) * DIM
            T1[r:r + DIM, br * DIM:(br + 1) * DIM] = _toeplitz(p[nm + "_w"], DIM)
            b1[r:r + DIM] = p[nm + "_b"][0]
    T2 = _toeplitz(p["mcc_w"], 8 * DIM)
    bias2 = T2 @ b1 + p["mcc_b"][0]
    M = p["fc_w"] @ T2 @ T1                   # (256, 512)
    d = p["fc_w"] @ bias2 + p["fc_b"]         # (256,)
    o["wg"] = _bf16(M.T.reshape(4, 128, DIM).transpose(1, 0, 2))   # (128,4,256)
    o["dg"] = _cols(d, 2)                                          # (128,2)

    # --- mcam mid (cse1/cse2/fc1) ---
    o["wcse1"] = _bf16(_pack_conv(p["cse1_w"]))[:, :, 0, :]        # (128,2,64)
    o["bcse1"] = _f32(p["cse1_b"].reshape(64, 1))
    o["wcse2"] = _bf16(p["cse2_w"][:, :, 0, 0].T)                  # (64,19)
    o["bcse2"] = _f32(p["cse2_b"].reshape(NCLS, 1))
    o["wfc1"] = _bf16(_pack_conv(p["fc1_w"], p["fc1_s"]))[:, :, 0, :]  # (128,2,19)
    o["bfc1"] = _f32(p["fc1_t"].reshape(NCLS, 1))

    # --- conv weights, BN scale folded ---
    o["wms"] = _bf16(_pack_conv(p["ms_w"], p["ms_s"]))             # (128,2,9,256)
    o["tms"] = _cols(p["ms_t"], 2)
    o["wfq"] = _bf16(_pack_conv(p["fq_w"], p["fq_s"]))             # (128,2,9,128)
    o["tfq"] = _f32(p["fq_t"].reshape(1, 128).T)                   # (128,1)
    o["wfk"] = _bf16(_pack_conv(p["fk_w"], p["fk_s"]))[:, :, 0, :]  # (128,2,128)
    o["tfk"] = _f32(p["fk_t"].reshape(1, 128).T)
    o["wfv"] = _bf16(_pack_conv(p["fv_w"], p["fv_s"]))[:, :, 0, :]
    o["tfv"] = _f32(p["fv_t"].reshape(1, 128).T)
    o["wfup"] = _bf16(_pack_conv(p["fup_w"], p["fup_s"]))[:, 0, 0, :]  # (128,256)
    o["tfup"] = _cols(p["fup_t"], 2)
    o["wfuse"] = _bf16(_pack_conv(p["fuse_w"], p["fuse_s"]))
    o["tfuse"] = _cols(p["fuse_t"], 2)
    o["wc3"] = _bf16(_pack_conv(p["c3_w"], p["c3_s"]))
    o["tc3"] = _cols(p["c3_t"], 2)
    o["wc1"] = _bf16(_pack_conv(p["c1_w"], p["c1_s"]))[:, :, 0, :]  # (128,4,256)
    o["tc1"] = _cols(p["c1_t"], 2)

    o["ident"] = _bf16(np.eye(128))
    return o


# ------------------------------------------------------------- device program

def build_program():
    import concourse.tile as tile
    from concourse import bacc, mybir

    AF = mybir.ActivationFunctionType
    ALU = mybir.AluOpType
    F32 = mybir.dt.float32
    BF16 = mybir.dt.bfloat16

    nc = bacc.Bacc("TRN2", target_bir_lowering=False, debug=False)

    specs = [
        ("x", (DIM, HW), F32), ("skip", (DIM, HW), F32),
        ("wms", (128, 2, 9, DIM), BF16), ("tms", (128, 2), F32),
        ("wfq", (128, 2, 9, KC), BF16), ("tfq", (128, 1), F32),
        ("wg", (128, 4, DIM), BF16), ("dg", (128, 2), F32),
        ("wcse1", (128, 2, 64), BF16), ("bcse1", (64, 1), F32),
        ("wcse2", (64, NCLS), BF16), ("bcse2", (NCLS, 1), F32),
        ("wfc1", (128, 2, NCLS), BF16), ("bfc1", (NCLS, 1), F32),
        ("wfk", (128, 2, KC), BF16), ("tfk", (128, 1), F32),
        ("wfv", (128, 2, KC), BF16), ("tfv", (128, 1), F32),
        ("wfup", (128, DIM), BF16), ("tfup", (128, 2), F32),
        ("wfuse", (128, 2, 9, DIM), BF16), ("tfuse", (128, 2), F32),
        ("wc3", (128, 2, 9, DIM), BF16), ("tc3", (128, 2), F32),
        ("wc1", (128, 4, DIM), BF16), ("tc1", (128, 2), F32),
        ("ident", (128, 128), BF16),
    ]
    dram = {n: nc.dram_tensor(n, list(s), dt, kind="ExternalInput")
            for n, s, dt in specs}
    out_d = nc.dram_tensor("out", [DIM, HW], F32, kind="ExternalOutput")

    with tile.TileContext(nc) as tc:
        with tc.tile_pool(name="cst", bufs=1) as cst, \
             tc.tile_pool(name="glob", bufs=1) as glob:

            # ---- padded conv-input buffers ----
            def padded(tagname):
                ts = [glob.tile([128, PD, PD], BF16, tag=f"{tagname}{i}",
                                name=f"{tagname}{i}") for i in range(2)]
                for t in ts:
                    nc.gpsimd.memset(t[:, 0, :], 0.0)
                    nc.gpsimd.memset(t[:, PD - 1, :], 0.0)
                    nc.gpsimd.memset(t[:, 1:PD - 1, 0], 0.0)
                    nc.gpsimd.memset(t[:, 1:PD - 1, PD - 1], 0.0)
                return ts

            P1 = padded("P1")   # msam y  (ms conv input)
            P2 = padded("P2")   # sa      (fq / c3 / fuse-add input)
            P3 = padded("P3")   # fup_out + sa (fuse conv input)

            def interior(P, nt=None):
                if nt is None:
                    return P[:, 1:1 + SZ, 1:1 + SZ]
                return P[:, 1 + nt * RT:1 + (nt + 1) * RT, 1:1 + SZ]

            # small tensors that cross the early/late phase boundary
            def gt(shape, dt, nm):
                return glob.tile(list(shape), dt, tag=nm, name=nm)

            proxy_b = [gt((128, NCLS), BF16, f"proxy{i}") for i in range(2)]
            k_b = gt((128, NCLS), BF16, "k_b")
            vT_b = gt((NCLS, 128), BF16, "vT_b")
            avg_b = [gt((128, 1), BF16, f"ab{i}") for i in range(2)]
            max_b = [gt((128, 1), BF16, f"mb{i}") for i in range(2)]

            relu6 = lambda ap: nc.vector.tensor_scalar_min(ap, ap, 6.0)

            # ============ early phase: stream inputs, MSAM -> P1 ============
            W = {}

            def load_const(names):
                for n, s, dt in specs:
                    if n in ("x", "skip") or n in W or n not in names:
                        continue
                    t = cst.tile(list(s), dt, tag=n, name=f"c_{n}")
                    nc.sync.dma_start(out=t, in_=dram[n].ap())
                    W[n] = t

            # ms-conv weights must beat the input stream to the DMA queues
            load_const({"wms", "tms"})

            with tc.tile_pool(name="early", bufs=1) as early:
                xx = [early.tile([128, HW], F32, tag=f"xx{i}", name=f"xx{i}")
                      for i in range(2)]
                xw = [early.tile([128, SZ], F32, tag=f"xw{i}", name=f"xw{i}")
                      for i in range(2)]
                xhs = [early.tile([128, SZ], F32, tag=f"xhs{i}", name=f"xhs{i}")
                       for i in range(2)]
                xhp_all = [early.tile([128, SZ, NT], F32, tag=f"xhp{i}", name=f"xhp{i}")
                           for i in range(2)]
                xmaxp = [early.tile([128, NT], F32, tag=f"xmaxp{i}", name=f"xmaxp{i}")
                         for i in range(2)]

                def load_block(ib):
                    x3 = xx[ib][:].rearrange("p (h w) -> p h w", w=SZ)
                    for c in range(NT):
                        sl = slice(c * 512, (c + 1) * 512)
                        xt = early.tile([128, 512], F32, tag="xt", bufs=6,
                                        name=f"xt{ib}_{c}")
                        st = early.tile([128, 512], F32, tag="st", bufs=6,
                                        name=f"st{ib}_{c}")
                        nc.sync.dma_start(out=xt, in_=dram["x"].ap()[ib * 128:(ib + 1) * 128, sl])
                        nc.sync.dma_start(out=st, in_=dram["skip"].ap()[ib * 128:(ib + 1) * 128, sl])
                        nc.gpsimd.tensor_add(xx[ib][:, sl], xt, st)
                        ch3 = x3[:, c * RT:(c + 1) * RT, :]
                        nc.vector.tensor_reduce(out=xw[ib][:, c * RT:(c + 1) * RT],
                                                in_=ch3, axis=mybir.AxisListType.X,
                                                op=ALU.add)
                        # column-sum partial for x_h: reduce the chunk's 8 rows
                        ch3t = xx[ib][:, sl].rearrange("p (h w) -> p w h", w=SZ)
                        nc.vector.tensor_reduce(out=xhp_all[ib][:, :, c], in_=ch3t,
                                                axis=mybir.AxisListType.X, op=ALU.add)
                        nc.vector.tensor_reduce(out=xmaxp[ib][:, c:c + 1], in_=ch3,
                                                axis=mybir.AxisListType.XY, op=ALU.max)

                def msam_block(ib):
                    x3 = xx[ib][:].rearrange("p (h w) -> p h w", w=SZ)
                    xh = xhs[ib]
                    nc.vector.tensor_reduce(out=xh, in_=xhp_all[ib][:], 
                                            axis=mybir.AxisListType.X, op=ALU.add)
                    qk = early.tile([128, SZ, SZ], BF16, tag="qk", name=f"qk{ib}")
                    Ee = early.tile([128, SZ, SZ], F32, tag="Ee", name=f"Ee{ib}")
                    sqh = [early.tile([128, 1], F32, tag=f"sqh{h}", name=f"sq{ib}_{h}")
                           for h in range(2)]
                    for h in range(2):
                        rows = slice(h * (SZ // 2), (h + 1) * (SZ // 2))
                        nc.vector.scalar_tensor_tensor(
                            out=qk[:, rows, :],
                            in0=xw[ib][:, rows].unsqueeze(2).broadcast_to([128, SZ // 2, SZ]),
                            scalar=1.0 / (HW * 1.0),
                            in1=xh[:].unsqueeze(1).broadcast_to([128, SZ // 2, SZ]),
                            op0=ALU.mult, op1=ALU.mult)
                        nc.scalar.activation(out=Ee[:, rows, :], in_=qk[:, rows, :],
                                             func=AF.Exp, bias=0.0, scale=1.0,
                                             accum_out=sqh[h][:, 0:1])
                    sq = early.tile([128, 1], F32, tag=f"sq{ib}", name=f"sq{ib}")
                    nc.vector.tensor_add(sq, sqh[0], sqh[1])
                    rq = early.tile([128, 1], F32, tag=f"rq{ib}", name=f"rq{ib}")
                    nc.vector.reciprocal(rq, sq)
                    for c in range(NT):
                        Ech = Ee[:, c * RT:(c + 1) * RT, :]
                        nc.vector.tensor_scalar(out=Ech, in0=Ech, scalar1=rq[:, 0:1],
                                                scalar2=1.0, op0=ALU.mult, op1=ALU.add)
                        nc.gpsimd.tensor_tensor(out=interior(P1[ib], c), in0=Ech,
                                                in1=x3[:, c * RT:(c + 1) * RT, :],
                                                op=ALU.mult)

                def stats_block(ib):
                    ssum = early.tile([128, 1], F32, tag=f"ssum{ib}", name=f"ssum{ib}")
                    smax = early.tile([128, 1], F32, tag=f"smax{ib}", name=f"smax{ib}")
                    nc.vector.tensor_reduce(out=ssum, in_=xw[ib][:],
                                            axis=mybir.AxisListType.X, op=ALU.add)
                    nc.vector.tensor_reduce(out=smax, in_=xmaxp[ib][:],
                                            axis=mybir.AxisListType.X, op=ALU.max)
                    nc.scalar.activation(out=avg_b[ib], in_=ssum, func=AF.Copy,
                                         scale=1.0 / HW)
                    nc.scalar.activation(out=max_b[ib], in_=smax, func=AF.Copy)

                load_block(0)
                msam_block(0)
                load_block(1)
                msam_block(1)
                stats_block(0)
                stats_block(1)

                # remaining constants (queued behind the input loads)
                load_const({n for n, _, _ in specs})

            # =============== late phase: convs + mcam + attention ===========
            with tc.tile_pool(name="late", bufs=1) as late, \
                 tc.tile_pool(name="stg", bufs=1) as stg, \
                 tc.tile_pool(name="psp", bufs=8, space="PSUM") as psp:

                def pst(name):
                    return psp.tile([128, 512], F32, tag="ps", name=name)

                def psq(name, shape, dt=None):
                    return psp.tile(list(shape), dt or F32, tag="ps", name=name)

                TAPS_OUTER = bool(int(os.environ.get("KERNEL_TAPS_OUTER", "0")))

                def conv3x3(Pin, wname, n_ob, emit, ib_split=False, filler=None):
                    for ob in range(n_ob):
                        pslist = [pst(f"ps_{wname}_{ob}_{nt}") for nt in range(NT)]
                        ib_order = ((0, 1),) if not ib_split else ((0,), (1,))
                        if TAPS_OUTER:
                            # same stationary weight across all 8 n-tiles:
                            # 8x fewer LDWEIGHTS switches on the PE
                            for ibs in ib_order:
                                for ib in ibs:
                                    for t9 in range(9):
                                        ky, kx = divmod(t9, 3)
                                        lhsT = W[wname][:, ib, t9, ob * 128:(ob + 1) * 128]
                                        for nt in range(NT):
                                            y0 = nt * RT
                                            nc.tensor.matmul(
                                                pslist[nt][:], lhsT=lhsT,
                                                rhs=Pin[ib][:, y0 + ky:y0 + ky + RT, kx:kx + SZ],
                                                start=(ib == 0 and t9 == 0),
                                                stop=(ib == 1 and t9 == 8))
                                            if ib == 1 and t9 == 8:
                                                emit(ob, nt, pslist[nt])
                                    if ib == 1 and filler is not None:
                                        for step in (filler.pop(0) for _ in range(3) if filler):
                                            step()
                        else:
                            for ibs in ib_order:
                                for nt in range(NT):
                                    y0 = nt * RT
                                    for ib in ibs:
                                        for t9 in range(9):
                                            ky, kx = divmod(t9, 3)
                                            nc.tensor.matmul(
                                                pslist[nt][:],
                                                lhsT=W[wname][:, ib, t9, ob * 128:(ob + 1) * 128],
                                                rhs=Pin[ib][:, y0 + ky:y0 + ky + RT, kx:kx + SZ],
                                                start=(ib == 0 and t9 == 0),
                                                stop=(ib == 1 and t9 == 8))
                                    if ibs[-1] == 1:
                                        emit(ob, nt, pslist[nt])
                                    if filler is not None:
                                        for step in (filler.pop(0) for _ in range(2) if filler):
                                            step()

                def emit_ms(ob, nt, ps):
                    pv = interior(P2[ob], nt)
                    nc.scalar.activation(out=pv, in_=ps[:], func=AF.Relu,
                                         bias=W["tms"][:, ob:ob + 1], scale=1.0)
                    relu6(pv)
                conv3x3(P1, "wms", 2, emit_ms, ib_split=True)

                # ---- MCAM chain as filler steps (interleaved into fq conv) ----
                vblocks = [avg_b[0], avg_b[1], max_b[0], max_b[1]]
                g_colb = [late.tile([128, 1], BF16, tag=f"gc{i}", name=f"gc{i}")
                          for i in range(2)]
                g_row = late.tile([1, DIM], F32, tag="g_row", name="g_row")
                h_b = late.tile([64, 1], BF16, tag="h_b", name="h_b")
                y1c = late.tile([NCLS, 1], BF16, tag="y1c", name="y1c")
                f1c = late.tile([NCLS, 1], BF16, tag="f1c", name="f1c")
                rowt = {nm: late.tile([1, NCLS], F32, tag=f"row_{nm}", name=f"row_{nm}")
                        for nm in ("f1", "y1")}
                cfr = late.tile([1, NCLS], BF16, tag="cfr", name="cfr")
                cfc = late.tile([NCLS, 1], F32, tag="cfc", name="cfc")
                g19 = late.tile([NCLS, DIM], F32, tag="g19", name="g19")
                cmT = late.tile([NCLS, DIM], F32, tag="cmT", name="cmT")
                cme = late.tile([NCLS, DIM], BF16, tag="cme", name="cme")
                v_b = late.tile([128, NCLS], BF16, tag="v_b", name="v_b")
                steps = []

                def s_g(ob):
                    def f():
                        pg = psq(f"psg{ob}", (128, 1))
                        for j in range(4):
                            nc.tensor.matmul(pg[:], lhsT=W["wg"][:, j, ob * 128:(ob + 1) * 128],
                                             rhs=vblocks[j], start=(j == 0), stop=(j == 3))
                        nc.scalar.activation(out=g_colb[ob], in_=pg[:], func=AF.Sigmoid,
                                             bias=W["dg"][:, ob:ob + 1], scale=1.0)
                        pr = psq(f"psgr{ob}", (1, 128), BF16)
                        nc.tensor.transpose(pr[:], g_colb[ob][:], W["ident"][:, :])
                        nc.scalar.activation(out=g_row[:, ob * 128:(ob + 1) * 128],
                                             in_=pr[:], func=AF.Copy)
                    return f
                steps += [s_g(0), s_g(1)]

                def s_h():
                    ph = psq("psh", (64, 1))
                    for ib in range(2):
                        nc.tensor.matmul(ph[:], lhsT=W["wcse1"][:, ib, :], rhs=g_colb[ib],
                                         start=(ib == 0), stop=(ib == 1))
                    nc.scalar.activation(out=h_b, in_=ph[:], func=AF.Relu,
                                         bias=W["bcse1"][:, 0:1], scale=1.0)
                steps.append(s_h)

                def s_y1():
                    py = psq("psy", (NCLS, 1))
                    nc.tensor.matmul(py[:], lhsT=W["wcse2"][:, :], rhs=h_b,
                                     start=True, stop=True)
                    nc.scalar.activation(out=y1c, in_=py[:], func=AF.Sigmoid,
                                         bias=W["bcse2"][:, 0:1], scale=1.0)
                steps.append(s_y1)

                def s_f1():
                    pf = psq("psf", (NCLS, 1))
                    for ib in range(2):
                        nc.tensor.matmul(pf[:], lhsT=W["wfc1"][:, ib, :], rhs=g_colb[ib],
                                         start=(ib == 0), stop=(ib == 1))
                    nc.scalar.activation(out=f1c, in_=pf[:], func=AF.Identity,
                                         bias=W["bfc1"][:, 0:1], scale=1.0)
                steps.append(s_f1)

                def s_row(nm, src_t):
                    def f():
                        pr = psq(f"pr_{nm}", (1, NCLS), BF16)
                        nc.tensor.transpose(pr[:], src_t[:], W["ident"][0:NCLS, 0:NCLS])
                        nc.scalar.activation(out=rowt[nm], in_=pr[:], func=AF.Copy)
                    return f
                steps += [s_row("f1", f1c), s_row("y1", y1c)]

                def s_sm1():
                    f1r, y1r = rowt["f1"], rowt["y1"]
                    s1 = late.tile([1, 1], F32, tag="s1", name="s1")
                    e1 = late.tile([1, NCLS], F32, tag="e1", name="e1")
                    nc.scalar.activation(out=e1, in_=f1r[:], func=AF.Exp,
                                         bias=0.0, scale=1.0, accum_out=s1[:, 0:1])
                    r1 = late.tile([1, 1], F32, tag="r1", name="r1")
                    nc.vector.reciprocal(r1, s1)
                    nc.vector.tensor_scalar(out=e1, in0=e1, scalar1=r1[:, 0:1],
                                            scalar2=None, op0=ALU.mult)
                    nc.vector.tensor_add(cfr, e1, y1r)
                steps.append(s_sm1)

                def s_cfc():
                    pcx = psq("pc_cf", (NCLS, 1), BF16)
                    nc.tensor.transpose(pcx[:], cfr[:], W["ident"][0:1, 0:1])
                    nc.scalar.activation(out=cfc, in_=pcx[:], func=AF.Copy)
                    nc.gpsimd.partition_broadcast(g19[:], g_row[:])
                steps.append(s_cfc)

                def s_cm():
                    nc.vector.tensor_scalar(out=cmT, in0=g19, scalar1=cfc[:, 0:1],
                                            scalar2=None, op0=ALU.mult)
                    s2 = late.tile([NCLS, 1], F32, tag="s2", name="s2")
                    nc.scalar.activation(out=cme, in_=cmT[:], func=AF.Exp,
                                         bias=0.0, scale=1.0, accum_out=s2[:, 0:1])
                    r2 = late.tile([NCLS, 1], F32, tag="r2", name="r2")
                    nc.vector.reciprocal(r2, s2)
                    nc.vector.tensor_scalar(out=cme, in0=cme, scalar1=r2[:, 0:1],
                                            scalar2=None, op0=ALU.mult)
                steps.append(s_cm)

                def s_half(ib):
                    def f():
                        pt = psq(f"pt{ib}", (128, NCLS), BF16)
                        nc.tensor.transpose(pt[:], cme[:, ib * 128:(ib + 1) * 128],
                                            W["ident"][0:NCLS, 0:NCLS])
                        pp = late.tile([128, NCLS], F32, tag=f"pp{ib}", name=f"pp{ib}")
                        nc.scalar.activation(out=pp, in_=pt[:], func=AF.Copy)
                        s3 = late.tile([128, 1], F32, tag=f"s3_{ib}", name=f"s3_{ib}")
                        nc.scalar.activation(out=proxy_b[ib], in_=pp[:], func=AF.Exp,
                                             bias=0.0, scale=1.0, accum_out=s3[:, 0:1])
                        r3 = late.tile([128, 1], F32, tag=f"r3_{ib}", name=f"r3_{ib}")
                        nc.vector.reciprocal(r3, s3)
                        nc.vector.tensor_scalar(out=proxy_b[ib], in0=proxy_b[ib],
                                                scalar1=r3[:, 0:1], scalar2=None, op0=ALU.mult)
                    return f
                steps += [s_half(0), s_half(1)]

                def s_kv(wname, tname, dst):
                    def f():
                        pkv = psq(f"pkv_{wname}", (128, NCLS))
                        for ib in range(2):
                            nc.tensor.matmul(pkv[:], lhsT=W[wname][:, ib, :],
                                             rhs=proxy_b[ib], start=(ib == 0), stop=(ib == 1))
                        nc.scalar.activation(out=dst, in_=pkv[:], func=AF.Relu,
                                             bias=W[tname][:, 0:1], scale=1.0)
                        relu6(dst[:])
                    return f
                steps += [s_kv("wfk", "tfk", k_b), s_kv("wfv", "tfv", v_b)]

                def s_vT():
                    pvT = psq("pvT", (NCLS, 128), BF16)
                    nc.tensor.transpose(pvT[:], v_b[:], W["ident"][:, :])
                    nc.scalar.activation(out=vT_b, in_=pvT[:], func=AF.Copy)
                steps.append(s_vT)

                # ---- fq conv: P2 -> q ----
                q_b = late.tile([128, HW], BF16, tag="q_b", name="q_b")
                def emit_fq(ob, nt, ps):
                    pv = q_b[:, nt * 512:(nt + 1) * 512]
                    nc.scalar.activation(out=pv, in_=ps[:], func=AF.Relu,
                                         bias=W["tfq"][:, 0:1], scale=1.0)
                    relu6(pv)
                for _ in range(3):
                    if steps:
                        steps.pop(0)()
                conv3x3(P2, "wfq", 1, emit_fq, filler=steps)
                while steps:
                    steps.pop(0)()

                # attention mm1: (19, 4096) = k^T q
                mm_b = late.tile([NCLS, HW], BF16, tag="mm_b", name="mm_b")
                for nt in range(NT):
                    pm = psp.tile([NCLS, 512], F32, tag="ps", name=f"pmm{nt}")
                    nc.tensor.matmul(pm[:], lhsT=k_b[:], rhs=q_b[:, nt * 512:(nt + 1) * 512],
                                     start=True, stop=True)
                    nc.scalar.activation(out=mm_b[:, nt * 512:(nt + 1) * 512],
                                         in_=pm[:], func=AF.Copy)

                # c3 conv traced now so PE stays busy during the softmax
                sp = [late.tile([128, HW], BF16, tag=f"sp{i}", name=f"sp{i}")
                      for i in range(2)]
                def emit_c3(ob, nt, ps):
                    pv = sp[ob][:, nt * 512:(nt + 1) * 512]
                    nc.scalar.activation(out=pv, in_=ps[:], func=AF.Relu,
                                         bias=W["tc3"][:, ob:ob + 1], scale=1.0)
                    relu6(pv)
                conv3x3(P2, "wc3", 2, emit_c3)

                # softmax over hw rows of mm (inputs are bounded: no max-sub needed)
                sa_s = late.tile([NCLS, 1], F32, tag="sa_s", name="sa_s")
                A_b = late.tile([NCLS, HW], BF16, tag="A_b", name="A_b")
                nc.scalar.activation(out=A_b, in_=mm_b[:], func=AF.Exp,
                                     bias=0.0, scale=S_ATT, accum_out=sa_s[:, 0:1])
                ra = late.tile([NCLS, 1], F32, tag="ra", name="ra")
                nc.vector.reciprocal(ra, sa_s)
                nc.vector.tensor_scalar(out=A_b, in0=A_b, scalar1=ra[:, 0:1],
                                        scalar2=None, op0=ALU.mult)

                # mm2 + fup + add sa -> P3
                ctx_b = late.tile([128, HW], BF16, tag="ctx_b", name="ctx_b")
                for nt in range(NT):
                    pc2 = pst(f"pctx{nt}")
                    nc.tensor.matmul(pc2[:], lhsT=vT_b[:], rhs=A_b[:, nt * 512:(nt + 1) * 512],
                                     start=True, stop=True)
                    nc.scalar.activation(out=ctx_b[:, nt * 512:(nt + 1) * 512],
                                         in_=pc2[:], func=AF.Copy)
                for ob in range(2):
                    for nt in range(NT):
                        pu = pst(f"pfup{ob}_{nt}")
                        nc.tensor.matmul(pu[:], lhsT=W["wfup"][:, ob * 128:(ob + 1) * 128],
                                         rhs=ctx_b[:, nt * 512:(nt + 1) * 512],
                                         start=True, stop=True)
                        fs = stg.tile([128, 512], BF16, tag="fs", bufs=3,
                                      name=f"fs{ob}_{nt}")
                        nc.scalar.activation(out=fs, in_=pu[:], func=AF.Relu,
                                             bias=W["tfup"][:, ob:ob + 1], scale=1.0)
                        nc.vector.scalar_tensor_tensor(
                            out=interior(P3[ob], nt), in0=fs, scalar=6.0,
                            in1=interior(P2[ob], nt), op0=ALU.min, op1=ALU.add)

                # fuse conv: P3 -> ctxf
                ctxf = [late.tile([128, HW], BF16, tag=f"ctxf{i}", name=f"ctxf{i}")
                        for i in range(2)]
                def emit_fuse(ob, nt, ps):
                    pv = ctxf[ob][:, nt * 512:(nt + 1) * 512]
                    nc.scalar.activation(out=pv, in_=ps[:], func=AF.Relu,
                                         bias=W["tfuse"][:, ob:ob + 1], scale=1.0)
                    relu6(pv)
                conv3x3(P3, "wfuse", 2, emit_fuse)

                # c1 1x1 over concat([ctxf, sp]) -> out
                cat = [ctxf[0], ctxf[1], sp[0], sp[1]]
                for ob in range(2):
                    for nt in range(NT):
                        ps = pst(f"pc1_{ob}_{nt}")
                        for j in range(4):
                            nc.tensor.matmul(ps[:], lhsT=W["wc1"][:, j, ob * 128:(ob + 1) * 128],
                                             rhs=cat[j][:, nt * 512:(nt + 1) * 512],
                                             start=(j == 0), stop=(j == 3))
                        og = stg.tile([128, 512], F32, tag="og", bufs=4,
                                      name=f"og{ob}_{nt}")
                        nc.scalar.activation(out=og, in_=ps[:], func=AF.Relu,
                                             bias=W["tc1"][:, ob:ob + 1], scale=1.0)
                        relu6(og[:])
                        nc.sync.dma_start(
                            out=out_d.ap()[ob * 128:(ob + 1) * 128, nt * 512:(nt + 1) * 512],
                            in_=og)

    nc.compile()
    return nc


# ------------------------------------------------------------------- wrapper

_CACHE = {}


def kernel(x, skip, params):
    from concourse import bass_utils

    x = np.asarray(x, np.float32)
    skip = np.asarray(skip, np.float32)
    B = x.shape[0]
    packed = _prep(params)

    if "nc" not in _CACHE:
        _CACHE["nc"] = build_program()
    nc = _CACHE["nc"]

    in_maps = []
    for i in range(B):
        m = dict(packed)
        m["x"] = np.ascontiguousarray(x[i].reshape(DIM, HW))
        m["skip"] = np.ascontiguousarray(skip[i].reshape(DIM, HW))
        in_maps.append(m)

    res = bass_utils.run_bass_kernel_spmd(
        nc, in_maps, core_ids=list(range(B)),
        trace=bool(int(os.environ.get("KBENCH_TRACE", "0"))))
    _CACHE["last_result"] = res
    out = np.stack([r["out"].reshape(DIM, SZ, SZ) for r in res.results])
    return out.astype(np.float32)


# revision 22
# speedup vs baseline: 17.0969x; 1.0040x over previous
"""Trainium2 Bass kernel for nn_Attention (MCAM + MSAM + CIAFM block).

Sharding: pure data parallelism — B=8 samples across 8 NeuronCores.
Per core: x,skip (256, 64, 64) f32 -> out (256, 64, 64) f32.

Heavy compute = four 3x3 convs (ms, fq, fuse, c3) done as 9-tap
PSUM-accumulated bf16 matmuls over zero-padded (128, 66, 66) SBUF tiles,
plus 1x1 convs (fup, c1) and a tiny NC=19 cross-attention.

Host-side preprocessing folds:
  - BN scales into conv weights (cbr -> relu6(conv(x, W*s) + t))
  - the entire MCAM front end (4 ECA conv1ds + k=3 mixer + FC) into one
    linear map  g_pre = M @ [avgpool; maxpool] + d   (M: 256x512)

Schedule: inputs stream in per 512-column chunk (adds on GPSIMD, partial
pool stats on DVE during the DMA window); MSAM softmax+modulation feeds
the ms conv per chunk; the ms conv is split by input channel block so its
block-0 matmuls start before block-1's modulated input is finished. The
serial MCAM chain (tiny) is traced between ms and fq so it never blocks
the PE FIFO; c3 is traced before the attention softmax for the same
reason.
"""

import os
import numpy as np
import ml_dtypes

BF = ml_dtypes.bfloat16

DIM, NCLS, SZ = 256, 19, 64
KC = 128
HW = SZ * SZ          # 4096
PD = SZ + 2           # 66
NT = 8                # 512-wide output column tiles
RT = SZ // NT         # 8 rows per tile
S_ATT = float(KC) ** -0.5


# ------------------------------------------------------------------ host prep

def _toeplitz(w, n):
    """Dense matrix of 'same'-padded 1-D cross-correlation with kernel w."""
    w = np.asarray(w, np.float64).reshape(-1)
    k = len(w)
    pad = (k - 1) // 2
    T = np.zeros((n, n), np.float64)
    for j in range(k):
        d = j - pad
        lo, hi = max(0, -d), min(n, n - d)
        idx = np.arange(lo, hi)
        T[idx, idx + d] += w[j]
    return T


def _pack_conv(w, scale=None):
    """(O, I, kh, kw) -> (128, I//128, kh*kw, O): lhsT tiles per (in-block, tap)."""
    w = np.asarray(w, np.float64)
    if scale is not None:
        w = w * np.asarray(scale, np.float64)[:, None, None, None]
    O, I, kh, kw = w.shape
    t = w.reshape(O, I, kh * kw).transpose(1, 2, 0)          # (I, taps, O)
    t = t.reshape(I // 128, 128, kh * kw, O).transpose(1, 0, 2, 3)
    return np.ascontiguousarray(t)


def _bf16(a):
    return np.ascontiguousarray(np.asarray(a, np.float32)).astype(BF)


def _f32(a):
    return np.ascontiguousarray(np.asarray(a, np.float32))


def _cols(v, nb):
    """(nb*128,) bias vector -> (128, nb): column ob = v[ob*128:(ob+1)*128]."""
    return _f32(np.asarray(v, np.float64).reshape(nb, 128).T)


def _prep(params):
    p = {k: np.asarray(v, np.float64) for k, v in params.items()}
    o = {}

    # --- fused MCAM front end: g_pre = M @ [avg; max] + d ---
    T1 = np.zeros((8 * DIM, 2 * DIM))
    b1 = np.zeros(8 * DIM)
    for br in range(2):                       # 0 = avg branch, 1 = max branch
        for kk, nm in enumerate(("mc0", "mc1", "mc2", "mc3")):
            r = (br * 4 + kk) * DIM
            T1[r:r + DIM, br * DIM:(br + 1) * DIM] = _toeplitz(p[nm + "_w"], DIM)
            b1[r:r + DIM] = p[nm + "_b"][0]
    T2 = _toeplitz(p["mcc_w"], 8 * DIM)
    bias2 = T2 @ b1 + p["mcc_b"][0]
    M = p["fc_w"] @ T2 @ T1                   # (256, 512)
    d = p["fc_w"] @ bias2 + p["fc_b"]         # (256,)
    o["wg"] = _bf16(M.T.reshape(4, 128, DIM).transpose(1, 0, 2))   # (128,4,256)
    o["dg"] = _cols(d, 2)                                          # (128,2)

    # --- mcam mid (cse1/cse2/fc1) ---
    o["wcse1"] = _bf16(_pack_conv(p["cse1_w"]))[:, :, 0, :]        # (128,2,64)
    o["bcse1"] = _f32(p["cse1_b"].reshape(64, 1))
    o["wcse2"] = _bf16(p["cse2_w"][:, :, 0, 0].T)                  # (64,19)
    o["bcse2"] = _f32(p["cse2_b"].reshape(NCLS, 1))
    o["wfc1"] = _bf16(_pack_conv(p["fc1_w"], p["fc1_s"]))[:, :, 0, :]  # (128,2,19)
    o["bfc1"] = _f32(p["fc1_t"].reshape(NCLS, 1))

    # --- conv weights, BN scale folded ---
    o["wms"] = _bf16(_pack_conv(p["ms_w"], p["ms_s"]))             # (128,2,9,256)
    o["tms"] = _cols(p["ms_t"], 2)
    o["wfq"] = _bf16(_pack_conv(p["fq_w"], p["fq_s"]))             # (128,2,9,128)
    o["tfq"] = _f32(p["fq_t"].reshape(1, 128).T)                   # (128,1)
    o["wfk"] = _bf16(_pack_conv(p["fk_w"], p["fk_s"]))[:, :, 0, :]  # (128,2,128)
    o["tfk"] = _f32(p["fk_t"].reshape(1, 128).T)
    o["wfv"] = _bf16(_pack_conv(p["fv_w"], p["fv_s"]))[:, :, 0, :]
    o["tfv"] = _f32(p["fv_t"].reshape(1, 128).T)
    o["wfup"] = _bf16(_pack_conv(p["fup_w"], p["fup_s"]))[:, 0, 0, :]  # (128,256)
    o["tfup"] = _cols(p["fup_t"], 2)
    o["wfuse"] = _bf16(_pack_conv(p["fuse_w"], p["fuse_s"]))
    o["tfuse"] = _cols(p["fuse_t"], 2)
    o["wc3"] = _bf16(_pack_conv(p["c3_w"], p["c3_s"]))
    o["tc3"] = _cols(p["c3_t"], 2)
    o["wc1"] = _bf16(_pack_conv(p["c1_w"], p["c1_s"]))[:, :, 0, :]  # (128,4,256)
    o["tc1"] = _cols(p["c1_t"], 2)

    o["ident"] = _bf16(np.eye(128))
    return o


# ------------------------------------------------------------- device program

def build_program():
    import concourse.tile as tile
    from concourse import bacc, mybir

    AF = mybir.ActivationFunctionType
    ALU = mybir.AluOpType
    F32 = mybir.dt.float32
    BF16 = mybir.dt.bfloat16

    nc = bacc.Bacc("TRN2", target_bir_lowering=False, debug=False)

    specs = [
        ("x", (DIM, HW), F32), ("skip", (DIM, HW), F32),
        ("wms", (128, 2, 9, DIM), BF16), ("tms", (128, 2), F32),
        ("wfq", (128, 2, 9, KC), BF16), ("tfq", (128, 1), F32),
        ("wg", (128, 4, DIM), BF16), ("dg", (128, 2), F32),
        ("wcse1", (128, 2, 64), BF16), ("bcse1", (64, 1), F32),
        ("wcse2", (64, NCLS), BF16), ("bcse2", (NCLS, 1), F32),
        ("wfc1", (128, 2, NCLS), BF16), ("bfc1", (NCLS, 1), F32),
        ("wfk", (128, 2, KC), BF16), ("tfk", (128, 1), F32),
        ("wfv", (128, 2, KC), BF16), ("tfv", (128, 1), F32),
        ("wfup", (128, DIM), BF16), ("tfup", (128, 2), F32),
        ("wfuse", (128, 2, 9, DIM), BF16), ("tfuse", (128, 2), F32),
        ("wc3", (128, 2, 9, DIM), BF16), ("tc3", (128, 2), F32),
        ("wc1", (128, 4, DIM), BF16), ("tc1", (128, 2), F32),
        ("ident", (128, 128), BF16),
    ]
    dram = {n: nc.dram_tensor(n, list(s), dt, kind="ExternalInput")
            for n, s, dt in specs}
    out_d = nc.dram_tensor("out", [DIM, HW], F32, kind="ExternalOutput")

    with tile.TileContext(nc) as tc:
        with tc.tile_pool(name="cst", bufs=1) as cst, \
             tc.tile_pool(name="glob", bufs=1) as glob:

            # ---- padded conv-input buffers ----
            def padded(tagname):
                ts = [glob.tile([128, PD, PD], BF16, tag=f"{tagname}{i}",
                                name=f"{tagname}{i}") for i in range(2)]
                for t in ts:
                    nc.gpsimd.memset(t[:, 0, :], 0.0)
                    nc.gpsimd.memset(t[:, PD - 1, :], 0.0)
                    nc.gpsimd.memset(t[:, 1:PD - 1, 0], 0.0)
                    nc.gpsimd.memset(t[:, 1:PD - 1, PD - 1], 0.0)
                return ts

            P1 = padded("P1")   # msam y  (ms conv input)
            P2 = padded("P2")   # sa      (fq / c3 / fuse-add input)
            P3 = padded("P3")   # fup_out + sa (fuse conv input)

            def interior(P, nt=None):
                if nt is None:
                    return P[:, 1:1 + SZ, 1:1 + SZ]
                return P[:, 1 + nt * RT:1 + (nt + 1) * RT, 1:1 + SZ]

            # small tensors that cross the early/late phase boundary
            def gt(shape, dt, nm):
                return glob.tile(list(shape), dt, tag=nm, name=nm)

            proxy_b = [gt((128, NCLS), BF16, f"proxy{i}") for i in range(2)]
            k_b = gt((128, NCLS), BF16, "k_b")
            vT_b = gt((NCLS, 128), BF16, "vT_b")
            avg_b = [gt((128, 1), BF16, f"ab{i}") for i in range(2)]
            max_b = [gt((128, 1), BF16, f"mb{i}") for i in range(2)]

            relu6 = lambda ap: nc.vector.tensor_scalar_min(ap, ap, 6.0)

            # ============ early phase: stream inputs, MSAM -> P1 ============
            W = {}

            def load_const(names):
                for n, s, dt in specs:
                    if n in ("x", "skip") or n in W or n not in names:
                        continue
                    t = cst.tile(list(s), dt, tag=n, name=f"c_{n}")
                    nc.sync.dma_start(out=t, in_=dram[n].ap())
                    W[n] = t

            # ms-conv weights must beat the input stream to the DMA queues
            load_const({"wms", "tms"})

            with tc.tile_pool(name="early", bufs=1) as early:
                xx = [early.tile([128, HW], F32, tag=f"xx{i}", name=f"xx{i}")
                      for i in range(2)]
                xw = [early.tile([128, SZ], F32, tag=f"xw{i}", name=f"xw{i}")
                      for i in range(2)]
                xhs = [early.tile([128, SZ], F32, tag=f"xhs{i}", name=f"xhs{i}")
                       for i in range(2)]
                xhp_all = [early.tile([128, SZ, NT], F32, tag=f"xhp{i}", name=f"xhp{i}")
                           for i in range(2)]
                xmaxp = [early.tile([128, NT], F32, tag=f"xmaxp{i}", name=f"xmaxp{i}")
                         for i in range(2)]

                def load_block(ib):
                    x3 = xx[ib][:].rearrange("p (h w) -> p h w", w=SZ)
                    for c in range(NT):
                        sl = slice(c * 512, (c + 1) * 512)
                        xt = early.tile([128, 512], F32, tag="xt", bufs=6,
                                        name=f"xt{ib}_{c}")
                        st = early.tile([128, 512], F32, tag="st", bufs=6,
                                        name=f"st{ib}_{c}")
                        nc.sync.dma_start(out=xt, in_=dram["x"].ap()[ib * 128:(ib + 1) * 128, sl])
                        nc.sync.dma_start(out=st, in_=dram["skip"].ap()[ib * 128:(ib + 1) * 128, sl])
                        nc.gpsimd.tensor_add(xx[ib][:, sl], xt, st)
                        ch3 = x3[:, c * RT:(c + 1) * RT, :]
                        nc.vector.tensor_reduce(out=xw[ib][:, c * RT:(c + 1) * RT],
                                                in_=ch3, axis=mybir.AxisListType.X,
                                                op=ALU.add)
                        # column-sum partial for x_h: reduce the chunk's 8 rows
                        ch3t = xx[ib][:, sl].rearrange("p (h w) -> p w h", w=SZ)
                        nc.vector.tensor_reduce(out=xhp_all[ib][:, :, c], in_=ch3t,
                                                axis=mybir.AxisListType.X, op=ALU.add)

                def msam_block(ib):
                    x3 = xx[ib][:].rearrange("p (h w) -> p h w", w=SZ)
                    xh = xhs[ib]
                    nc.vector.tensor_reduce(out=xh, in_=xhp_all[ib][:], 
                                            axis=mybir.AxisListType.X, op=ALU.add)
                    qk = early.tile([128, SZ, SZ], BF16, tag="qk", name=f"qk{ib}")
                    Ee = early.tile([128, SZ, SZ], F32, tag="Ee", name=f"Ee{ib}")
                    sqh = [early.tile([128, 1], F32, tag=f"sqh{h}", name=f"sq{ib}_{h}")
                           for h in range(2)]
                    for h in range(2):
                        rows = slice(h * (SZ // 2), (h + 1) * (SZ // 2))
                        nc.vector.scalar_tensor_tensor(
                            out=qk[:, rows, :],
                            in0=xw[ib][:, rows].unsqueeze(2).broadcast_to([128, SZ // 2, SZ]),
                            scalar=1.0 / (HW * 1.0),
                            in1=xh[:].unsqueeze(1).broadcast_to([128, SZ // 2, SZ]),
                            op0=ALU.mult, op1=ALU.mult)
                        nc.scalar.activation(out=Ee[:, rows, :], in_=qk[:, rows, :],
                                             func=AF.Exp, bias=0.0, scale=1.0,
                                             accum_out=sqh[h][:, 0:1])
                    sq = early.tile([128, 1], F32, tag=f"sq{ib}", name=f"sq{ib}")
                    nc.vector.tensor_add(sq, sqh[0], sqh[1])
                    rq = early.tile([128, 1], F32, tag=f"rq{ib}", name=f"rq{ib}")
                    nc.vector.reciprocal(rq, sq)
                    for c in range(NT):
                        Ech = Ee[:, c * RT:(c + 1) * RT, :]
                        nc.vector.tensor_scalar(out=Ech, in0=Ech, scalar1=rq[:, 0:1],
                                                scalar2=1.0, op0=ALU.mult, op1=ALU.add)
                        nc.gpsimd.tensor_tensor(out=interior(P1[ib], c), in0=Ech,
                                                in1=x3[:, c * RT:(c + 1) * RT, :],
                                                op=ALU.mult)

                def stats_block(ib):
                    x3 = xx[ib][:].rearrange("p (h w) -> p h w", w=SZ)
                    for c in range(NT):
                        nc.vector.tensor_reduce(out=xmaxp[ib][:, c:c + 1],
                                                in_=x3[:, c * RT:(c + 1) * RT, :],
                                                axis=mybir.AxisListType.XY, op=ALU.max)
                    ssum = early.tile([128, 1], F32, tag=f"ssum{ib}", name=f"ssum{ib}")
                    smax = early.tile([128, 1], F32, tag=f"smax{ib}", name=f"smax{ib}")
                    nc.vector.tensor_reduce(out=ssum, in_=xw[ib][:],
                                            axis=mybir.AxisListType.X, op=ALU.add)
                    nc.vector.tensor_reduce(out=smax, in_=xmaxp[ib][:],
                                            axis=mybir.AxisListType.X, op=ALU.max)
                    nc.scalar.activation(out=avg_b[ib], in_=ssum, func=AF.Copy,
                                         scale=1.0 / HW)
                    nc.scalar.activation(out=max_b[ib], in_=smax, func=AF.Copy)

                load_block(0)
                msam_block(0)
                load_block(1)
                msam_block(1)
                stats_block(0)
                stats_block(1)

                # remaining constants (queued behind the input loads)
                load_const({n for n, _, _ in specs})

            # =============== late phase: convs + mcam + attention ===========
            with tc.tile_pool(name="late", bufs=1) as late, \
                 tc.tile_pool(name="stg", bufs=1) as stg, \
                 tc.tile_pool(name="psp", bufs=8, space="PSUM") as psp:

                def pst(name):
                    return psp.tile([128, 512], F32, tag="ps", name=name)

                def psq(name, shape, dt=None):
                    return psp.tile(list(shape), dt or F32, tag="ps", name=name)

                TAPS_OUTER = bool(int(os.environ.get("KERNEL_TAPS_OUTER", "0")))

                def conv3x3(Pin, wname, n_ob, emit, ib_split=False, filler=None):
                    for ob in range(n_ob):
                        pslist = [pst(f"ps_{wname}_{ob}_{nt}") for nt in range(NT)]
                        ib_order = ((0, 1),) if not ib_split else ((0,), (1,))
                        if TAPS_OUTER:
                            # same stationary weight across all 8 n-tiles:
                            # 8x fewer LDWEIGHTS switches on the PE
                            for ibs in ib_order:
                                for ib in ibs:
                                    for t9 in range(9):
                                        ky, kx = divmod(t9, 3)
                                        lhsT = W[wname][:, ib, t9, ob * 128:(ob + 1) * 128]
                                        for nt in range(NT):
                                            y0 = nt * RT
                                            nc.tensor.matmul(
                                                pslist[nt][:], lhsT=lhsT,
                                                rhs=Pin[ib][:, y0 + ky:y0 + ky + RT, kx:kx + SZ],
                                                start=(ib == 0 and t9 == 0),
                                                stop=(ib == 1 and t9 == 8))
                                            if ib == 1 and t9 == 8:
                                                emit(ob, nt, pslist[nt])
                                    if ib == 1 and filler is not None:
                                        for step in (filler.pop(0) for _ in range(3) if filler):
                                            step()
                        else:
                            for ibs in ib_order:
                                for nt in range(NT):
                                    y0 = nt * RT
                                    for ib in ibs:
                                        for t9 in range(9):
                                            ky, kx = divmod(t9, 3)
                                            nc.tensor.matmul(
                                                pslist[nt][:],
                                                lhsT=W[wname][:, ib, t9, ob * 128:(ob + 1) * 128],
                                                rhs=Pin[ib][:, y0 + ky:y0 + ky + RT, kx:kx + SZ],
                                                start=(ib == 0 and t9 == 0),
                                                stop=(ib == 1 and t9 == 8))
                                    if ibs[-1] == 1:
                                        emit(ob, nt, pslist[nt])
                                    if filler is not None:
                                        for step in (filler.pop(0) for _ in range(2) if filler):
                                            step()

                def emit_ms(ob, nt, ps):
                    pv = interior(P2[ob], nt)
                    nc.scalar.activation(out=pv, in_=ps[:], func=AF.Relu,
                                         bias=W["tms"][:, ob:ob + 1], scale=1.0)
                    relu6(pv)
                conv3x3(P1, "wms", 2, emit_ms, ib_split=True)

                # ---- MCAM chain as filler steps (interleaved into fq conv) ----
                vblocks = [avg_b[0], avg_b[1], max_b[0], max_b[1]]
                g_colb = [late.tile([128, 1], BF16, tag=f"gc{i}", name=f"gc{i}")
                          for i in range(2)]
                g_row = late.tile([1, DIM], F32, tag="g_row", name="g_row")
                h_b = late.tile([64, 1], BF16, tag="h_b", name="h_b")
                y1c = late.tile([NCLS, 1], BF16, tag="y1c", name="y1c")
                f1c = late.tile([NCLS, 1], BF16, tag="f1c", name="f1c")
                rowt = {nm: late.tile([1, NCLS], F32, tag=f"row_{nm}", name=f"row_{nm}")
                        for nm in ("f1", "y1")}
                cfr = late.tile([1, NCLS], BF16, tag="cfr", name="cfr")
                cfc = late.tile([NCLS, 1], F32, tag="cfc", name="cfc")
                g19 = late.tile([NCLS, DIM], F32, tag="g19", name="g19")
                cmT = late.tile([NCLS, DIM], F32, tag="cmT", name="cmT")
                cme = late.tile([NCLS, DIM], BF16, tag="cme", name="cme")
                v_b = late.tile([128, NCLS], BF16, tag="v_b", name="v_b")
                steps = []

                def s_g(ob):
                    def f():
                        pg = psq(f"psg{ob}", (128, 1))
                        for j in range(4):
                            nc.tensor.matmul(pg[:], lhsT=W["wg"][:, j, ob * 128:(ob + 1) * 128],
                                             rhs=vblocks[j], start=(j == 0), stop=(j == 3))
                        nc.scalar.activation(out=g_colb[ob], in_=pg[:], func=AF.Sigmoid,
                                             bias=W["dg"][:, ob:ob + 1], scale=1.0)
                        pr = psq(f"psgr{ob}", (1, 128), BF16)
                        nc.tensor.transpose(pr[:], g_colb[ob][:], W["ident"][:, :])
                        nc.scalar.activation(out=g_row[:, ob * 128:(ob + 1) * 128],
                                             in_=pr[:], func=AF.Copy)
                    return f
                steps += [s_g(0), s_g(1)]

                def s_h():
                    ph = psq("psh", (64, 1))
                    for ib in range(2):
                        nc.tensor.matmul(ph[:], lhsT=W["wcse1"][:, ib, :], rhs=g_colb[ib],
                                         start=(ib == 0), stop=(ib == 1))
                    nc.scalar.activation(out=h_b, in_=ph[:], func=AF.Relu,
                                         bias=W["bcse1"][:, 0:1], scale=1.0)
                steps.append(s_h)

                def s_y1():
                    py = psq("psy", (NCLS, 1))
                    nc.tensor.matmul(py[:], lhsT=W["wcse2"][:, :], rhs=h_b,
                                     start=True, stop=True)
                    nc.scalar.activation(out=y1c, in_=py[:], func=AF.Sigmoid,
                                         bias=W["bcse2"][:, 0:1], scale=1.0)
                steps.append(s_y1)

                def s_f1():
                    pf = psq("psf", (NCLS, 1))
                    for ib in range(2):
                        nc.tensor.matmul(pf[:], lhsT=W["wfc1"][:, ib, :], rhs=g_colb[ib],
                                         start=(ib == 0), stop=(ib == 1))
                    nc.scalar.activation(out=f1c, in_=pf[:], func=AF.Identity,
                                         bias=W["bfc1"][:, 0:1], scale=1.0)
                steps.append(s_f1)

                def s_row(nm, src_t):
                    def f():
                        pr = psq(f"pr_{nm}", (1, NCLS), BF16)
                        nc.tensor.transpose(pr[:], src_t[:], W["ident"][0:NCLS, 0:NCLS])
                        nc.scalar.activation(out=rowt[nm], in_=pr[:], func=AF.Copy)
                    return f
                steps += [s_row("f1", f1c), s_row("y1", y1c)]

                def s_sm1():
                    f1r, y1r = rowt["f1"], rowt["y1"]
                    s1 = late.tile([1, 1], F32, tag="s1", name="s1")
                    e1 = late.tile([1, NCLS], F32, tag="e1", name="e1")
                    nc.scalar.activation(out=e1, in_=f1r[:], func=AF.Exp,
                                         bias=0.0, scale=1.0, accum_out=s1[:, 0:1])
                    r1 = late.tile([1, 1], F32, tag="r1", name="r1")
                    nc.vector.reciprocal(r1, s1)
                    nc.vector.tensor_scalar(out=e1, in0=e1, scalar1=r1[:, 0:1],
                                            scalar2=None, op0=ALU.mult)
                    nc.vector.tensor_add(cfr, e1, y1r)
                steps.append(s_sm1)

                def s_cfc():
                    pcx = psq("pc_cf", (NCLS, 1), BF16)
                    nc.tensor.transpose(pcx[:], cfr[:], W["ident"][0:1, 0:1])
                    nc.scalar.activation(out=cfc, in_=pcx[:], func=AF.Copy)
                    nc.gpsimd.partition_broadcast(g19[:], g_row[:])
                steps.append(s_cfc)

                def s_cm():
                    nc.vector.tensor_scalar(out=cmT, in0=g19, scalar1=cfc[:, 0:1],
                                            scalar2=None, op0=ALU.mult)
                    s2 = late.tile([NCLS, 1], F32, tag="s2", name="s2")
                    nc.scalar.activation(out=cme, in_=cmT[:], func=AF.Exp,
                                         bias=0.0, scale=1.0, accum_out=s2[:, 0:1])
                    r2 = late.tile([NCLS, 1], F32, tag="r2", name="r2")
                    nc.vector.reciprocal(r2, s2)
                    nc.vector.tensor_scalar(out=cme, in0=cme, scalar1=r2[:, 0:1],
                                            scalar2=None, op0=ALU.mult)
                steps.append(s_cm)

                def s_half(ib):
                    def f():
                        pt = psq(f"pt{ib}", (128, NCLS), BF16)
                        nc.tensor.transpose(pt[:], cme[:, ib * 128:(ib + 1) * 128],
                                            W["ident"][0:NCLS, 0:NCLS])
                        pp = late.tile([128, NCLS], F32, tag=f"pp{ib}", name=f"pp{ib}")
                        nc.scalar.activation(out=pp, in_=pt[:], func=AF.Copy)
                        s3 = late.tile([128, 1], F32, tag=f"s3_{ib}", name=f"s3_{ib}")
                        nc.scalar.activation(out=proxy_b[ib], in_=pp[:], func=AF.Exp,
                                             bias=0.0, scale=1.0, accum_out=s3[:, 0:1])
                        r3 = late.tile([128, 1], F32, tag=f"r3_{ib}", name=f"r3_{ib}")
                        nc.vector.reciprocal(r3, s3)
                        nc.vector.tensor_scalar(out=proxy_b[ib], in0=proxy_b[ib],
                                                scalar1=r3[:, 0:1], scalar2=None, op0=ALU.mult)
                    return f
                steps += [s_half(0), s_half(1)]

                def s_kv(wname, tname, dst):
                    def f():
                        pkv = psq(f"pkv_{wname}", (128, NCLS))
                        for ib in range(2):
                            nc.tensor.matmul(pkv[:], lhsT=W[wname][:, ib, :],
                                             rhs=proxy_b[ib], start=(ib == 0), stop=(ib == 1))
                        nc.scalar.activation(out=dst, in_=pkv[:], func=AF.Relu,
                                             bias=W[tname][:, 0:1], scale=1.0)
                        relu6(dst[:])
                    return f
                steps += [s_kv("wfk", "tfk", k_b), s_kv("wfv", "tfv", v_b)]

                def s_vT():
                    pvT = psq("pvT", (NCLS, 128), BF16)
                    nc.tensor.transpose(pvT[:], v_b[:], W["ident"][:, :])
                    nc.scalar.activation(out=vT_b, in_=pvT[:], func=AF.Copy)
                steps.append(s_vT)

                # ---- fq conv: P2 -> q ----
                q_b = late.tile([128, HW], BF16, tag="q_b", name="q_b")
                def emit_fq(ob, nt, ps):
                    pv = q_b[:, nt * 512:(nt + 1) * 512]
                    nc.scalar.activation(out=pv, in_=ps[:], func=AF.Relu,
                                         bias=W["tfq"][:, 0:1], scale=1.0)
                    relu6(pv)
                for _ in range(3):
                    if steps:
                        steps.pop(0)()
                conv3x3(P2, "wfq", 1, emit_fq, filler=steps)
                while steps:
                    steps.pop(0)()

                # attention mm1: (19, 4096) = k^T q
                mm_b = late.tile([NCLS, HW], BF16, tag="mm_b", name="mm_b")
                for nt in range(NT):
                    pm = psp.tile([NCLS, 512], F32, tag="ps", name=f"pmm{nt}")
                    nc.tensor.matmul(pm[:], lhsT=k_b[:], rhs=q_b[:, nt * 512:(nt + 1) * 512],
                                     start=True, stop=True)
                    nc.scalar.activation(out=mm_b[:, nt * 512:(nt + 1) * 512],
                                         in_=pm[:], func=AF.Copy)

                # c3 conv traced now so PE stays busy during the softmax
                sp = [late.tile([128, HW], BF16, tag=f"sp{i}", name=f"sp{i}")
                      for i in range(2)]
                def emit_c3(ob, nt, ps):
                    pv = sp[ob][:, nt * 512:(nt + 1) * 512]
                    nc.scalar.activation(out=pv, in_=ps[:], func=AF.Relu,
                                         bias=W["tc3"][:, ob:ob + 1], scale=1.0)
                    relu6(pv)
                conv3x3(P2, "wc3", 2, emit_c3)

                # softmax over hw rows of mm (inputs are bounded: no max-sub needed)
                sa_s = late.tile([NCLS, 1], F32, tag="sa_s", name="sa_s")
                A_b = late.tile([NCLS, HW], BF16, tag="A_b", name="A_b")
                nc.scalar.activation(out=A_b, in_=mm_b[:], func=AF.Exp,
                                     bias=0.0, scale=S_ATT, accum_out=sa_s[:, 0:1])
                ra = late.tile([NCLS, 1], F32, tag="ra", name="ra")
                nc.vector.reciprocal(ra, sa_s)
                nc.vector.tensor_scalar(out=A_b, in0=A_b, scalar1=ra[:, 0:1],
                                        scalar2=None, op0=ALU.mult)

                # mm2 + fup + add sa -> P3
                ctx_b = late.tile([128, HW], BF16, tag="ctx_b", name="ctx_b")
                for nt in range(NT):
                    pc2 = pst(f"pctx{nt}")
                    nc.tensor.matmul(pc2[:], lhsT=vT_b[:], rhs=A_b[:, nt * 512:(nt + 1) * 512],
                                     start=True, stop=True)
                    nc.scalar.activation(out=ctx_b[:, nt * 512:(nt + 1) * 512],
                                         in_=pc2[:], func=AF.Copy)
                for ob in range(2):
                    for nt in range(NT):
                        pu = pst(f"pfup{ob}_{nt}")
                        nc.tensor.matmul(pu[:], lhsT=W["wfup"][:, ob * 128:(ob + 1) * 128],
                                         rhs=ctx_b[:, nt * 512:(nt + 1) * 512],
                                         start=True, stop=True)
                        fs = stg.tile([128, 512], BF16, tag="fs", bufs=3,
                                      name=f"fs{ob}_{nt}")
                        nc.scalar.activation(out=fs, in_=pu[:], func=AF.Relu,
                                             bias=W["tfup"][:, ob:ob + 1], scale=1.0)
                        nc.vector.scalar_tensor_tensor(
                            out=interior(P3[ob], nt), in0=fs, scalar=6.0,
                            in1=interior(P2[ob], nt), op0=ALU.min, op1=ALU.add)

                # fuse conv: P3 -> ctxf
                ctxf = [late.tile([128, HW], BF16, tag=f"ctxf{i}", name=f"ctxf{i}")
                        for i in range(2)]
                def emit_fuse(ob, nt, ps):
                    pv = ctxf[ob][:, nt * 512:(nt + 1) * 512]
                    nc.scalar.activation(out=pv, in_=ps[:], func=AF.Relu,
                                         bias=W["tfuse"][:, ob:ob + 1], scale=1.0)
                    relu6(pv)
                conv3x3(P3, "wfuse", 2, emit_fuse)

                # c1 1x1 over concat([ctxf, sp]) -> out
                cat = [ctxf[0], ctxf[1], sp[0], sp[1]]
                for ob in range(2):
                    for nt in range(NT):
                        ps = pst(f"pc1_{ob}_{nt}")
                        for j in range(4):
                            nc.tensor.matmul(ps[:], lhsT=W["wc1"][:, j, ob * 128:(ob + 1) * 128],
                                             rhs=cat[j][:, nt * 512:(nt + 1) * 512],
                                             start=(j == 0), stop=(j == 3))
                        og = stg.tile([128, 512], F32, tag="og", bufs=4,
                                      name=f"og{ob}_{nt}")
                        nc.scalar.activation(out=og, in_=ps[:], func=AF.Relu,
                                             bias=W["tc1"][:, ob:ob + 1], scale=1.0)
                        relu6(og[:])
                        nc.sync.dma_start(
                            out=out_d.ap()[ob * 128:(ob + 1) * 128, nt * 512:(nt + 1) * 512],
                            in_=og)

    nc.compile()
    return nc


# ------------------------------------------------------------------- wrapper

_CACHE = {}


def kernel(x, skip, params):
    from concourse import bass_utils

    x = np.asarray(x, np.float32)
    skip = np.asarray(skip, np.float32)
    B = x.shape[0]
    packed = _prep(params)

    if "nc" not in _CACHE:
        _CACHE["nc"] = build_program()
    nc = _CACHE["nc"]

    in_maps = []
    for i in range(B):
        m = dict(packed)
        m["x"] = np.ascontiguousarray(x[i].reshape(DIM, HW))
        m["skip"] = np.ascontiguousarray(skip[i].reshape(DIM, HW))
        in_maps.append(m)

    trace = bool(int(os.environ.get("KBENCH_TRACE", "0")))
    try:
        res = bass_utils.run_bass_kernel_spmd(
            nc, in_maps, core_ids=list(range(B)), trace=trace)
    except ModuleNotFoundError:
        # axon NTFF profiling hook unavailable in this environment
        os.environ["BASS_NEVER_TRACE"] = "1"
        res = bass_utils.run_bass_kernel_spmd(
            nc, in_maps, core_ids=list(range(B)), trace=False)
    _CACHE["last_result"] = res
    out = np.stack([r["out"].reshape(DIM, SZ, SZ) for r in res.results])
    return out.astype(np.float32)


# revision 29
# speedup vs baseline: 17.4163x; 1.0187x over previous
"""Trainium2 Bass kernel for nn_Attention (MCAM + MSAM + CIAFM block).

Sharding: pure data parallelism — B=8 samples across 8 NeuronCores.
Per core: x,skip (256, 64, 64) f32 -> out (256, 64, 64) f32.

Heavy compute = four 3x3 convs (ms, fq, fuse, c3) done as 9-tap
PSUM-accumulated bf16 matmuls over zero-padded (128, 66, 66) SBUF tiles,
plus 1x1 convs (fup, c1) and a tiny NC=19 cross-attention.

Host-side preprocessing folds:
  - BN scales into conv weights (cbr -> relu6(conv(x, W*s) + t))
  - the entire MCAM front end (4 ECA conv1ds + k=3 mixer + FC) into one
    linear map  g_pre = M @ [avgpool; maxpool] + d   (M: 256x512)

Schedule: inputs stream in per 512-column chunk (adds on GPSIMD, partial
pool stats on DVE during the DMA window); MSAM softmax+modulation feeds
the ms conv per chunk; the ms conv is split by input channel block so its
block-0 matmuls start before block-1's modulated input is finished. The
serial MCAM chain (tiny) is traced between ms and fq so it never blocks
the PE FIFO; c3 is traced before the attention softmax for the same
reason.
"""

import os
import numpy as np
import ml_dtypes

BF = ml_dtypes.bfloat16

DIM, NCLS, SZ = 256, 19, 64
KC = 128
HW = SZ * SZ          # 4096
PD = SZ + 2           # 66
NT = 8                # 512-wide output column tiles
RT = SZ // NT         # 8 rows per tile
S_ATT = float(KC) ** -0.5


# ------------------------------------------------------------------ host prep

def _toeplitz(w, n):
    """Dense matrix of 'same'-padded 1-D cross-correlation with kernel w."""
    w = np.asarray(w, np.float64).reshape(-1)
    k = len(w)
    pad = (k - 1) // 2
    T = np.zeros((n, n), np.float64)
    for j in range(k):
        d = j - pad
        lo, hi = max(0, -d), min(n, n - d)
        idx = np.arange(lo, hi)
        T[idx, idx + d] += w[j]
    return T


def _pack_conv(w, scale=None):
    """(O, I, kh, kw) -> (128, I//128, kh*kw, O): lhsT tiles per (in-block, tap)."""
    w = np.asarray(w, np.float64)
    if scale is not None:
        w = w * np.asarray(scale, np.float64)[:, None, None, None]
    O, I, kh, kw = w.shape
    t = w.reshape(O, I, kh * kw).transpose(1, 2, 0)          # (I, taps, O)
    t = t.reshape(I // 128, 128, kh * kw, O).transpose(1, 0, 2, 3)
    return np.ascontiguousarray(t)


def _bf16(a):
    return np.ascontiguousarray(np.asarray(a, np.float32)).astype(BF)


def _f32(a):
    return np.ascontiguousarray(np.asarray(a, np.float32))


def _cols(v, nb):
    """(nb*128,) bias vector -> (128, nb): column ob = v[ob*128:(ob+1)*128]."""
    return _f32(np.asarray(v, np.float64).reshape(nb, 128).T)


def _prep(params):
    p = {k: np.asarray(v, np.float64) for k, v in params.items()}
    o = {}

    # --- fused MCAM front end: g_pre = M @ [avg; max] + d ---
    T1 = np.zeros((8 * DIM, 2 * DIM))
    b1 = np.zeros(8 * DIM)
    for br in range(2):                       # 0 = avg branch, 1 = max branch
        for kk, nm in enumerate(("mc0", "mc1", "mc2", "mc3")):
            r = (br * 4 + kk) * DIM
            T1[r:r + DIM, br * DIM:(br + 1) * DIM] = _toeplitz(p[nm + "_w"], DIM)
            b1[r:r + DIM] = p[nm + "_b"][0]
    T2 = _toeplitz(p["mcc_w"], 8 * DIM)
    bias2 = T2 @ b1 + p["mcc_b"][0]
    M = p["fc_w"] @ T2 @ T1                   # (256, 512)
    d = p["fc_w"] @ bias2 + p["fc_b"]         # (256,)
    o["wg"] = _bf16(M.T.reshape(4, 128, DIM).transpose(1, 0, 2))   # (128,4,256)
    o["dg"] = _cols(d, 2)                                          # (128,2)

    # --- mcam mid (cse1/cse2/fc1) ---
    o["wcse1"] = _bf16(_pack_conv(p["cse1_w"]))[:, :, 0, :]        # (128,2,64)
    o["bcse1"] = _f32(p["cse1_b"].reshape(64, 1))
    o["wcse2"] = _bf16(p["cse2_w"][:, :, 0, 0].T)                  # (64,19)
    o["bcse2"] = _f32(p["cse2_b"].reshape(NCLS, 1))
    o["wfc1"] = _bf16(_pack_conv(p["fc1_w"], p["fc1_s"]))[:, :, 0, :]  # (128,2,19)
    o["bfc1"] = _f32(p["fc1_t"].reshape(NCLS, 1))

    # --- conv weights, BN scale folded ---
    o["wms"] = _bf16(_pack_conv(p["ms_w"], p["ms_s"]))             # (128,2,9,256)
    o["tms"] = _cols(p["ms_t"], 2)
    o["wfq"] = _bf16(_pack_conv(p["fq_w"], p["fq_s"]))             # (128,2,9,128)
    o["tfq"] = _f32(p["fq_t"].reshape(1, 128).T)                   # (128,1)
    o["wfk"] = _bf16(_pack_conv(p["fk_w"], p["fk_s"]))[:, :, 0, :]  # (128,2,128)
    o["tfk"] = _f32(p["fk_t"].reshape(1, 128).T)
    o["wfv"] = _bf16(_pack_conv(p["fv_w"], p["fv_s"]))[:, :, 0, :]
    o["tfv"] = _f32(p["fv_t"].reshape(1, 128).T)
    o["wfup"] = _bf16(_pack_conv(p["fup_w"], p["fup_s"]))[:, 0, 0, :]  # (128,256)
    o["tfup"] = _cols(p["fup_t"], 2)
    o["wfuse"] = _bf16(_pack_conv(p["fuse_w"], p["fuse_s"]))
    o["tfuse"] = _cols(p["fuse_t"], 2)
    o["wc3"] = _bf16(_pack_conv(p["c3_w"], p["c3_s"]))
    o["tc3"] = _cols(p["c3_t"], 2)
    o["wc1"] = _bf16(_pack_conv(p["c1_w"], p["c1_s"]))[:, :, 0, :]  # (128,4,256)
    o["tc1"] = _cols(p["c1_t"], 2)

    o["ident"] = _bf16(np.eye(128))
    return o


# ------------------------------------------------------------- device program

def build_program():
    import concourse.tile as tile
    from concourse import bacc, mybir

    AF = mybir.ActivationFunctionType
    ALU = mybir.AluOpType
    F32 = mybir.dt.float32
    BF16 = mybir.dt.bfloat16

    nc = bacc.Bacc("TRN2", target_bir_lowering=False, debug=False)

    specs = [
        ("x", (DIM, HW), F32), ("skip", (DIM, HW), F32),
        ("wms", (128, 2, 9, DIM), BF16), ("tms", (128, 2), F32),
        ("wfq", (128, 2, 9, KC), BF16), ("tfq", (128, 1), F32),
        ("wg", (128, 4, DIM), BF16), ("dg", (128, 2), F32),
        ("wcse1", (128, 2, 64), BF16), ("bcse1", (64, 1), F32),
        ("wcse2", (64, NCLS), BF16), ("bcse2", (NCLS, 1), F32),
        ("wfc1", (128, 2, NCLS), BF16), ("bfc1", (NCLS, 1), F32),
        ("wfk", (128, 2, KC), BF16), ("tfk", (128, 1), F32),
        ("wfv", (128, 2, KC), BF16), ("tfv", (128, 1), F32),
        ("wfup", (128, DIM), BF16), ("tfup", (128, 2), F32),
        ("wfuse", (128, 2, 9, DIM), BF16), ("tfuse", (128, 2), F32),
        ("wc3", (128, 2, 9, DIM), BF16), ("tc3", (128, 2), F32),
        ("wc1", (128, 4, DIM), BF16), ("tc1", (128, 2), F32),
        ("ident", (128, 128), BF16),
    ]
    dram = {n: nc.dram_tensor(n, list(s), dt, kind="ExternalInput")
            for n, s, dt in specs}
    out_d = nc.dram_tensor("out", [DIM, HW], F32, kind="ExternalOutput")

    from concourse.tile import add_dep_helper

    with tile.TileContext(nc) as tc:
        with tc.tile_pool(name="cst", bufs=1) as cst, \
             tc.tile_pool(name="glob", bufs=1) as glob:

            # ---- padded conv-input buffers ----
            def padded(tagname):
                ts = [glob.tile([128, PD, PD], BF16, tag=f"{tagname}{i}",
                                name=f"{tagname}{i}") for i in range(2)]
                for t in ts:
                    nc.gpsimd.memset(t[:, 0, :], 0.0)
                    nc.gpsimd.memset(t[:, PD - 1, :], 0.0)
                    nc.gpsimd.memset(t[:, 1:PD - 1, 0], 0.0)
                    nc.gpsimd.memset(t[:, 1:PD - 1, PD - 1], 0.0)
                return ts

            P1 = padded("P1")   # msam y  (ms conv input)
            P2 = padded("P2")   # sa      (fq / c3 / fuse-add input)
            P3 = padded("P3")   # fup_out + sa (fuse conv input)

            def interior(P, nt=None):
                if nt is None:
                    return P[:, 1:1 + SZ, 1:1 + SZ]
                return P[:, 1 + nt * RT:1 + (nt + 1) * RT, 1:1 + SZ]

            # small tensors that cross the early/late phase boundary
            def gt(shape, dt, nm):
                return glob.tile(list(shape), dt, tag=nm, name=nm)

            proxy_b = [gt((128, NCLS), BF16, f"proxy{i}") for i in range(2)]
            k_b = gt((128, NCLS), BF16, "k_b")
            vT_b = gt((NCLS, 128), BF16, "vT_b")
            avg_b = [gt((128, 1), BF16, f"ab{i}") for i in range(2)]
            max_b = [gt((128, 1), BF16, f"mb{i}") for i in range(2)]

            relu6 = lambda ap: nc.vector.tensor_scalar_min(ap, ap, 6.0)

            # ============ early phase: stream inputs, MSAM -> P1 ============
            W = {}

            def load_const(names):
                for n, s, dt in specs:
                    if n in ("x", "skip") or n in W or n not in names:
                        continue
                    t = cst.tile(list(s), dt, tag=n, name=f"c_{n}")
                    nc.sync.dma_start(out=t, in_=dram[n].ap())
                    W[n] = t

            with tc.tile_pool(name="early", bufs=1) as early:
                xx = [early.tile([128, HW], F32, tag=f"xx{i}", name=f"xx{i}")
                      for i in range(2)]
                xw = [early.tile([128, SZ], F32, tag=f"xw{i}", name=f"xw{i}")
                      for i in range(2)]
                xhs = [early.tile([128, SZ], F32, tag=f"xhs{i}", name=f"xhs{i}")
                       for i in range(2)]
                xhp_all = [early.tile([128, SZ, NT], F32, tag=f"xhp{i}", name=f"xhp{i}")
                           for i in range(2)]
                xmaxp = [early.tile([128, NT], F32, tag=f"xmaxp{i}", name=f"xmaxp{i}")
                         for i in range(2)]

                def load_block(ib, dep=None):
                    x3 = xx[ib][:].rearrange("p (h w) -> p h w", w=SZ)
                    for c in range(NT):
                        sl = slice(c * 512, (c + 1) * 512)
                        xt = early.tile([128, 512], F32, tag="xt", bufs=6,
                                        name=f"xt{ib}_{c}")
                        st = early.tile([128, 512], F32, tag="st", bufs=6,
                                        name=f"st{ib}_{c}")
                        nc.sync.dma_start(out=xt, in_=dram["x"].ap()[ib * 128:(ib + 1) * 128, sl])
                        nc.sync.dma_start(out=st, in_=dram["skip"].ap()[ib * 128:(ib + 1) * 128, sl])
                        nc.gpsimd.tensor_add(xx[ib][:, sl], xt, st)
                        ch3 = x3[:, c * RT:(c + 1) * RT, :]
                        i1 = nc.vector.tensor_reduce(out=xw[ib][:, c * RT:(c + 1) * RT],
                                                     in_=ch3, axis=mybir.AxisListType.X,
                                                     op=ALU.add)
                        # column-sum partial for x_h: reduce the chunk's 8 rows
                        ch3t = xx[ib][:, sl].rearrange("p (h w) -> p w h", w=SZ)
                        i2 = nc.vector.tensor_reduce(out=xhp_all[ib][:, :, c], in_=ch3t,
                                                     axis=mybir.AxisListType.X, op=ALU.add)
                        if dep is not None:
                            add_dep_helper(i1.ins, dep.ins, sync=False,
                                           reason="keep b1 partials off the b0 softmax chain")
                            add_dep_helper(i2.ins, dep.ins, sync=False,
                                           reason="keep b1 partials off the b0 softmax chain")

                def msam_block(ib):
                    x3 = xx[ib][:].rearrange("p (h w) -> p h w", w=SZ)
                    xh = xhs[ib]
                    nc.vector.tensor_reduce(out=xh, in_=xhp_all[ib][:], 
                                            axis=mybir.AxisListType.X, op=ALU.add)
                    qk = early.tile([128, SZ, SZ], BF16, tag="qk", name=f"qk{ib}")
                    Ee = early.tile([128, SZ, SZ], F32, tag="Ee", name=f"Ee{ib}")
                    sqh = [early.tile([128, 1], F32, tag=f"sqh{h}", name=f"sq{ib}_{h}")
                           for h in range(2)]
                    qk_insts = []
                    for h in range(2):
                        rows = slice(h * (SZ // 2), (h + 1) * (SZ // 2))
                        qk_insts.append(nc.vector.scalar_tensor_tensor(
                            out=qk[:, rows, :],
                            in0=xw[ib][:, rows].unsqueeze(2).broadcast_to([128, SZ // 2, SZ]),
                            scalar=1.0 / (HW * 1.0),
                            in1=xh[:].unsqueeze(1).broadcast_to([128, SZ // 2, SZ]),
                            op0=ALU.mult, op1=ALU.mult))
                        nc.scalar.activation(out=Ee[:, rows, :], in_=qk[:, rows, :],
                                             func=AF.Exp, bias=0.0, scale=1.0,
                                             accum_out=sqh[h][:, 0:1])
                    sq = early.tile([128, 1], F32, tag=f"sq{ib}", name=f"sq{ib}")
                    nc.vector.tensor_add(sq, sqh[0], sqh[1])
                    rq = early.tile([128, 1], F32, tag=f"rq{ib}", name=f"rq{ib}")
                    nc.vector.reciprocal(rq, sq)
                    for c in range(NT):
                        Ech = Ee[:, c * RT:(c + 1) * RT, :]
                        nc.vector.tensor_scalar(out=Ech, in0=Ech, scalar1=rq[:, 0:1],
                                                scalar2=1.0, op0=ALU.mult, op1=ALU.add)
                        nc.gpsimd.tensor_tensor(out=interior(P1[ib], c), in0=Ech,
                                                in1=x3[:, c * RT:(c + 1) * RT, :],
                                                op=ALU.mult)
                    return qk_insts[-1]

                def stats_block(ib, dep=None):
                    x3 = xx[ib][:].rearrange("p (h w) -> p h w", w=SZ)
                    for c in range(NT):
                        i1 = nc.vector.tensor_reduce(out=xmaxp[ib][:, c:c + 1],
                                                     in_=x3[:, c * RT:(c + 1) * RT, :],
                                                     axis=mybir.AxisListType.XY, op=ALU.max)
                        if dep is not None:
                            add_dep_helper(i1.ins, dep.ins, sync=False,
                                           reason="stats off the softmax chain")
                    ssum = early.tile([128, 1], F32, tag=f"ssum{ib}", name=f"ssum{ib}")
                    smax = early.tile([128, 1], F32, tag=f"smax{ib}", name=f"smax{ib}")
                    nc.vector.tensor_reduce(out=ssum, in_=xw[ib][:],
                                            axis=mybir.AxisListType.X, op=ALU.add)
                    nc.vector.tensor_reduce(out=smax, in_=xmaxp[ib][:],
                                            axis=mybir.AxisListType.X, op=ALU.max)
                    nc.scalar.activation(out=avg_b[ib], in_=ssum, func=AF.Copy,
                                         scale=1.0 / HW)
                    nc.scalar.activation(out=max_b[ib], in_=smax, func=AF.Copy)

                load_block(0)
                # ms-conv weights: after block-0's stream, before block-1's
                load_const({"wms", "tms"})
                qk0_last = msam_block(0)
                load_block(1, dep=qk0_last)
                qk1_last = msam_block(1)
                stats_block(0, dep=qk1_last)
                stats_block(1, dep=qk1_last)

                # remaining constants (queued behind the input loads)
                load_const({n for n, _, _ in specs})

            # =============== late phase: convs + mcam + attention ===========
            with tc.tile_pool(name="late", bufs=1) as late, \
                 tc.tile_pool(name="stg", bufs=1) as stg, \
                 tc.tile_pool(name="psp", bufs=8, space="PSUM") as psp:

                def pst(name):
                    return psp.tile([128, 512], F32, tag="ps", name=name)

                def psq(name, shape, dt=None):
                    return psp.tile(list(shape), dt or F32, tag="ps", name=name)

                TAPS_OUTER = bool(int(os.environ.get("KERNEL_TAPS_OUTER", "0")))

                def conv3x3(Pin, wname, n_ob, emit, ib_split=False, filler=None):
                    for ob in range(n_ob):
                        pslist = [pst(f"ps_{wname}_{ob}_{nt}") for nt in range(NT)]
                        ib_order = ((0, 1),) if not ib_split else ((0,), (1,))
                        if TAPS_OUTER:
                            # same stationary weight across all 8 n-tiles:
                            # 8x fewer LDWEIGHTS switches on the PE
                            for ibs in ib_order:
                                for ib in ibs:
                                    for t9 in range(9):
                                        ky, kx = divmod(t9, 3)
                                        lhsT = W[wname][:, ib, t9, ob * 128:(ob + 1) * 128]
                                        for nt in range(NT):
                                            y0 = nt * RT
                                            nc.tensor.matmul(
                                                pslist[nt][:], lhsT=lhsT,
                                                rhs=Pin[ib][:, y0 + ky:y0 + ky + RT, kx:kx + SZ],
                                                start=(ib == 0 and t9 == 0),
                                                stop=(ib == 1 and t9 == 8))
                                            if ib == 1 and t9 == 8:
                                                emit(ob, nt, pslist[nt])
                                    if ib == 1 and filler is not None:
                                        for step in (filler.pop(0) for _ in range(3) if filler):
                                            step()
                        else:
                            for ibs in ib_order:
                                for nt in range(NT):
                                    y0 = nt * RT
                                    for ib in ibs:
                                        for t9 in range(9):
                                            ky, kx = divmod(t9, 3)
                                            nc.tensor.matmul(
                                                pslist[nt][:],
                                                lhsT=W[wname][:, ib, t9, ob * 128:(ob + 1) * 128],
                                                rhs=Pin[ib][:, y0 + ky:y0 + ky + RT, kx:kx + SZ],
                                                start=(ib == 0 and t9 == 0),
                                                stop=(ib == 1 and t9 == 8))
                                    if ibs[-1] == 1:
                                        emit(ob, nt, pslist[nt])
                                    if filler is not None:
                                        for step in (filler.pop(0) for _ in range(2) if filler):
                                            step()

                def emit_ms(ob, nt, ps):
                    pv = interior(P2[ob], nt)
                    nc.scalar.activation(out=pv, in_=ps[:], func=AF.Relu,
                                         bias=W["tms"][:, ob:ob + 1], scale=1.0)
                    relu6(pv)
                conv3x3(P1, "wms", 2, emit_ms, ib_split=True)

                # ---- MCAM chain as filler steps (interleaved into fq conv) ----
                vblocks = [avg_b[0], avg_b[1], max_b[0], max_b[1]]
                g_colb = [late.tile([128, 1], BF16, tag=f"gc{i}", name=f"gc{i}")
                          for i in range(2)]
                g_row = late.tile([1, DIM], F32, tag="g_row", name="g_row")
                h_b = late.tile([64, 1], BF16, tag="h_b", name="h_b")
                y1c = late.tile([NCLS, 1], BF16, tag="y1c", name="y1c")
                f1c = late.tile([NCLS, 1], BF16, tag="f1c", name="f1c")
                rowt = {nm: late.tile([1, NCLS], F32, tag=f"row_{nm}", name=f"row_{nm}")
                        for nm in ("f1", "y1")}
                cfr = late.tile([1, NCLS], BF16, tag="cfr", name="cfr")
                cfc = late.tile([NCLS, 1], F32, tag="cfc", name="cfc")
                g19 = late.tile([NCLS, DIM], F32, tag="g19", name="g19")
                cmT = late.tile([NCLS, DIM], F32, tag="cmT", name="cmT")
                cme = late.tile([NCLS, DIM], BF16, tag="cme", name="cme")
                v_b = late.tile([128, NCLS], BF16, tag="v_b", name="v_b")
                steps = []

                def s_g(ob):
                    def f():
                        pg = psq(f"psg{ob}", (128, 1))
                        for j in range(4):
                            nc.tensor.matmul(pg[:], lhsT=W["wg"][:, j, ob * 128:(ob + 1) * 128],
                                             rhs=vblocks[j], start=(j == 0), stop=(j == 3))
                        nc.scalar.activation(out=g_colb[ob], in_=pg[:], func=AF.Sigmoid,
                                             bias=W["dg"][:, ob:ob + 1], scale=1.0)
                        pr = psq(f"psgr{ob}", (1, 128), BF16)
                        nc.tensor.transpose(pr[:], g_colb[ob][:], W["ident"][:, :])
                        nc.scalar.activation(out=g_row[:, ob * 128:(ob + 1) * 128],
                                             in_=pr[:], func=AF.Copy)
                    return f
                steps += [s_g(0), s_g(1)]

                def s_h():
                    ph = psq("psh", (64, 1))
                    for ib in range(2):
                        nc.tensor.matmul(ph[:], lhsT=W["wcse1"][:, ib, :], rhs=g_colb[ib],
                                         start=(ib == 0), stop=(ib == 1))
                    nc.scalar.activation(out=h_b, in_=ph[:], func=AF.Relu,
                                         bias=W["bcse1"][:, 0:1], scale=1.0)
                steps.append(s_h)

                def s_y1():
                    py = psq("psy", (NCLS, 1))
                    nc.tensor.matmul(py[:], lhsT=W["wcse2"][:, :], rhs=h_b,
                                     start=True, stop=True)
                    nc.scalar.activation(out=y1c, in_=py[:], func=AF.Sigmoid,
                                         bias=W["bcse2"][:, 0:1], scale=1.0)
                steps.append(s_y1)

                def s_f1():
                    pf = psq("psf", (NCLS, 1))
                    for ib in range(2):
                        nc.tensor.matmul(pf[:], lhsT=W["wfc1"][:, ib, :], rhs=g_colb[ib],
                                         start=(ib == 0), stop=(ib == 1))
                    nc.scalar.activation(out=f1c, in_=pf[:], func=AF.Identity,
                                         bias=W["bfc1"][:, 0:1], scale=1.0)
                steps.append(s_f1)

                def s_row(nm, src_t):
                    def f():
                        pr = psq(f"pr_{nm}", (1, NCLS), BF16)
                        nc.tensor.transpose(pr[:], src_t[:], W["ident"][0:NCLS, 0:NCLS])
                        nc.scalar.activation(out=rowt[nm], in_=pr[:], func=AF.Copy)
                    return f
                steps += [s_row("f1", f1c), s_row("y1", y1c)]

                def s_sm1():
                    f1r, y1r = rowt["f1"], rowt["y1"]
                    s1 = late.tile([1, 1], F32, tag="s1", name="s1")
                    e1 = late.tile([1, NCLS], F32, tag="e1", name="e1")
                    nc.scalar.activation(out=e1, in_=f1r[:], func=AF.Exp,
                                         bias=0.0, scale=1.0, accum_out=s1[:, 0:1])
                    r1 = late.tile([1, 1], F32, tag="r1", name="r1")
                    nc.vector.reciprocal(r1, s1)
                    nc.vector.tensor_scalar(out=e1, in0=e1, scalar1=r1[:, 0:1],
                                            scalar2=None, op0=ALU.mult)
                    nc.vector.tensor_add(cfr, e1, y1r)
                steps.append(s_sm1)

                def s_cfc():
                    pcx = psq("pc_cf", (NCLS, 1), BF16)
                    nc.tensor.transpose(pcx[:], cfr[:], W["ident"][0:1, 0:1])
                    nc.scalar.activation(out=cfc, in_=pcx[:], func=AF.Copy)
                    nc.gpsimd.partition_broadcast(g19[:], g_row[:])
                steps.append(s_cfc)

                def s_cm():
                    nc.vector.tensor_scalar(out=cmT, in0=g19, scalar1=cfc[:, 0:1],
                                            scalar2=None, op0=ALU.mult)
                    s2 = late.tile([NCLS, 1], F32, tag="s2", name="s2")
                    nc.scalar.activation(out=cme, in_=cmT[:], func=AF.Exp,
                                         bias=0.0, scale=1.0, accum_out=s2[:, 0:1])
                    r2 = late.tile([NCLS, 1], F32, tag="r2", name="r2")
                    nc.vector.reciprocal(r2, s2)
                    nc.vector.tensor_scalar(out=cme, in0=cme, scalar1=r2[:, 0:1],
                                            scalar2=None, op0=ALU.mult)
                steps.append(s_cm)

                def s_half(ib):
                    def f():
                        pt = psq(f"pt{ib}", (128, NCLS), BF16)
                        nc.tensor.transpose(pt[:], cme[:, ib * 128:(ib + 1) * 128],
                                            W["ident"][0:NCLS, 0:NCLS])
                        pp = late.tile([128, NCLS], F32, tag=f"pp{ib}", name=f"pp{ib}")
                        nc.scalar.activation(out=pp, in_=pt[:], func=AF.Copy)
                        s3 = late.tile([128, 1], F32, tag=f"s3_{ib}", name=f"s3_{ib}")
                        nc.scalar.activation(out=proxy_b[ib], in_=pp[:], func=AF.Exp,
                                             bias=0.0, scale=1.0, accum_out=s3[:, 0:1])
                        r3 = late.tile([128, 1], F32, tag=f"r3_{ib}", name=f"r3_{ib}")
                        nc.vector.reciprocal(r3, s3)
                        nc.vector.tensor_scalar(out=proxy_b[ib], in0=proxy_b[ib],
                                                scalar1=r3[:, 0:1], scalar2=None, op0=ALU.mult)
                    return f
                steps += [s_half(0), s_half(1)]

                def s_kv(wname, tname, dst):
                    def f():
                        pkv = psq(f"pkv_{wname}", (128, NCLS))
                        for ib in range(2):
                            nc.tensor.matmul(pkv[:], lhsT=W[wname][:, ib, :],
                                             rhs=proxy_b[ib], start=(ib == 0), stop=(ib == 1))
                        nc.scalar.activation(out=dst, in_=pkv[:], func=AF.Relu,
                                             bias=W[tname][:, 0:1], scale=1.0)
                        relu6(dst[:])
                    return f
                steps += [s_kv("wfk", "tfk", k_b), s_kv("wfv", "tfv", v_b)]

                def s_vT():
                    pvT = psq("pvT", (NCLS, 128), BF16)
                    nc.tensor.transpose(pvT[:], v_b[:], W["ident"][:, :])
                    nc.scalar.activation(out=vT_b, in_=pvT[:], func=AF.Copy)
                steps.append(s_vT)

                # ---- fq conv: P2 -> q ----
                q_b = late.tile([128, HW], BF16, tag="q_b", name="q_b")
                def emit_fq(ob, nt, ps):
                    pv = q_b[:, nt * 512:(nt + 1) * 512]
                    nc.scalar.activation(out=pv, in_=ps[:], func=AF.Relu,
                                         bias=W["tfq"][:, 0:1], scale=1.0)
                    relu6(pv)
                for _ in range(3):
                    if steps:
                        steps.pop(0)()
                conv3x3(P2, "wfq", 1, emit_fq, filler=steps)
                while steps:
                    steps.pop(0)()

                # attention mm1: (19, 4096) = k^T q
                mm_b = late.tile([NCLS, HW], BF16, tag="mm_b", name="mm_b")
                for nt in range(NT):
                    pm = psp.tile([NCLS, 512], F32, tag="ps", name=f"pmm{nt}")
                    nc.tensor.matmul(pm[:], lhsT=k_b[:], rhs=q_b[:, nt * 512:(nt + 1) * 512],
                                     start=True, stop=True)
                    nc.scalar.activation(out=mm_b[:, nt * 512:(nt + 1) * 512],
                                         in_=pm[:], func=AF.Copy)

                # c3 conv traced now so PE stays busy during the softmax
                sp = [late.tile([128, HW], BF16, tag=f"sp{i}", name=f"sp{i}")
                      for i in range(2)]
                def emit_c3(ob, nt, ps):
                    pv = sp[ob][:, nt * 512:(nt + 1) * 512]
                    nc.scalar.activation(out=pv, in_=ps[:], func=AF.Relu,
                                         bias=W["tc3"][:, ob:ob + 1], scale=1.0)
                    relu6(pv)
                conv3x3(P2, "wc3", 2, emit_c3)

                # softmax over hw rows of mm (inputs are bounded: no max-sub needed)
                sa_s = late.tile([NCLS, 1], F32, tag="sa_s", name="sa_s")
                A_b = late.tile([NCLS, HW], BF16, tag="A_b", name="A_b")
                nc.scalar.activation(out=A_b, in_=mm_b[:], func=AF.Exp,
                                     bias=0.0, scale=S_ATT, accum_out=sa_s[:, 0:1])
                ra = late.tile([NCLS, 1], F32, tag="ra", name="ra")
                nc.vector.reciprocal(ra, sa_s)
                nc.vector.tensor_scalar(out=A_b, in0=A_b, scalar1=ra[:, 0:1],
                                        scalar2=None, op0=ALU.mult)

                # mm2 + fup + add sa -> P3
                ctx_b = late.tile([128, HW], BF16, tag="ctx_b", name="ctx_b")
                for nt in range(NT):
                    pc2 = pst(f"pctx{nt}")
                    nc.tensor.matmul(pc2[:], lhsT=vT_b[:], rhs=A_b[:, nt * 512:(nt + 1) * 512],
                                     start=True, stop=True)
                    nc.scalar.activation(out=ctx_b[:, nt * 512:(nt + 1) * 512],
                                         in_=pc2[:], func=AF.Copy)
                for ob in range(2):
                    for nt in range(NT):
                        pu = pst(f"pfup{ob}_{nt}")
                        nc.tensor.matmul(pu[:], lhsT=W["wfup"][:, ob * 128:(ob + 1) * 128],
                                         rhs=ctx_b[:, nt * 512:(nt + 1) * 512],
                                         start=True, stop=True)
                        fs = stg.tile([128, 512], BF16, tag="fs", bufs=3,
                                      name=f"fs{ob}_{nt}")
                        nc.scalar.activation(out=fs, in_=pu[:], func=AF.Relu,
                                             bias=W["tfup"][:, ob:ob + 1], scale=1.0)
                        nc.vector.scalar_tensor_tensor(
                            out=interior(P3[ob], nt), in0=fs, scalar=6.0,
                            in1=interior(P2[ob], nt), op0=ALU.min, op1=ALU.add)

                # fuse conv: P3 -> ctxf
                ctxf = [late.tile([128, HW], BF16, tag=f"ctxf{i}", name=f"ctxf{i}")
                        for i in range(2)]
                def emit_fuse(ob, nt, ps):
                    pv = ctxf[ob][:, nt * 512:(nt + 1) * 512]
                    nc.scalar.activation(out=pv, in_=ps[:], func=AF.Relu,
                                         bias=W["tfuse"][:, ob:ob + 1], scale=1.0)
                    relu6(pv)
                conv3x3(P3, "wfuse", 2, emit_fuse)

                # c1 1x1 over concat([ctxf, sp]) -> out
                cat = [ctxf[0], ctxf[1], sp[0], sp[1]]
                for ob in range(2):
                    for nt in range(NT):
                        ps = pst(f"pc1_{ob}_{nt}")
                        for j in range(4):
                            nc.tensor.matmul(ps[:], lhsT=W["wc1"][:, j, ob * 128:(ob + 1) * 128],
                                             rhs=cat[j][:, nt * 512:(nt + 1) * 512],
                                             start=(j == 0), stop=(j == 3))
                        og = stg.tile([128, 512], F32, tag="og", bufs=4,
                                      name=f"og{ob}_{nt}")
                        nc.scalar.activation(out=og, in_=ps[:], func=AF.Relu,
                                             bias=W["tc1"][:, ob:ob + 1], scale=1.0)
                        relu6(og[:])
                        nc.sync.dma_start(
                            out=out_d.ap()[ob * 128:(ob + 1) * 128, nt * 512:(nt + 1) * 512],
                            in_=og)

    nc.compile()
    return nc


# ------------------------------------------------------------------- wrapper

_CACHE = {}


def kernel(x, skip, params):
    from concourse import bass_utils

    x = np.asarray(x, np.float32)
    skip = np.asarray(skip, np.float32)
    B = x.shape[0]
    packed = _prep(params)

    if "nc" not in _CACHE:
        _CACHE["nc"] = build_program()
    nc = _CACHE["nc"]

    in_maps = []
    for i in range(B):
        m = dict(packed)
        m["x"] = np.ascontiguousarray(x[i].reshape(DIM, HW))
        m["skip"] = np.ascontiguousarray(skip[i].reshape(DIM, HW))
        in_maps.append(m)

    trace = bool(int(os.environ.get("KBENCH_TRACE", "0")))
    try:
        res = bass_utils.run_bass_kernel_spmd(
            nc, in_maps, core_ids=list(range(B)), trace=trace)
    except ModuleNotFoundError:
        # axon NTFF profiling hook unavailable in this environment
        os.environ["BASS_NEVER_TRACE"] = "1"
        res = bass_utils.run_bass_kernel_spmd(
            nc, in_maps, core_ids=list(range(B)), trace=False)
    _CACHE["last_result"] = res
    out = np.stack([r["out"].reshape(DIM, SZ, SZ) for r in res.results])
    return out.astype(np.float32)


# revision 36
# speedup vs baseline: 18.9168x; 1.0862x over previous
"""Trainium2 Bass kernel for nn_Attention (MCAM + MSAM + CIAFM block).

Sharding: pure data parallelism — B=8 samples across 8 NeuronCores.
Per core: x,skip (256, 64, 64) f32 -> out (256, 64, 64) f32.

Heavy compute = four 3x3 convs (ms, fq, fuse, c3) done as 9-tap
PSUM-accumulated bf16 matmuls over zero-padded (128, 66, 66) SBUF tiles,
plus 1x1 convs (fup, c1) and a tiny NC=19 cross-attention.

Host-side preprocessing folds:
  - BN scales into conv weights (cbr -> relu6(conv(x, W*s) + t))
  - the entire MCAM front end (4 ECA conv1ds + k=3 mixer + FC) into one
    linear map  g_pre = M @ [avgpool; maxpool] + d   (M: 256x512)

Schedule: inputs stream in per 512-column chunk (adds on GPSIMD, partial
pool stats on DVE during the DMA window); MSAM softmax+modulation feeds
the ms conv per chunk; the ms conv is split by input channel block so its
block-0 matmuls start before block-1's modulated input is finished. The
serial MCAM chain (tiny) is traced between ms and fq so it never blocks
the PE FIFO; c3 is traced before the attention softmax for the same
reason.
"""

import os
import numpy as np
import ml_dtypes

BF = ml_dtypes.bfloat16

DIM, NCLS, SZ = 256, 19, 64
KC = 128
HW = SZ * SZ          # 4096
PD = SZ + 2           # 66
NT = 8                # 512-wide output column tiles
RT = SZ // NT         # 8 rows per tile
S_ATT = float(KC) ** -0.5


# ------------------------------------------------------------------ host prep

def _toeplitz(w, n):
    """Dense matrix of 'same'-padded 1-D cross-correlation with kernel w."""
    w = np.asarray(w, np.float64).reshape(-1)
    k = len(w)
    pad = (k - 1) // 2
    T = np.zeros((n, n), np.float64)
    for j in range(k):
        d = j - pad
        lo, hi = max(0, -d), min(n, n - d)
        idx = np.arange(lo, hi)
        T[idx, idx + d] += w[j]
    return T


def _pack_conv(w, scale=None):
    """(O, I, kh, kw) -> (128, I//128, kh*kw, O): lhsT tiles per (in-block, tap)."""
    w = np.asarray(w, np.float64)
    if scale is not None:
        w = w * np.asarray(scale, np.float64)[:, None, None, None]
    O, I, kh, kw = w.shape
    t = w.reshape(O, I, kh * kw).transpose(1, 2, 0)          # (I, taps, O)
    t = t.reshape(I // 128, 128, kh * kw, O).transpose(1, 0, 2, 3)
    return np.ascontiguousarray(t)


def _bf16(a):
    return np.ascontiguousarray(np.asarray(a, np.float32)).astype(BF)


def _f32(a):
    return np.ascontiguousarray(np.asarray(a, np.float32))


def _cols(v, nb):
    """(nb*128,) bias vector -> (128, nb): column ob = v[ob*128:(ob+1)*128]."""
    return _f32(np.asarray(v, np.float64).reshape(nb, 128).T)


def _prep(params):
    p = {k: np.asarray(v, np.float64) for k, v in params.items()}
    o = {}

    # --- fused MCAM front end: g_pre = M @ [avg; max] + d ---
    T1 = np.zeros((8 * DIM, 2 * DIM))
    b1 = np.zeros(8 * DIM)
    for br in range(2):                       # 0 = avg branch, 1 = max branch
        for kk, nm in enumerate(("mc0", "mc1", "mc2", "mc3")):
            r = (br * 4 + kk) * DIM
            T1[r:r + DIM, br * DIM:(br + 1) * DIM] = _toeplitz(p[nm + "_w"], DIM)
            b1[r:r + DIM] = p[nm + "_b"][0]
    T2 = _toeplitz(p["mcc_w"], 8 * DIM)
    bias2 = T2 @ b1 + p["mcc_b"][0]
    M = p["fc_w"] @ T2 @ T1                   # (256, 512)
    d = p["fc_w"] @ bias2 + p["fc_b"]         # (256,)
    o["wg"] = _bf16(M.T.reshape(4, 128, DIM).transpose(1, 0, 2))   # (128,4,256)
    o["dg"] = _cols(d, 2)                                          # (128,2)

    # --- mcam mid (cse1/cse2/fc1) ---
    o["wcse1"] = _bf16(_pack_conv(p["cse1_w"]))[:, :, 0, :]        # (128,2,64)
    o["bcse1"] = _f32(p["cse1_b"].reshape(64, 1))
    o["wcse2"] = _bf16(p["cse2_w"][:, :, 0, 0].T)                  # (64,19)
    o["bcse2"] = _f32(p["cse2_b"].reshape(NCLS, 1))
    o["wfc1"] = _bf16(_pack_conv(p["fc1_w"], p["fc1_s"]))[:, :, 0, :]  # (128,2,19)
    o["bfc1"] = _f32(p["fc1_t"].reshape(NCLS, 1))

    # --- conv weights, BN scale folded ---
    o["wms"] = _bf16(_pack_conv(p["ms_w"], p["ms_s"]))             # (128,2,9,256)
    o["tms"] = _cols(p["ms_t"], 2)
    # fq weights in fp8 e4m3, DoubleRow layout (Ki=128, tap, Ko=2, M=128):
    # the attention branch is insensitive (8% q noise -> 3e-7 output change)
    wfq = _pack_conv(p["fq_w"], p["fq_s"])                         # (128,2,9,128)
    o["wfq8"] = np.ascontiguousarray(
        wfq.transpose(0, 2, 1, 3)).astype(np.float32).astype(
        ml_dtypes.float8_e4m3)                                     # (128,9,2,128)
    o["tfq"] = _f32(p["fq_t"].reshape(1, 128).T)                   # (128,1)
    o["wfk"] = _bf16(_pack_conv(p["fk_w"], p["fk_s"]))[:, :, 0, :]  # (128,2,128)
    o["tfk"] = _f32(p["fk_t"].reshape(1, 128).T)
    o["wfv"] = _bf16(_pack_conv(p["fv_w"], p["fv_s"]))[:, :, 0, :]
    o["tfv"] = _f32(p["fv_t"].reshape(1, 128).T)
    o["wfup"] = _bf16(_pack_conv(p["fup_w"], p["fup_s"]))[:, 0, 0, :]  # (128,256)
    o["tfup"] = _cols(p["fup_t"], 2)
    o["wfuse"] = _bf16(_pack_conv(p["fuse_w"], p["fuse_s"]))
    o["tfuse"] = _cols(p["fuse_t"], 2)
    o["wc3"] = _bf16(_pack_conv(p["c3_w"], p["c3_s"]))
    o["tc3"] = _cols(p["c3_t"], 2)
    o["wc1"] = _bf16(_pack_conv(p["c1_w"], p["c1_s"]))[:, :, 0, :]  # (128,4,256)
    o["tc1"] = _cols(p["c1_t"], 2)

    o["ident"] = _bf16(np.eye(128))
    return o


# ------------------------------------------------------------- device program

def build_program():
    import concourse.tile as tile
    from concourse import bacc, mybir

    AF = mybir.ActivationFunctionType
    ALU = mybir.AluOpType
    F32 = mybir.dt.float32
    BF16 = mybir.dt.bfloat16

    nc = bacc.Bacc("TRN2", target_bir_lowering=False, debug=False)

    specs = [
        ("x", (DIM, HW), F32), ("skip", (DIM, HW), F32),
        ("wms", (128, 2, 9, DIM), BF16), ("tms", (128, 2), F32),
        ("wfq8", (128, 9, 2, KC), mybir.dt.float8e4), ("tfq", (128, 1), F32),
        ("wg", (128, 4, DIM), BF16), ("dg", (128, 2), F32),
        ("wcse1", (128, 2, 64), BF16), ("bcse1", (64, 1), F32),
        ("wcse2", (64, NCLS), BF16), ("bcse2", (NCLS, 1), F32),
        ("wfc1", (128, 2, NCLS), BF16), ("bfc1", (NCLS, 1), F32),
        ("wfk", (128, 2, KC), BF16), ("tfk", (128, 1), F32),
        ("wfv", (128, 2, KC), BF16), ("tfv", (128, 1), F32),
        ("wfup", (128, DIM), BF16), ("tfup", (128, 2), F32),
        ("wfuse", (128, 2, 9, DIM), BF16), ("tfuse", (128, 2), F32),
        ("wc3", (128, 2, 9, DIM), BF16), ("tc3", (128, 2), F32),
        ("wc1", (128, 4, DIM), BF16), ("tc1", (128, 2), F32),
        ("ident", (128, 128), BF16),
    ]
    dram = {n: nc.dram_tensor(n, list(s), dt, kind="ExternalInput")
            for n, s, dt in specs}
    out_d = nc.dram_tensor("out", [DIM, HW], F32, kind="ExternalOutput")

    from concourse.tile import add_dep_helper

    with tile.TileContext(nc) as tc:
        with tc.tile_pool(name="cst", bufs=1) as cst, \
             tc.tile_pool(name="glob", bufs=1) as glob:

            # ---- padded conv-input buffers ----
            def padded(tagname):
                ts = [glob.tile([128, PD, PD], BF16, tag=f"{tagname}{i}",
                                name=f"{tagname}{i}") for i in range(2)]
                for t in ts:
                    nc.gpsimd.memset(t[:, 0, :], 0.0)
                    nc.gpsimd.memset(t[:, PD - 1, :], 0.0)
                    nc.gpsimd.memset(t[:, 1:PD - 1, 0], 0.0)
                    nc.gpsimd.memset(t[:, 1:PD - 1, PD - 1], 0.0)
                return ts

            P1 = padded("P1")   # msam y  (ms conv input)
            P2 = padded("P2")   # sa      (fq / c3 / fuse-add input)
            P3 = padded("P3")   # fup_out + sa (fuse conv input)

            def interior(P, nt=None):
                if nt is None:
                    return P[:, 1:1 + SZ, 1:1 + SZ]
                return P[:, 1 + nt * RT:1 + (nt + 1) * RT, 1:1 + SZ]

            # small tensors that cross the early/late phase boundary
            def gt(shape, dt, nm):
                return glob.tile(list(shape), dt, tag=nm, name=nm)

            proxy_b = [gt((128, NCLS), BF16, f"proxy{i}") for i in range(2)]
            k_b = gt((128, NCLS), BF16, "k_b")
            vT_b = gt((NCLS, 128), BF16, "vT_b")
            avg_b = [gt((128, 1), BF16, f"ab{i}") for i in range(2)]
            max_b = [gt((128, 1), BF16, f"mb{i}") for i in range(2)]

            relu6 = lambda ap: nc.vector.tensor_scalar_min(ap, ap, 6.0)

            # ============ early phase: stream inputs, MSAM -> P1 ============
            W = {}

            def load_const(names):
                for n, s, dt in specs:
                    if n in ("x", "skip") or n in W or n not in names:
                        continue
                    t = cst.tile(list(s), dt, tag=n, name=f"c_{n}")
                    nc.sync.dma_start(out=t, in_=dram[n].ap())
                    W[n] = t

            with tc.tile_pool(name="early", bufs=1) as early:
                xx = [early.tile([128, HW], F32, tag=f"xx{i}", name=f"xx{i}")
                      for i in range(2)]
                xw = [early.tile([128, SZ], F32, tag=f"xw{i}", name=f"xw{i}")
                      for i in range(2)]
                xhs = [early.tile([128, SZ], F32, tag=f"xhs{i}", name=f"xhs{i}")
                       for i in range(2)]
                xhp_all = [early.tile([128, SZ, NT], F32, tag=f"xhp{i}", name=f"xhp{i}")
                           for i in range(2)]
                xmaxp = [early.tile([128, NT], F32, tag=f"xmaxp{i}", name=f"xmaxp{i}")
                         for i in range(2)]

                def load_block(ib, dep=None):
                    x3 = xx[ib][:].rearrange("p (h w) -> p h w", w=SZ)
                    for c in range(NT):
                        sl = slice(c * 512, (c + 1) * 512)
                        xt = early.tile([128, 512], F32, tag="xt", bufs=6,
                                        name=f"xt{ib}_{c}")
                        st = early.tile([128, 512], F32, tag="st", bufs=6,
                                        name=f"st{ib}_{c}")
                        nc.sync.dma_start(out=xt, in_=dram["x"].ap()[ib * 128:(ib + 1) * 128, sl])
                        nc.sync.dma_start(out=st, in_=dram["skip"].ap()[ib * 128:(ib + 1) * 128, sl])
                        nc.gpsimd.tensor_add(xx[ib][:, sl], xt, st)
                        ch3 = x3[:, c * RT:(c + 1) * RT, :]
                        i1 = nc.vector.tensor_reduce(out=xw[ib][:, c * RT:(c + 1) * RT],
                                                     in_=ch3, axis=mybir.AxisListType.X,
                                                     op=ALU.add)
                        # column-sum partial for x_h: reduce the chunk's 8 rows
                        ch3t = xx[ib][:, sl].rearrange("p (h w) -> p w h", w=SZ)
                        i2 = nc.vector.tensor_reduce(out=xhp_all[ib][:, :, c], in_=ch3t,
                                                     axis=mybir.AxisListType.X, op=ALU.add)
                        if dep is not None:
                            add_dep_helper(i1.ins, dep.ins, sync=False,
                                           reason="keep b1 partials off the b0 softmax chain")
                            add_dep_helper(i2.ins, dep.ins, sync=False,
                                           reason="keep b1 partials off the b0 softmax chain")

                def msam_block(ib):
                    x3 = xx[ib][:].rearrange("p (h w) -> p h w", w=SZ)
                    xh = xhs[ib]
                    nc.vector.tensor_reduce(out=xh, in_=xhp_all[ib][:], 
                                            axis=mybir.AxisListType.X, op=ALU.add)
                    qk = early.tile([128, SZ, SZ], BF16, tag="qk", name=f"qk{ib}")
                    Ee = early.tile([128, SZ, SZ], F32, tag="Ee", name=f"Ee{ib}")
                    sqh = [early.tile([128, 1], F32, tag=f"sqh{h}", name=f"sq{ib}_{h}")
                           for h in range(2)]
                    qk_insts = []
                    for h in range(2):
                        rows = slice(h * (SZ // 2), (h + 1) * (SZ // 2))
                        qk_insts.append(nc.vector.scalar_tensor_tensor(
                            out=qk[:, rows, :],
                            in0=xw[ib][:, rows].unsqueeze(2).broadcast_to([128, SZ // 2, SZ]),
                            scalar=1.0 / (HW * 1.0),
                            in1=xh[:].unsqueeze(1).broadcast_to([128, SZ // 2, SZ]),
                            op0=ALU.mult, op1=ALU.mult))
                        nc.scalar.activation(out=Ee[:, rows, :], in_=qk[:, rows, :],
                                             func=AF.Exp, bias=0.0, scale=1.0,
                                             accum_out=sqh[h][:, 0:1])
                    sq = early.tile([128, 1], F32, tag=f"sq{ib}", name=f"sq{ib}")
                    nc.vector.tensor_add(sq, sqh[0], sqh[1])
                    rq = early.tile([128, 1], F32, tag=f"rq{ib}", name=f"rq{ib}")
                    nc.vector.reciprocal(rq, sq)
                    for c in range(NT):
                        Ech = Ee[:, c * RT:(c + 1) * RT, :]
                        nc.vector.tensor_scalar(out=Ech, in0=Ech, scalar1=rq[:, 0:1],
                                                scalar2=1.0, op0=ALU.mult, op1=ALU.add)
                        nc.gpsimd.tensor_tensor(out=interior(P1[ib], c), in0=Ech,
                                                in1=x3[:, c * RT:(c + 1) * RT, :],
                                                op=ALU.mult)
                    return qk_insts[-1]

                def stats_block(ib, dep=None):
                    x3 = xx[ib][:].rearrange("p (h w) -> p h w", w=SZ)
                    for c in range(NT):
                        i1 = nc.vector.tensor_reduce(out=xmaxp[ib][:, c:c + 1],
                                                     in_=x3[:, c * RT:(c + 1) * RT, :],
                                                     axis=mybir.AxisListType.XY, op=ALU.max)
                        if dep is not None:
                            add_dep_helper(i1.ins, dep.ins, sync=False,
                                           reason="stats off the softmax chain")
                    ssum = early.tile([128, 1], F32, tag=f"ssum{ib}", name=f"ssum{ib}")
                    smax = early.tile([128, 1], F32, tag=f"smax{ib}", name=f"smax{ib}")
                    nc.vector.tensor_reduce(out=ssum, in_=xw[ib][:],
                                            axis=mybir.AxisListType.X, op=ALU.add)
                    nc.vector.tensor_reduce(out=smax, in_=xmaxp[ib][:],
                                            axis=mybir.AxisListType.X, op=ALU.max)
                    nc.scalar.activation(out=avg_b[ib], in_=ssum, func=AF.Copy,
                                         scale=1.0 / HW)
                    nc.scalar.activation(out=max_b[ib], in_=smax, func=AF.Copy)

                load_block(0)
                # ms-conv weights: after block-0's stream, before block-1's
                load_const({"wms", "tms"})
                qk0_last = msam_block(0)
                load_block(1, dep=qk0_last)
                qk1_last = msam_block(1)
                stats_block(0, dep=qk1_last)
                stats_block(1, dep=qk1_last)

                # remaining constants (queued behind the input loads)
                load_const({n for n, _, _ in specs})

            # =============== late phase: convs + mcam + attention ===========
            with tc.tile_pool(name="late", bufs=1) as late, \
                 tc.tile_pool(name="stg", bufs=1) as stg, \
                 tc.tile_pool(name="psp", bufs=8, space="PSUM") as psp:

                def pst(name):
                    return psp.tile([128, 512], F32, tag="ps", name=name)

                def psq(name, shape, dt=None):
                    return psp.tile(list(shape), dt or F32, tag="ps", name=name)

                TAPS_OUTER = bool(int(os.environ.get("KERNEL_TAPS_OUTER", "0")))

                def conv3x3(Pin, wname, n_ob, emit, ib_split=False, filler=None,
                            obs=None):
                    for ob in (range(n_ob) if obs is None else obs):
                        pslist = [pst(f"ps_{wname}_{ob}_{nt}") for nt in range(NT)]
                        ib_order = ((0, 1),) if not ib_split else ((0,), (1,))
                        if TAPS_OUTER:
                            # same stationary weight across all 8 n-tiles:
                            # 8x fewer LDWEIGHTS switches on the PE
                            for ibs in ib_order:
                                for ib in ibs:
                                    for t9 in range(9):
                                        ky, kx = divmod(t9, 3)
                                        lhsT = W[wname][:, ib, t9, ob * 128:(ob + 1) * 128]
                                        for nt in range(NT):
                                            y0 = nt * RT
                                            nc.tensor.matmul(
                                                pslist[nt][:], lhsT=lhsT,
                                                rhs=Pin[ib][:, y0 + ky:y0 + ky + RT, kx:kx + SZ],
                                                start=(ib == 0 and t9 == 0),
                                                stop=(ib == 1 and t9 == 8))
                                            if ib == 1 and t9 == 8:
                                                emit(ob, nt, pslist[nt])
                                    if ib == 1 and filler is not None:
                                        for step in (filler.pop(0) for _ in range(3) if filler):
                                            step()
                        else:
                            for ibs in ib_order:
                                for nt in range(NT):
                                    y0 = nt * RT
                                    for ib in ibs:
                                        for t9 in range(9):
                                            ky, kx = divmod(t9, 3)
                                            nc.tensor.matmul(
                                                pslist[nt][:],
                                                lhsT=W[wname][:, ib, t9, ob * 128:(ob + 1) * 128],
                                                rhs=Pin[ib][:, y0 + ky:y0 + ky + RT, kx:kx + SZ],
                                                start=(ib == 0 and t9 == 0),
                                                stop=(ib == 1 and t9 == 8))
                                    if ibs[-1] == 1:
                                        emit(ob, nt, pslist[nt])
                                    if filler is not None:
                                        for step in (filler.pop(0) for _ in range(2) if filler):
                                            step()

                # flat fp8 copy of sa (+zero margins) for the DoubleRow fq conv
                MARG = 80
                q8 = late.tile([128, 2, MARG + HW + MARG], mybir.dt.float8e4,
                               tag="q8", name="q8")
                nc.gpsimd.memset(q8[:, :, 0:MARG], 0.0)
                nc.gpsimd.memset(q8[:, :, MARG + HW:], 0.0)

                def emit_ms(ob, nt, ps):
                    pv = interior(P2[ob], nt)
                    nc.scalar.activation(out=pv, in_=ps[:], func=AF.Relu,
                                         bias=W["tms"][:, ob:ob + 1], scale=1.0)
                    relu6(pv)
                    nc.vector.tensor_copy(
                        q8[:, ob, MARG + nt * 512:MARG + (nt + 1) * 512], pv)
                conv3x3(P1, "wms", 2, emit_ms, ib_split=True)

                # ---- MCAM chain as filler steps (interleaved into fq conv) ----
                vblocks = [avg_b[0], avg_b[1], max_b[0], max_b[1]]
                g_colb = [late.tile([128, 1], BF16, tag=f"gc{i}", name=f"gc{i}")
                          for i in range(2)]
                g_row = late.tile([1, DIM], F32, tag="g_row", name="g_row")
                h_b = late.tile([64, 1], BF16, tag="h_b", name="h_b")
                y1c = late.tile([NCLS, 1], BF16, tag="y1c", name="y1c")
                f1c = late.tile([NCLS, 1], BF16, tag="f1c", name="f1c")
                rowt = {nm: late.tile([1, NCLS], F32, tag=f"row_{nm}", name=f"row_{nm}")
                        for nm in ("f1", "y1")}
                cfr = late.tile([1, NCLS], BF16, tag="cfr", name="cfr")
                cfc = late.tile([NCLS, 1], F32, tag="cfc", name="cfc")
                g19 = late.tile([NCLS, DIM], F32, tag="g19", name="g19")
                cmT = late.tile([NCLS, DIM], F32, tag="cmT", name="cmT")
                cme = late.tile([NCLS, DIM], BF16, tag="cme", name="cme")
                v_b = late.tile([128, NCLS], BF16, tag="v_b", name="v_b")
                steps = []

                def s_g(ob):
                    def f():
                        pg = psq(f"psg{ob}", (128, 1))
                        for j in range(4):
                            nc.tensor.matmul(pg[:], lhsT=W["wg"][:, j, ob * 128:(ob + 1) * 128],
                                             rhs=vblocks[j], start=(j == 0), stop=(j == 3))
                        nc.scalar.activation(out=g_colb[ob], in_=pg[:], func=AF.Sigmoid,
                                             bias=W["dg"][:, ob:ob + 1], scale=1.0)
                        pr = psq(f"psgr{ob}", (1, 128), BF16)
                        nc.tensor.transpose(pr[:], g_colb[ob][:], W["ident"][:, :])
                        nc.scalar.activation(out=g_row[:, ob * 128:(ob + 1) * 128],
                                             in_=pr[:], func=AF.Copy)
                    return f
                steps += [s_g(0), s_g(1)]

                def s_h():
                    ph = psq("psh", (64, 1))
                    for ib in range(2):
                        nc.tensor.matmul(ph[:], lhsT=W["wcse1"][:, ib, :], rhs=g_colb[ib],
                                         start=(ib == 0), stop=(ib == 1))
                    nc.scalar.activation(out=h_b, in_=ph[:], func=AF.Relu,
                                         bias=W["bcse1"][:, 0:1], scale=1.0)
                steps.append(s_h)

                def s_y1():
                    py = psq("psy", (NCLS, 1))
                    nc.tensor.matmul(py[:], lhsT=W["wcse2"][:, :], rhs=h_b,
                                     start=True, stop=True)
                    nc.scalar.activation(out=y1c, in_=py[:], func=AF.Sigmoid,
                                         bias=W["bcse2"][:, 0:1], scale=1.0)
                steps.append(s_y1)

                def s_f1():
                    pf = psq("psf", (NCLS, 1))
                    for ib in range(2):
                        nc.tensor.matmul(pf[:], lhsT=W["wfc1"][:, ib, :], rhs=g_colb[ib],
                                         start=(ib == 0), stop=(ib == 1))
                    nc.scalar.activation(out=f1c, in_=pf[:], func=AF.Identity,
                                         bias=W["bfc1"][:, 0:1], scale=1.0)
                steps.append(s_f1)

                def s_row(nm, src_t):
                    def f():
                        pr = psq(f"pr_{nm}", (1, NCLS), BF16)
                        nc.tensor.transpose(pr[:], src_t[:], W["ident"][0:NCLS, 0:NCLS])
                        nc.scalar.activation(out=rowt[nm], in_=pr[:], func=AF.Copy)
                    return f
                steps += [s_row("f1", f1c), s_row("y1", y1c)]

                def s_sm1():
                    f1r, y1r = rowt["f1"], rowt["y1"]
                    s1 = late.tile([1, 1], F32, tag="s1", name="s1")
                    e1 = late.tile([1, NCLS], F32, tag="e1", name="e1")
                    nc.scalar.activation(out=e1, in_=f1r[:], func=AF.Exp,
                                         bias=0.0, scale=1.0, accum_out=s1[:, 0:1])
                    r1 = late.tile([1, 1], F32, tag="r1", name="r1")
                    nc.vector.reciprocal(r1, s1)
                    nc.vector.tensor_scalar(out=e1, in0=e1, scalar1=r1[:, 0:1],
                                            scalar2=None, op0=ALU.mult)
                    nc.vector.tensor_add(cfr, e1, y1r)
                steps.append(s_sm1)

                def s_cfc():
                    pcx = psq("pc_cf", (NCLS, 1), BF16)
                    nc.tensor.transpose(pcx[:], cfr[:], W["ident"][0:1, 0:1])
                    nc.scalar.activation(out=cfc, in_=pcx[:], func=AF.Copy)
                    nc.gpsimd.partition_broadcast(g19[:], g_row[:])
                steps.append(s_cfc)

                def s_cm():
                    nc.vector.tensor_scalar(out=cmT, in0=g19, scalar1=cfc[:, 0:1],
                                            scalar2=None, op0=ALU.mult)
                    s2 = late.tile([NCLS, 1], F32, tag="s2", name="s2")
                    nc.scalar.activation(out=cme, in_=cmT[:], func=AF.Exp,
                                         bias=0.0, scale=1.0, accum_out=s2[:, 0:1])
                    r2 = late.tile([NCLS, 1], F32, tag="r2", name="r2")
                    nc.vector.reciprocal(r2, s2)
                    nc.vector.tensor_scalar(out=cme, in0=cme, scalar1=r2[:, 0:1],
                                            scalar2=None, op0=ALU.mult)
                steps.append(s_cm)

                def s_half(ib):
                    def f():
                        pt = psq(f"pt{ib}", (128, NCLS), BF16)
                        nc.tensor.transpose(pt[:], cme[:, ib * 128:(ib + 1) * 128],
                                            W["ident"][0:NCLS, 0:NCLS])
                        pp = late.tile([128, NCLS], F32, tag=f"pp{ib}", name=f"pp{ib}")
                        nc.scalar.activation(out=pp, in_=pt[:], func=AF.Copy)
                        s3 = late.tile([128, 1], F32, tag=f"s3_{ib}", name=f"s3_{ib}")
                        nc.scalar.activation(out=proxy_b[ib], in_=pp[:], func=AF.Exp,
                                             bias=0.0, scale=1.0, accum_out=s3[:, 0:1])
                        r3 = late.tile([128, 1], F32, tag=f"r3_{ib}", name=f"r3_{ib}")
                        nc.vector.reciprocal(r3, s3)
                        nc.vector.tensor_scalar(out=proxy_b[ib], in0=proxy_b[ib],
                                                scalar1=r3[:, 0:1], scalar2=None, op0=ALU.mult)
                    return f
                steps += [s_half(0), s_half(1)]

                def s_kv(wname, tname, dst):
                    def f():
                        pkv = psq(f"pkv_{wname}", (128, NCLS))
                        for ib in range(2):
                            nc.tensor.matmul(pkv[:], lhsT=W[wname][:, ib, :],
                                             rhs=proxy_b[ib], start=(ib == 0), stop=(ib == 1))
                        nc.scalar.activation(out=dst, in_=pkv[:], func=AF.Relu,
                                             bias=W[tname][:, 0:1], scale=1.0)
                        relu6(dst[:])
                    return f
                steps += [s_kv("wfk", "tfk", k_b), s_kv("wfv", "tfv", v_b)]

                def s_vT():
                    pvT = psq("pvT", (NCLS, 128), BF16)
                    nc.tensor.transpose(pvT[:], v_b[:], W["ident"][:, :])
                    nc.scalar.activation(out=vT_b, in_=pvT[:], func=AF.Copy)
                steps.append(s_vT)

                # ---- fq conv: P2 -> q ----
                q_b = late.tile([128, HW], BF16, tag="q_b", name="q_b")
                def emit_fq(ob, nt, ps):
                    pv = q_b[:, nt * 512:(nt + 1) * 512]
                    nc.scalar.activation(out=pv, in_=ps[:], func=AF.Relu,
                                         bias=W["tfq"][:, 0:1], scale=1.0)
                    relu6(pv)
                for _ in range(3):
                    if steps:
                        steps.pop(0)()
                for nt in range(NT):
                    ps = pst(f"ps_fq_{nt}")
                    for t9 in range(9):
                        ky, kx = divmod(t9, 3)
                        off = (ky - 1) * SZ + (kx - 1)
                        base = MARG + nt * 512 + off
                        nc.tensor.matmul(
                            ps[:], lhsT=W["wfq8"][:, t9, :, :],
                            rhs=q8[:, :, base:base + 512],
                            start=(t9 == 0), stop=(t9 == 8),
                            perf_mode=mybir.MatmulPerfMode.DoubleRow)
                    emit_fq(0, nt, ps)

                # c3 conv block 0: covers the tail of the MCAM chain
                sp = [late.tile([128, HW], BF16, tag=f"sp{i}", name=f"sp{i}")
                      for i in range(2)]
                def emit_c3(ob, nt, ps):
                    pv = sp[ob][:, nt * 512:(nt + 1) * 512]
                    nc.scalar.activation(out=pv, in_=ps[:], func=AF.Relu,
                                         bias=W["tc3"][:, ob:ob + 1], scale=1.0)
                    relu6(pv)
                conv3x3(P2, "wc3", 2, emit_c3, obs=(0,), filler=steps)
                while steps:
                    steps.pop(0)()

                # attention mm1: (19, 4096) = k^T q
                mm_b = late.tile([NCLS, HW], BF16, tag="mm_b", name="mm_b")
                for nt in range(NT):
                    pm = psp.tile([NCLS, 512], F32, tag="ps", name=f"pmm{nt}")
                    nc.tensor.matmul(pm[:], lhsT=k_b[:], rhs=q_b[:, nt * 512:(nt + 1) * 512],
                                     start=True, stop=True)
                    nc.scalar.activation(out=mm_b[:, nt * 512:(nt + 1) * 512],
                                         in_=pm[:], func=AF.Copy)

                # c3 conv block 1: covers the attention softmax
                conv3x3(P2, "wc3", 2, emit_c3, obs=(1,))

                # softmax over hw rows of mm (inputs are bounded: no max-sub needed)
                sa_s = late.tile([NCLS, 1], F32, tag="sa_s", name="sa_s")
                A_b = late.tile([NCLS, HW], BF16, tag="A_b", name="A_b")
                nc.scalar.activation(out=A_b, in_=mm_b[:], func=AF.Exp,
                                     bias=0.0, scale=S_ATT, accum_out=sa_s[:, 0:1])
                ra = late.tile([NCLS, 1], F32, tag="ra", name="ra")
                nc.vector.reciprocal(ra, sa_s)
                nc.vector.tensor_scalar(out=A_b, in0=A_b, scalar1=ra[:, 0:1],
                                        scalar2=None, op0=ALU.mult)

                # mm2 + fup + add sa -> P3
                ctx_b = late.tile([128, HW], BF16, tag="ctx_b", name="ctx_b")
                for nt in range(NT):
                    pc2 = pst(f"pctx{nt}")
                    nc.tensor.matmul(pc2[:], lhsT=vT_b[:], rhs=A_b[:, nt * 512:(nt + 1) * 512],
                                     start=True, stop=True)
                    nc.scalar.activation(out=ctx_b[:, nt * 512:(nt + 1) * 512],
                                         in_=pc2[:], func=AF.Copy)
                for ob in range(2):
                    for nt in range(NT):
                        pu = pst(f"pfup{ob}_{nt}")
                        nc.tensor.matmul(pu[:], lhsT=W["wfup"][:, ob * 128:(ob + 1) * 128],
                                         rhs=ctx_b[:, nt * 512:(nt + 1) * 512],
                                         start=True, stop=True)
                        fs = stg.tile([128, 512], BF16, tag="fs", bufs=3,
                                      name=f"fs{ob}_{nt}")
                        nc.scalar.activation(out=fs, in_=pu[:], func=AF.Relu,
                                             bias=W["tfup"][:, ob:ob + 1], scale=1.0)
                        nc.vector.scalar_tensor_tensor(
                            out=interior(P3[ob], nt), in0=fs, scalar=6.0,
                            in1=interior(P2[ob], nt), op0=ALU.min, op1=ALU.add)

                # fuse conv: P3 -> ctxf
                ctxf = [late.tile([128, HW], BF16, tag=f"ctxf{i}", name=f"ctxf{i}")
                        for i in range(2)]
                def emit_fuse(ob, nt, ps):
                    pv = ctxf[ob][:, nt * 512:(nt + 1) * 512]
                    nc.scalar.activation(out=pv, in_=ps[:], func=AF.Relu,
                                         bias=W["tfuse"][:, ob:ob + 1], scale=1.0)
                    relu6(pv)
                conv3x3(P3, "wfuse", 2, emit_fuse)

                # c1 1x1 over concat([ctxf, sp]) -> out
                cat = [ctxf[0], ctxf[1], sp[0], sp[1]]
                for ob in range(2):
                    for nt in range(NT):
                        ps = pst(f"pc1_{ob}_{nt}")
                        for j in range(4):
                            nc.tensor.matmul(ps[:], lhsT=W["wc1"][:, j, ob * 128:(ob + 1) * 128],
                                             rhs=cat[j][:, nt * 512:(nt + 1) * 512],
                                             start=(j == 0), stop=(j == 3))
                        og = stg.tile([128, 512], F32, tag="og", bufs=4,
                                      name=f"og{ob}_{nt}")
                        nc.scalar.activation(out=og, in_=ps[:], func=AF.Relu,
                                             bias=W["tc1"][:, ob:ob + 1], scale=1.0)
                        relu6(og[:])
                        nc.sync.dma_start(
                            out=out_d.ap()[ob * 128:(ob + 1) * 128, nt * 512:(nt + 1) * 512],
                            in_=og)

    nc.compile()
    return nc


# ------------------------------------------------------------------- wrapper

_CACHE = {}


def kernel(x, skip, params):
    from concourse import bass_utils

    x = np.asarray(x, np.float32)
    skip = np.asarray(skip, np.float32)
    B = x.shape[0]
    packed = _prep(params)

    if "nc" not in _CACHE:
        _CACHE["nc"] = build_program()
    nc = _CACHE["nc"]

    in_maps = []
    for i in range(B):
        m = dict(packed)
        m["x"] = np.ascontiguousarray(x[i].reshape(DIM, HW))
        m["skip"] = np.ascontiguousarray(skip[i].reshape(DIM, HW))
        in_maps.append(m)

    trace = bool(int(os.environ.get("KBENCH_TRACE", "0")))
    try:
        res = bass_utils.run_bass_kernel_spmd(
            nc, in_maps, core_ids=list(range(B)), trace=trace)
    except ModuleNotFoundError:
        # axon NTFF profiling hook unavailable in this environment
        os.environ["BASS_NEVER_TRACE"] = "1"
        res = bass_utils.run_bass_kernel_spmd(
            nc, in_maps, core_ids=list(range(B)), trace=False)
    _CACHE["last_result"] = res
    out = np.stack([r["out"].reshape(DIM, SZ, SZ) for r in res.results])
    return out.astype(np.float32)


# revision 39
# speedup vs baseline: 19.0859x; 1.0089x over previous
"""Trainium2 Bass kernel for nn_Attention (MCAM + MSAM + CIAFM block).

Sharding: pure data parallelism — B=8 samples across 8 NeuronCores.
Per core: x,skip (256, 64, 64) f32 -> out (256, 64, 64) f32.

Heavy compute = four 3x3 convs (ms, fq, fuse, c3) done as 9-tap
PSUM-accumulated bf16 matmuls over zero-padded (128, 66, 66) SBUF tiles,
plus 1x1 convs (fup, c1) and a tiny NC=19 cross-attention.

Host-side preprocessing folds:
  - BN scales into conv weights (cbr -> relu6(conv(x, W*s) + t))
  - the entire MCAM front end (4 ECA conv1ds + k=3 mixer + FC) into one
    linear map  g_pre = M @ [avgpool; maxpool] + d   (M: 256x512)

Schedule: inputs stream in per 512-column chunk (adds on GPSIMD, partial
pool stats on DVE during the DMA window); MSAM softmax+modulation feeds
the ms conv per chunk; the ms conv is split by input channel block so its
block-0 matmuls start before block-1's modulated input is finished. The
serial MCAM chain (tiny) is traced between ms and fq so it never blocks
the PE FIFO; c3 is traced before the attention softmax for the same
reason.
"""

import os
import numpy as np
import ml_dtypes

BF = ml_dtypes.bfloat16

DIM, NCLS, SZ = 256, 19, 64
KC = 128
HW = SZ * SZ          # 4096
PD = SZ + 2           # 66
NT = 8                # 512-wide output column tiles
RT = SZ // NT         # 8 rows per tile
S_ATT = float(KC) ** -0.5


# ------------------------------------------------------------------ host prep

def _toeplitz(w, n):
    """Dense matrix of 'same'-padded 1-D cross-correlation with kernel w."""
    w = np.asarray(w, np.float64).reshape(-1)
    k = len(w)
    pad = (k - 1) // 2
    T = np.zeros((n, n), np.float64)
    for j in range(k):
        d = j - pad
        lo, hi = max(0, -d), min(n, n - d)
        idx = np.arange(lo, hi)
        T[idx, idx + d] += w[j]
    return T


def _pack_conv(w, scale=None):
    """(O, I, kh, kw) -> (128, I//128, kh*kw, O): lhsT tiles per (in-block, tap)."""
    w = np.asarray(w, np.float64)
    if scale is not None:
        w = w * np.asarray(scale, np.float64)[:, None, None, None]
    O, I, kh, kw = w.shape
    t = w.reshape(O, I, kh * kw).transpose(1, 2, 0)          # (I, taps, O)
    t = t.reshape(I // 128, 128, kh * kw, O).transpose(1, 0, 2, 3)
    return np.ascontiguousarray(t)


def _bf16(a):
    return np.ascontiguousarray(np.asarray(a, np.float32)).astype(BF)


def _f32(a):
    return np.ascontiguousarray(np.asarray(a, np.float32))


def _cols(v, nb):
    """(nb*128,) bias vector -> (128, nb): column ob = v[ob*128:(ob+1)*128]."""
    return _f32(np.asarray(v, np.float64).reshape(nb, 128).T)


def _prep(params):
    p = {k: np.asarray(v, np.float64) for k, v in params.items()}
    o = {}

    # --- fused MCAM front end: g_pre = M @ [avg; max] + d ---
    T1 = np.zeros((8 * DIM, 2 * DIM))
    b1 = np.zeros(8 * DIM)
    for br in range(2):                       # 0 = avg branch, 1 = max branch
        for kk, nm in enumerate(("mc0", "mc1", "mc2", "mc3")):
            r = (br * 4 + kk) * DIM
            T1[r:r + DIM, br * DIM:(br + 1) * DIM] = _toeplitz(p[nm + "_w"], DIM)
            b1[r:r + DIM] = p[nm + "_b"][0]
    T2 = _toeplitz(p["mcc_w"], 8 * DIM)
    bias2 = T2 @ b1 + p["mcc_b"][0]
    M = p["fc_w"] @ T2 @ T1                   # (256, 512)
    d = p["fc_w"] @ bias2 + p["fc_b"]         # (256,)
    o["wg"] = _bf16(M.T.reshape(4, 128, DIM).transpose(1, 0, 2))   # (128,4,256)
    o["dg"] = _cols(d, 2)                                          # (128,2)

    # --- mcam mid (cse1/cse2/fc1) ---
    o["wcse1"] = _bf16(_pack_conv(p["cse1_w"]))[:, :, 0, :]        # (128,2,64)
    o["bcse1"] = _f32(p["cse1_b"].reshape(64, 1))
    o["wcse2"] = _bf16(p["cse2_w"][:, :, 0, 0].T)                  # (64,19)
    o["bcse2"] = _f32(p["cse2_b"].reshape(NCLS, 1))
    o["wfc1"] = _bf16(_pack_conv(p["fc1_w"], p["fc1_s"]))[:, :, 0, :]  # (128,2,19)
    o["bfc1"] = _f32(p["fc1_t"].reshape(NCLS, 1))

    # --- conv weights, BN scale folded ---
    o["wms"] = _bf16(_pack_conv(p["ms_w"], p["ms_s"]))             # (128,2,9,256)
    o["tms"] = _cols(p["ms_t"], 2)
    # fq weights in fp8 e4m3, DoubleRow layout (Ki=128, tap, Ko=2, M=128):
    # the attention branch is insensitive (8% q noise -> 3e-7 output change)
    wfq = _pack_conv(p["fq_w"], p["fq_s"])                         # (128,2,9,128)
    o["wfq8"] = np.ascontiguousarray(
        wfq.transpose(0, 2, 1, 3)).astype(np.float32).astype(
        ml_dtypes.float8_e4m3)                                     # (128,9,2,128)
    o["tfq"] = _f32(p["fq_t"].reshape(1, 128).T)                   # (128,1)
    o["wfk"] = _bf16(_pack_conv(p["fk_w"], p["fk_s"]))[:, :, 0, :]  # (128,2,128)
    o["tfk"] = _f32(p["fk_t"].reshape(1, 128).T)
    o["wfv"] = _bf16(_pack_conv(p["fv_w"], p["fv_s"]))[:, :, 0, :]
    o["tfv"] = _f32(p["fv_t"].reshape(1, 128).T)
    o["wfup"] = _bf16(_pack_conv(p["fup_w"], p["fup_s"]))[:, 0, 0, :]  # (128,256)
    o["tfup"] = _cols(p["fup_t"], 2)
    o["wfuse"] = _bf16(_pack_conv(p["fuse_w"], p["fuse_s"]))
    o["tfuse"] = _cols(p["fuse_t"], 2)
    o["wc3"] = _bf16(_pack_conv(p["c3_w"], p["c3_s"]))
    o["tc3"] = _cols(p["c3_t"], 2)
    o["wc1"] = _bf16(_pack_conv(p["c1_w"], p["c1_s"]))[:, :, 0, :]  # (128,4,256)
    o["tc1"] = _cols(p["c1_t"], 2)

    o["ident"] = _bf16(np.eye(128))
    return o


# ------------------------------------------------------------- device program

def build_program():
    import concourse.tile as tile
    from concourse import bacc, mybir

    AF = mybir.ActivationFunctionType
    ALU = mybir.AluOpType
    F32 = mybir.dt.float32
    BF16 = mybir.dt.bfloat16

    nc = bacc.Bacc("TRN2", target_bir_lowering=False, debug=False)

    specs = [
        ("x", (DIM, HW), F32), ("skip", (DIM, HW), F32),
        ("wms", (128, 2, 9, DIM), BF16), ("tms", (128, 2), F32),
        ("wfq8", (128, 9, 2, KC), mybir.dt.float8e4), ("tfq", (128, 1), F32),
        ("wg", (128, 4, DIM), BF16), ("dg", (128, 2), F32),
        ("wcse1", (128, 2, 64), BF16), ("bcse1", (64, 1), F32),
        ("wcse2", (64, NCLS), BF16), ("bcse2", (NCLS, 1), F32),
        ("wfc1", (128, 2, NCLS), BF16), ("bfc1", (NCLS, 1), F32),
        ("wfk", (128, 2, KC), BF16), ("tfk", (128, 1), F32),
        ("wfv", (128, 2, KC), BF16), ("tfv", (128, 1), F32),
        ("wfup", (128, DIM), BF16), ("tfup", (128, 2), F32),
        ("wfuse", (128, 2, 9, DIM), BF16), ("tfuse", (128, 2), F32),
        ("wc3", (128, 2, 9, DIM), BF16), ("tc3", (128, 2), F32),
        ("wc1", (128, 4, DIM), BF16), ("tc1", (128, 2), F32),
        ("ident", (128, 128), BF16),
    ]
    dram = {n: nc.dram_tensor(n, list(s), dt, kind="ExternalInput")
            for n, s, dt in specs}
    out_d = nc.dram_tensor("out", [DIM, HW], F32, kind="ExternalOutput")

    from concourse.tile import add_dep_helper

    with tile.TileContext(nc) as tc:
        with tc.tile_pool(name="cst", bufs=1) as cst, \
             tc.tile_pool(name="glob", bufs=1) as glob:

            # ---- padded conv-input buffers ----
            def padded(tagname):
                ts = [glob.tile([128, PD, PD], BF16, tag=f"{tagname}{i}",
                                name=f"{tagname}{i}") for i in range(2)]
                for t in ts:
                    nc.gpsimd.memset(t[:, 0, :], 0.0)
                    nc.gpsimd.memset(t[:, PD - 1, :], 0.0)
                    nc.gpsimd.memset(t[:, 1:PD - 1, 0], 0.0)
                    nc.gpsimd.memset(t[:, 1:PD - 1, PD - 1], 0.0)
                return ts

            P1 = padded("P1")   # msam y  (ms conv input)
            P2 = padded("P2")   # sa      (fq / c3 / fuse-add input)
            P3 = padded("P3")   # fup_out + sa (fuse conv input)

            def interior(P, nt=None):
                if nt is None:
                    return P[:, 1:1 + SZ, 1:1 + SZ]
                return P[:, 1 + nt * RT:1 + (nt + 1) * RT, 1:1 + SZ]

            # small tensors that cross the early/late phase boundary
            def gt(shape, dt, nm):
                return glob.tile(list(shape), dt, tag=nm, name=nm)

            proxy_b = [gt((128, NCLS), BF16, f"proxy{i}") for i in range(2)]
            k_b = gt((128, NCLS), BF16, "k_b")
            vT_b = gt((NCLS, 128), BF16, "vT_b")
            avg_b = [gt((128, 1), BF16, f"ab{i}") for i in range(2)]
            max_b = [gt((128, 1), BF16, f"mb{i}") for i in range(2)]

            relu6 = lambda ap: nc.vector.tensor_scalar_min(ap, ap, 6.0)

            # ============ early phase: stream inputs, MSAM -> P1 ============
            W = {}

            def load_const(names):
                for n, s, dt in specs:
                    if n in ("x", "skip") or n in W or n not in names:
                        continue
                    t = cst.tile(list(s), dt, tag=n, name=f"c_{n}")
                    nc.sync.dma_start(out=t, in_=dram[n].ap())
                    W[n] = t

            with tc.tile_pool(name="early", bufs=1) as early:
                xx = [early.tile([128, HW], F32, tag=f"xx{i}", name=f"xx{i}")
                      for i in range(2)]
                xw = [early.tile([128, SZ], F32, tag=f"xw{i}", name=f"xw{i}")
                      for i in range(2)]
                xhs = [early.tile([128, SZ], F32, tag=f"xhs{i}", name=f"xhs{i}")
                       for i in range(2)]
                xhp_all = [early.tile([128, SZ, NT], F32, tag=f"xhp{i}", name=f"xhp{i}")
                           for i in range(2)]
                xmaxp = [early.tile([128, NT], F32, tag=f"xmaxp{i}", name=f"xmaxp{i}")
                         for i in range(2)]

                def load_block(ib, dep=None):
                    x3 = xx[ib][:].rearrange("p (h w) -> p h w", w=SZ)
                    for c in range(NT):
                        sl = slice(c * 512, (c + 1) * 512)
                        xt = early.tile([128, 512], F32, tag="xt", bufs=6,
                                        name=f"xt{ib}_{c}")
                        st = early.tile([128, 512], F32, tag="st", bufs=6,
                                        name=f"st{ib}_{c}")
                        nc.sync.dma_start(out=xt, in_=dram["x"].ap()[ib * 128:(ib + 1) * 128, sl])
                        nc.sync.dma_start(out=st, in_=dram["skip"].ap()[ib * 128:(ib + 1) * 128, sl])
                        nc.gpsimd.tensor_add(xx[ib][:, sl], xt, st)
                        ch3 = x3[:, c * RT:(c + 1) * RT, :]
                        i1 = nc.vector.tensor_reduce(out=xw[ib][:, c * RT:(c + 1) * RT],
                                                     in_=ch3, axis=mybir.AxisListType.X,
                                                     op=ALU.add)
                        # column-sum partial for x_h: reduce the chunk's 8 rows
                        ch3t = xx[ib][:, sl].rearrange("p (h w) -> p w h", w=SZ)
                        i2 = nc.vector.tensor_reduce(out=xhp_all[ib][:, :, c], in_=ch3t,
                                                     axis=mybir.AxisListType.X, op=ALU.add)
                        if dep is not None:
                            add_dep_helper(i1.ins, dep.ins, sync=False,
                                           reason="keep b1 partials off the b0 softmax chain")
                            add_dep_helper(i2.ins, dep.ins, sync=False,
                                           reason="keep b1 partials off the b0 softmax chain")

                def msam_block(ib):
                    x3 = xx[ib][:].rearrange("p (h w) -> p h w", w=SZ)
                    xh = xhs[ib]
                    xh_inst = nc.vector.tensor_reduce(out=xh, in_=xhp_all[ib][:],
                                                      axis=mybir.AxisListType.X, op=ALU.add)
                    qk = early.tile([128, SZ, SZ], BF16, tag="qk", name=f"qk{ib}")
                    Ee = early.tile([128, SZ, SZ], F32, tag="Ee", name=f"Ee{ib}")
                    sqh = [early.tile([128, 1], F32, tag=f"sqh{h}", name=f"sq{ib}_{h}")
                           for h in range(2)]
                    qk_insts = []
                    for h in range(2):
                        rows = slice(h * (SZ // 2), (h + 1) * (SZ // 2))
                        qk_insts.append(nc.vector.scalar_tensor_tensor(
                            out=qk[:, rows, :],
                            in0=xw[ib][:, rows].unsqueeze(2).broadcast_to([128, SZ // 2, SZ]),
                            scalar=1.0 / (HW * 1.0),
                            in1=xh[:].unsqueeze(1).broadcast_to([128, SZ // 2, SZ]),
                            op0=ALU.mult, op1=ALU.mult))
                        nc.scalar.activation(out=Ee[:, rows, :], in_=qk[:, rows, :],
                                             func=AF.Exp, bias=0.0, scale=1.0,
                                             accum_out=sqh[h][:, 0:1])
                    sq = early.tile([128, 1], F32, tag=f"sq{ib}", name=f"sq{ib}")
                    nc.vector.tensor_add(sq, sqh[0], sqh[1])
                    rq = early.tile([128, 1], F32, tag=f"rq{ib}", name=f"rq{ib}")
                    nc.vector.reciprocal(rq, sq)
                    for c in range(NT):
                        Ech = Ee[:, c * RT:(c + 1) * RT, :]
                        xch = x3[:, c * RT:(c + 1) * RT, :]
                        # Ech <- xx*attn, then P1 <- Ech + xx (both on DVE so the
                        # y path never waits behind Pool's block-1 adds)
                        nc.vector.scalar_tensor_tensor(
                            out=Ech, in0=Ech, scalar=rq[:, 0:1], in1=xch,
                            op0=ALU.mult, op1=ALU.mult)
                        nc.vector.tensor_tensor(out=interior(P1[ib], c), in0=Ech,
                                                in1=xch, op=ALU.add)
                    return xh_inst, qk_insts[-1]

                def stats_block(ib, dep=None):
                    x3 = xx[ib][:].rearrange("p (h w) -> p h w", w=SZ)
                    for c in range(NT):
                        i1 = nc.vector.tensor_reduce(out=xmaxp[ib][:, c:c + 1],
                                                     in_=x3[:, c * RT:(c + 1) * RT, :],
                                                     axis=mybir.AxisListType.XY, op=ALU.max)
                        if dep is not None:
                            add_dep_helper(i1.ins, dep.ins, sync=False,
                                           reason="stats off the softmax chain")
                    ssum = early.tile([128, 1], F32, tag=f"ssum{ib}", name=f"ssum{ib}")
                    smax = early.tile([128, 1], F32, tag=f"smax{ib}", name=f"smax{ib}")
                    nc.vector.tensor_reduce(out=ssum, in_=xw[ib][:],
                                            axis=mybir.AxisListType.X, op=ALU.add)
                    nc.vector.tensor_reduce(out=smax, in_=xmaxp[ib][:],
                                            axis=mybir.AxisListType.X, op=ALU.max)
                    nc.scalar.activation(out=avg_b[ib], in_=ssum, func=AF.Copy,
                                         scale=1.0 / HW)
                    nc.scalar.activation(out=max_b[ib], in_=smax, func=AF.Copy)

                load_block(0)
                # ms-conv weights: after block-0's stream, before block-1's
                load_const({"wms", "tms"})
                xh0_inst, qk0_last = msam_block(0)
                load_block(1, dep=xh0_inst)
                _, qk1_last = msam_block(1)
                stats_block(0, dep=qk1_last)
                stats_block(1, dep=qk1_last)

                # remaining constants (queued behind the input loads)
                load_const({n for n, _, _ in specs})

            # =============== late phase: convs + mcam + attention ===========
            with tc.tile_pool(name="late", bufs=1) as late, \
                 tc.tile_pool(name="stg", bufs=1) as stg, \
                 tc.tile_pool(name="psp", bufs=8, space="PSUM") as psp:

                def pst(name):
                    return psp.tile([128, 512], F32, tag="ps", name=name)

                def psq(name, shape, dt=None):
                    return psp.tile(list(shape), dt or F32, tag="ps", name=name)

                TAPS_OUTER = bool(int(os.environ.get("KERNEL_TAPS_OUTER", "0")))

                def conv3x3(Pin, wname, n_ob, emit, ib_split=False, filler=None,
                            obs=None):
                    for ob in (range(n_ob) if obs is None else obs):
                        pslist = [pst(f"ps_{wname}_{ob}_{nt}") for nt in range(NT)]
                        ib_order = ((0, 1),) if not ib_split else ((0,), (1,))
                        if TAPS_OUTER:
                            # same stationary weight across all 8 n-tiles:
                            # 8x fewer LDWEIGHTS switches on the PE
                            for ibs in ib_order:
                                for ib in ibs:
                                    for t9 in range(9):
                                        ky, kx = divmod(t9, 3)
                                        lhsT = W[wname][:, ib, t9, ob * 128:(ob + 1) * 128]
                                        for nt in range(NT):
                                            y0 = nt * RT
                                            nc.tensor.matmul(
                                                pslist[nt][:], lhsT=lhsT,
                                                rhs=Pin[ib][:, y0 + ky:y0 + ky + RT, kx:kx + SZ],
                                                start=(ib == 0 and t9 == 0),
                                                stop=(ib == 1 and t9 == 8))
                                            if ib == 1 and t9 == 8:
                                                emit(ob, nt, pslist[nt])
                                    if ib == 1 and filler is not None:
                                        for step in (filler.pop(0) for _ in range(3) if filler):
                                            step()
                        else:
                            for ibs in ib_order:
                                for nt in range(NT):
                                    y0 = nt * RT
                                    for ib in ibs:
                                        for t9 in range(9):
                                            ky, kx = divmod(t9, 3)
                                            nc.tensor.matmul(
                                                pslist[nt][:],
                                                lhsT=W[wname][:, ib, t9, ob * 128:(ob + 1) * 128],
                                                rhs=Pin[ib][:, y0 + ky:y0 + ky + RT, kx:kx + SZ],
                                                start=(ib == 0 and t9 == 0),
                                                stop=(ib == 1 and t9 == 8))
                                    if ibs[-1] == 1:
                                        emit(ob, nt, pslist[nt])
                                    if filler is not None:
                                        for step in (filler.pop(0) for _ in range(2) if filler):
                                            step()

                # flat fp8 copy of sa (+zero margins) for the DoubleRow fq conv
                MARG = 80
                q8 = late.tile([128, 2, MARG + HW + MARG], mybir.dt.float8e4,
                               tag="q8", name="q8")
                nc.gpsimd.memset(q8[:, :, 0:MARG], 0.0)
                nc.gpsimd.memset(q8[:, :, MARG + HW:], 0.0)

                def emit_ms(ob, nt, ps):
                    pv = interior(P2[ob], nt)
                    nc.scalar.activation(out=pv, in_=ps[:], func=AF.Relu,
                                         bias=W["tms"][:, ob:ob + 1], scale=1.0)
                    relu6(pv)
                    nc.vector.tensor_copy(
                        q8[:, ob, MARG + nt * 512:MARG + (nt + 1) * 512], pv)
                conv3x3(P1, "wms", 2, emit_ms, ib_split=True)

                # ---- MCAM chain as filler steps (interleaved into fq conv) ----
                vblocks = [avg_b[0], avg_b[1], max_b[0], max_b[1]]
                g_colb = [late.tile([128, 1], BF16, tag=f"gc{i}", name=f"gc{i}")
                          for i in range(2)]
                g_row = late.tile([1, DIM], F32, tag="g_row", name="g_row")
                h_b = late.tile([64, 1], BF16, tag="h_b", name="h_b")
                y1c = late.tile([NCLS, 1], BF16, tag="y1c", name="y1c")
                f1c = late.tile([NCLS, 1], BF16, tag="f1c", name="f1c")
                rowt = {nm: late.tile([1, NCLS], F32, tag=f"row_{nm}", name=f"row_{nm}")
                        for nm in ("f1", "y1")}
                cfr = late.tile([1, NCLS], BF16, tag="cfr", name="cfr")
                cfc = late.tile([NCLS, 1], F32, tag="cfc", name="cfc")
                g19 = late.tile([NCLS, DIM], F32, tag="g19", name="g19")
                cmT = late.tile([NCLS, DIM], F32, tag="cmT", name="cmT")
                cme = late.tile([NCLS, DIM], BF16, tag="cme", name="cme")
                v_b = late.tile([128, NCLS], BF16, tag="v_b", name="v_b")
                steps = []

                def s_g_mms():
                    for ob in range(2):
                        pg = psq(f"psg{ob}", (128, 1))
                        for j in range(4):
                            nc.tensor.matmul(pg[:], lhsT=W["wg"][:, j, ob * 128:(ob + 1) * 128],
                                             rhs=vblocks[j], start=(j == 0), stop=(j == 3))
                        nc.scalar.activation(out=g_colb[ob], in_=pg[:], func=AF.Sigmoid,
                                             bias=W["dg"][:, ob:ob + 1], scale=1.0)

                def s_g_rows():
                    for ob in range(2):
                        pr = psq(f"psgr{ob}", (1, 128), BF16)
                        nc.tensor.transpose(pr[:], g_colb[ob][:], W["ident"][:, :])
                        nc.scalar.activation(out=g_row[:, ob * 128:(ob + 1) * 128],
                                             in_=pr[:], func=AF.Copy)
                steps += [s_g_mms, s_g_rows]

                def s_h():
                    ph = psq("psh", (64, 1))
                    for ib in range(2):
                        nc.tensor.matmul(ph[:], lhsT=W["wcse1"][:, ib, :], rhs=g_colb[ib],
                                         start=(ib == 0), stop=(ib == 1))
                    nc.scalar.activation(out=h_b, in_=ph[:], func=AF.Relu,
                                         bias=W["bcse1"][:, 0:1], scale=1.0)
                steps.append(s_h)

                def s_y1():
                    py = psq("psy", (NCLS, 1))
                    nc.tensor.matmul(py[:], lhsT=W["wcse2"][:, :], rhs=h_b,
                                     start=True, stop=True)
                    nc.scalar.activation(out=y1c, in_=py[:], func=AF.Sigmoid,
                                         bias=W["bcse2"][:, 0:1], scale=1.0)
                steps.append(s_y1)

                def s_f1():
                    pf = psq("psf", (NCLS, 1))
                    for ib in range(2):
                        nc.tensor.matmul(pf[:], lhsT=W["wfc1"][:, ib, :], rhs=g_colb[ib],
                                         start=(ib == 0), stop=(ib == 1))
                    nc.scalar.activation(out=f1c, in_=pf[:], func=AF.Identity,
                                         bias=W["bfc1"][:, 0:1], scale=1.0)
                steps.append(s_f1)

                def s_row(nm, src_t):
                    def f():
                        pr = psq(f"pr_{nm}", (1, NCLS), BF16)
                        nc.tensor.transpose(pr[:], src_t[:], W["ident"][0:NCLS, 0:NCLS])
                        nc.scalar.activation(out=rowt[nm], in_=pr[:], func=AF.Copy)
                    return f
                steps += [s_row("f1", f1c), s_row("y1", y1c)]

                def s_sm1():
                    f1r, y1r = rowt["f1"], rowt["y1"]
                    s1 = late.tile([1, 1], F32, tag="s1", name="s1")
                    e1 = late.tile([1, NCLS], F32, tag="e1", name="e1")
                    nc.scalar.activation(out=e1, in_=f1r[:], func=AF.Exp,
                                         bias=0.0, scale=1.0, accum_out=s1[:, 0:1])
                    r1 = late.tile([1, 1], F32, tag="r1", name="r1")
                    nc.vector.reciprocal(r1, s1)
                    nc.vector.tensor_scalar(out=e1, in0=e1, scalar1=r1[:, 0:1],
                                            scalar2=None, op0=ALU.mult)
                    nc.vector.tensor_add(cfr, e1, y1r)
                steps.append(s_sm1)

                def s_cfc():
                    pcx = psq("pc_cf", (NCLS, 1), BF16)
                    nc.tensor.transpose(pcx[:], cfr[:], W["ident"][0:1, 0:1])
                    nc.scalar.activation(out=cfc, in_=pcx[:], func=AF.Copy)
                    nc.gpsimd.partition_broadcast(g19[:], g_row[:])
                steps.append(s_cfc)

                def s_cm():
                    nc.vector.tensor_scalar(out=cmT, in0=g19, scalar1=cfc[:, 0:1],
                                            scalar2=None, op0=ALU.mult)
                    s2 = late.tile([NCLS, 1], F32, tag="s2", name="s2")
                    nc.scalar.activation(out=cme, in_=cmT[:], func=AF.Exp,
                                         bias=0.0, scale=1.0, accum_out=s2[:, 0:1])
                    r2 = late.tile([NCLS, 1], F32, tag="r2", name="r2")
                    nc.vector.reciprocal(r2, s2)
                    nc.vector.tensor_scalar(out=cme, in0=cme, scalar1=r2[:, 0:1],
                                            scalar2=None, op0=ALU.mult)
                steps.append(s_cm)

                def s_half(ib):
                    def f():
                        pt = psq(f"pt{ib}", (128, NCLS), BF16)
                        nc.tensor.transpose(pt[:], cme[:, ib * 128:(ib + 1) * 128],
                                            W["ident"][0:NCLS, 0:NCLS])
                        pp = late.tile([128, NCLS], F32, tag=f"pp{ib}", name=f"pp{ib}")
                        nc.scalar.activation(out=pp, in_=pt[:], func=AF.Copy)
                        s3 = late.tile([128, 1], F32, tag=f"s3_{ib}", name=f"s3_{ib}")
                        nc.scalar.activation(out=proxy_b[ib], in_=pp[:], func=AF.Exp,
                                             bias=0.0, scale=1.0, accum_out=s3[:, 0:1])
                        r3 = late.tile([128, 1], F32, tag=f"r3_{ib}", name=f"r3_{ib}")
                        nc.vector.reciprocal(r3, s3)
                        nc.vector.tensor_scalar(out=proxy_b[ib], in0=proxy_b[ib],
                                                scalar1=r3[:, 0:1], scalar2=None, op0=ALU.mult)
                    return f
                steps += [s_half(0), s_half(1)]

                def s_kv(wname, tname, dst):
                    def f():
                        pkv = psq(f"pkv_{wname}", (128, NCLS))
                        for ib in range(2):
                            nc.tensor.matmul(pkv[:], lhsT=W[wname][:, ib, :],
                                             rhs=proxy_b[ib], start=(ib == 0), stop=(ib == 1))
                        nc.scalar.activation(out=dst, in_=pkv[:], func=AF.Relu,
                                             bias=W[tname][:, 0:1], scale=1.0)
                        relu6(dst[:])
                    return f
                steps += [s_kv("wfk", "tfk", k_b), s_kv("wfv", "tfv", v_b)]

                def s_vT():
                    pvT = psq("pvT", (NCLS, 128), BF16)
                    nc.tensor.transpose(pvT[:], v_b[:], W["ident"][:, :])
                    nc.scalar.activation(out=vT_b, in_=pvT[:], func=AF.Copy)
                steps.append(s_vT)

                # ---- fq conv: P2 -> q ----
                q_b = late.tile([128, HW], BF16, tag="q_b", name="q_b")
                def emit_fq(ob, nt, ps):
                    pv = q_b[:, nt * 512:(nt + 1) * 512]
                    nc.scalar.activation(out=pv, in_=ps[:], func=AF.Relu,
                                         bias=W["tfq"][:, 0:1], scale=1.0)
                    relu6(pv)
                steps.pop(0)()          # g matmuls only; transposes wait out
                                        # the sigmoid latency under the fq conv
                for nt in range(NT):
                    ps = pst(f"ps_fq_{nt}")
                    for t9 in range(9):
                        ky, kx = divmod(t9, 3)
                        off = (ky - 1) * SZ + (kx - 1)
                        base = MARG + nt * 512 + off
                        nc.tensor.matmul(
                            ps[:], lhsT=W["wfq8"][:, t9, :, :],
                            rhs=q8[:, :, base:base + 512],
                            start=(t9 == 0), stop=(t9 == 8),
                            perf_mode=mybir.MatmulPerfMode.DoubleRow)
                    emit_fq(0, nt, ps)

                # c3 conv block 0: covers the tail of the MCAM chain
                sp = [late.tile([128, HW], BF16, tag=f"sp{i}", name=f"sp{i}")
                      for i in range(2)]
                def emit_c3(ob, nt, ps):
                    pv = sp[ob][:, nt * 512:(nt + 1) * 512]
                    nc.scalar.activation(out=pv, in_=ps[:], func=AF.Relu,
                                         bias=W["tc3"][:, ob:ob + 1], scale=1.0)
                    relu6(pv)
                conv3x3(P2, "wc3", 2, emit_c3, obs=(0,), filler=steps)
                while steps:
                    steps.pop(0)()

                # attention mm1: (19, 4096) = k^T q
                mm_b = late.tile([NCLS, HW], BF16, tag="mm_b", name="mm_b")
                for nt in range(NT):
                    pm = psp.tile([NCLS, 512], F32, tag="ps", name=f"pmm{nt}")
                    nc.tensor.matmul(pm[:], lhsT=k_b[:], rhs=q_b[:, nt * 512:(nt + 1) * 512],
                                     start=True, stop=True)
                    nc.scalar.activation(out=mm_b[:, nt * 512:(nt + 1) * 512],
                                         in_=pm[:], func=AF.Copy)

                # c3 conv block 1: covers the attention softmax
                conv3x3(P2, "wc3", 2, emit_c3, obs=(1,))

                # softmax over hw rows of mm (inputs are bounded: no max-sub needed)
                sa_s = late.tile([NCLS, 1], F32, tag="sa_s", name="sa_s")
                A_b = late.tile([NCLS, HW], BF16, tag="A_b", name="A_b")
                nc.scalar.activation(out=A_b, in_=mm_b[:], func=AF.Exp,
                                     bias=0.0, scale=S_ATT, accum_out=sa_s[:, 0:1])
                ra = late.tile([NCLS, 1], F32, tag="ra", name="ra")
                nc.vector.reciprocal(ra, sa_s)
                nc.vector.tensor_scalar(out=A_b, in0=A_b, scalar1=ra[:, 0:1],
                                        scalar2=None, op0=ALU.mult)

                # mm2 + fup + add sa -> P3
                ctx_b = late.tile([128, HW], BF16, tag="ctx_b", name="ctx_b")
                for nt in range(NT):
                    pc2 = pst(f"pctx{nt}")
                    nc.tensor.matmul(pc2[:], lhsT=vT_b[:], rhs=A_b[:, nt * 512:(nt + 1) * 512],
                                     start=True, stop=True)
                    nc.scalar.activation(out=ctx_b[:, nt * 512:(nt + 1) * 512],
                                         in_=pc2[:], func=AF.Copy)
                for ob in range(2):
                    for nt in range(NT):
                        pu = pst(f"pfup{ob}_{nt}")
                        nc.tensor.matmul(pu[:], lhsT=W["wfup"][:, ob * 128:(ob + 1) * 128],
                                         rhs=ctx_b[:, nt * 512:(nt + 1) * 512],
                                         start=True, stop=True)
                        fs = stg.tile([128, 512], BF16, tag="fs", bufs=3,
                                      name=f"fs{ob}_{nt}")
                        nc.scalar.activation(out=fs, in_=pu[:], func=AF.Relu,
                                             bias=W["tfup"][:, ob:ob + 1], scale=1.0)
                        nc.vector.scalar_tensor_tensor(
                            out=interior(P3[ob], nt), in0=fs, scalar=6.0,
                            in1=interior(P2[ob], nt), op0=ALU.min, op1=ALU.add)

                # fuse conv: P3 -> ctxf
                ctxf = [late.tile([128, HW], BF16, tag=f"ctxf{i}", name=f"ctxf{i}")
                        for i in range(2)]
                def emit_fuse(ob, nt, ps):
                    pv = ctxf[ob][:, nt * 512:(nt + 1) * 512]
                    nc.scalar.activation(out=pv, in_=ps[:], func=AF.Relu,
                                         bias=W["tfuse"][:, ob:ob + 1], scale=1.0)
                    relu6(pv)
                conv3x3(P3, "wfuse", 2, emit_fuse)

                # c1 1x1 over concat([ctxf, sp]) -> out
                cat = [ctxf[0], ctxf[1], sp[0], sp[1]]
                for ob in range(2):
                    for nt in range(NT):
                        ps = pst(f"pc1_{ob}_{nt}")
                        for j in range(4):
                            nc.tensor.matmul(ps[:], lhsT=W["wc1"][:, j, ob * 128:(ob + 1) * 128],
                                             rhs=cat[j][:, nt * 512:(nt + 1) * 512],
                                             start=(j == 0), stop=(j == 3))
                        og = stg.tile([128, 512], F32, tag="og", bufs=4,
                                      name=f"og{ob}_{nt}")
                        nc.scalar.activation(out=og, in_=ps[:], func=AF.Relu,
                                             bias=W["tc1"][:, ob:ob + 1], scale=1.0)
                        relu6(og[:])
                        nc.sync.dma_start(
                            out=out_d.ap()[ob * 128:(ob + 1) * 128, nt * 512:(nt + 1) * 512],
                            in_=og)

    nc.compile()
    return nc


# ------------------------------------------------------------------- wrapper

_CACHE = {}


def kernel(x, skip, params):
    from concourse import bass_utils

    x = np.asarray(x, np.float32)
    skip = np.asarray(skip, np.float32)
    B = x.shape[0]
    packed = _prep(params)

    if "nc" not in _CACHE:
        _CACHE["nc"] = build_program()
    nc = _CACHE["nc"]

    in_maps = []
    for i in range(B):
        m = dict(packed)
        m["x"] = np.ascontiguousarray(x[i].reshape(DIM, HW))
        m["skip"] = np.ascontiguousarray(skip[i].reshape(DIM, HW))
        in_maps.append(m)

    trace = bool(int(os.environ.get("KBENCH_TRACE", "0")))
    try:
        res = bass_utils.run_bass_kernel_spmd(
            nc, in_maps, core_ids=list(range(B)), trace=trace)
    except ModuleNotFoundError:
        # axon NTFF profiling hook unavailable in this environment
        os.environ["BASS_NEVER_TRACE"] = "1"
        res = bass_utils.run_bass_kernel_spmd(
            nc, in_maps, core_ids=list(range(B)), trace=False)
    _CACHE["last_result"] = res
    out = np.stack([r["out"].reshape(DIM, SZ, SZ) for r in res.results])
    return out.astype(np.float32)


# revision 43
# speedup vs baseline: 19.1517x; 1.0034x over previous
"""Trainium2 Bass kernel for nn_Attention (MCAM + MSAM + CIAFM block).

Sharding: pure data parallelism — B=8 samples across 8 NeuronCores.
Per core: x,skip (256, 64, 64) f32 -> out (256, 64, 64) f32.

Heavy compute = four 3x3 convs (ms, fq, fuse, c3) done as 9-tap
PSUM-accumulated bf16 matmuls over zero-padded (128, 66, 66) SBUF tiles,
plus 1x1 convs (fup, c1) and a tiny NC=19 cross-attention.

Host-side preprocessing folds:
  - BN scales into conv weights (cbr -> relu6(conv(x, W*s) + t))
  - the entire MCAM front end (4 ECA conv1ds + k=3 mixer + FC) into one
    linear map  g_pre = M @ [avgpool; maxpool] + d   (M: 256x512)

Schedule: inputs stream in per 512-column chunk (adds on GPSIMD, partial
pool stats on DVE during the DMA window); MSAM softmax+modulation feeds
the ms conv per chunk; the ms conv is split by input channel block so its
block-0 matmuls start before block-1's modulated input is finished. The
serial MCAM chain (tiny) is traced between ms and fq so it never blocks
the PE FIFO; c3 is traced before the attention softmax for the same
reason.
"""

import os
import numpy as np
import ml_dtypes

BF = ml_dtypes.bfloat16

DIM, NCLS, SZ = 256, 19, 64
KC = 128
HW = SZ * SZ          # 4096
PD = SZ + 2           # 66
NT = 8                # 512-wide output column tiles
RT = SZ // NT         # 8 rows per tile
S_ATT = float(KC) ** -0.5


# ------------------------------------------------------------------ host prep

def _toeplitz(w, n):
    """Dense matrix of 'same'-padded 1-D cross-correlation with kernel w."""
    w = np.asarray(w, np.float64).reshape(-1)
    k = len(w)
    pad = (k - 1) // 2
    T = np.zeros((n, n), np.float64)
    for j in range(k):
        d = j - pad
        lo, hi = max(0, -d), min(n, n - d)
        idx = np.arange(lo, hi)
        T[idx, idx + d] += w[j]
    return T


def _pack_conv(w, scale=None):
    """(O, I, kh, kw) -> (128, I//128, kh*kw, O): lhsT tiles per (in-block, tap)."""
    w = np.asarray(w, np.float64)
    if scale is not None:
        w = w * np.asarray(scale, np.float64)[:, None, None, None]
    O, I, kh, kw = w.shape
    t = w.reshape(O, I, kh * kw).transpose(1, 2, 0)          # (I, taps, O)
    t = t.reshape(I // 128, 128, kh * kw, O).transpose(1, 0, 2, 3)
    return np.ascontiguousarray(t)


def _bf16(a):
    return np.ascontiguousarray(np.asarray(a, np.float32)).astype(BF)


def _f32(a):
    return np.ascontiguousarray(np.asarray(a, np.float32))


def _cols(v, nb):
    """(nb*128,) bias vector -> (128, nb): column ob = v[ob*128:(ob+1)*128]."""
    return _f32(np.asarray(v, np.float64).reshape(nb, 128).T)


def _prep(params):
    p = {k: np.asarray(v, np.float64) for k, v in params.items()}
    o = {}

    # --- fused MCAM front end: g_pre = M @ [avg; max] + d ---
    T1 = np.zeros((8 * DIM, 2 * DIM))
    b1 = np.zeros(8 * DIM)
    for br in range(2):                       # 0 = avg branch, 1 = max branch
        for kk, nm in enumerate(("mc0", "mc1", "mc2", "mc3")):
            r = (br * 4 + kk) * DIM
            T1[r:r + DIM, br * DIM:(br + 1) * DIM] = _toeplitz(p[nm + "_w"], DIM)
            b1[r:r + DIM] = p[nm + "_b"][0]
    T2 = _toeplitz(p["mcc_w"], 8 * DIM)
    bias2 = T2 @ b1 + p["mcc_b"][0]
    M = p["fc_w"] @ T2 @ T1                   # (256, 512)
    d = p["fc_w"] @ bias2 + p["fc_b"]         # (256,)
    o["wg"] = _bf16(M.T.reshape(4, 128, DIM).transpose(1, 0, 2))   # (128,4,256)
    o["dg"] = _cols(d, 2)                                          # (128,2)

    # --- mcam mid (cse1/cse2/fc1) ---
    o["wcse1"] = _bf16(_pack_conv(p["cse1_w"]))[:, :, 0, :]        # (128,2,64)
    o["bcse1"] = _f32(p["cse1_b"].reshape(64, 1))
    o["wcse2"] = _bf16(p["cse2_w"][:, :, 0, 0].T)                  # (64,19)
    o["bcse2"] = _f32(p["cse2_b"].reshape(NCLS, 1))
    o["wfc1"] = _bf16(_pack_conv(p["fc1_w"], p["fc1_s"]))[:, :, 0, :]  # (128,2,19)
    o["bfc1"] = _f32(p["fc1_t"].reshape(NCLS, 1))

    # --- conv weights, BN scale folded ---
    o["wms"] = _bf16(_pack_conv(p["ms_w"], p["ms_s"]))             # (128,2,9,256)
    o["tms"] = _cols(p["ms_t"], 2)
    # fq weights in fp8 e4m3, DoubleRow layout (Ki=128, tap, Ko=2, M=128):
    # the attention branch is insensitive (8% q noise -> 3e-7 output change)
    wfq = _pack_conv(p["fq_w"], p["fq_s"])                         # (128,2,9,128)
    o["wfq8"] = np.ascontiguousarray(
        wfq.transpose(0, 2, 1, 3)).astype(np.float32).astype(
        ml_dtypes.float8_e4m3)                                     # (128,9,2,128)
    o["tfq"] = _f32(p["fq_t"].reshape(1, 128).T)                   # (128,1)
    o["wfk"] = _bf16(_pack_conv(p["fk_w"], p["fk_s"]))[:, :, 0, :]  # (128,2,128)
    o["tfk"] = _f32(p["fk_t"].reshape(1, 128).T)
    o["wfv"] = _bf16(_pack_conv(p["fv_w"], p["fv_s"]))[:, :, 0, :]
    o["tfv"] = _f32(p["fv_t"].reshape(1, 128).T)
    o["wfup"] = _bf16(_pack_conv(p["fup_w"], p["fup_s"]))[:, 0, 0, :]  # (128,256)
    o["tfup"] = _cols(p["fup_t"], 2)
    o["wfuse"] = _bf16(_pack_conv(p["fuse_w"], p["fuse_s"]))
    o["tfuse"] = _cols(p["fuse_t"], 2)
    o["wc3"] = _bf16(_pack_conv(p["c3_w"], p["c3_s"]))
    o["tc3"] = _cols(p["c3_t"], 2)
    o["wc1"] = _bf16(_pack_conv(p["c1_w"], p["c1_s"]))[:, :, 0, :]  # (128,4,256)
    o["tc1"] = _cols(p["c1_t"], 2)

    o["ident"] = _bf16(np.eye(128))
    return o


# ------------------------------------------------------------- device program

def build_program():
    import concourse.tile as tile
    from concourse import bacc, mybir

    AF = mybir.ActivationFunctionType
    ALU = mybir.AluOpType
    F32 = mybir.dt.float32
    BF16 = mybir.dt.bfloat16

    nc = bacc.Bacc("TRN2", target_bir_lowering=False, debug=False)

    specs = [
        ("x", (DIM, HW), F32), ("skip", (DIM, HW), F32),
        ("wms", (128, 2, 9, DIM), BF16), ("tms", (128, 2), F32),
        ("wfq8", (128, 9, 2, KC), mybir.dt.float8e4), ("tfq", (128, 1), F32),
        ("wg", (128, 4, DIM), BF16), ("dg", (128, 2), F32),
        ("wcse1", (128, 2, 64), BF16), ("bcse1", (64, 1), F32),
        ("wcse2", (64, NCLS), BF16), ("bcse2", (NCLS, 1), F32),
        ("wfc1", (128, 2, NCLS), BF16), ("bfc1", (NCLS, 1), F32),
        ("wfk", (128, 2, KC), BF16), ("tfk", (128, 1), F32),
        ("wfv", (128, 2, KC), BF16), ("tfv", (128, 1), F32),
        ("wfup", (128, DIM), BF16), ("tfup", (128, 2), F32),
        ("wfuse", (128, 2, 9, DIM), BF16), ("tfuse", (128, 2), F32),
        ("wc3", (128, 2, 9, DIM), BF16), ("tc3", (128, 2), F32),
        ("wc1", (128, 4, DIM), BF16), ("tc1", (128, 2), F32),
        ("ident", (128, 128), BF16),
    ]
    dram = {n: nc.dram_tensor(n, list(s), dt, kind="ExternalInput")
            for n, s, dt in specs}
    out_d = nc.dram_tensor("out", [DIM, HW], F32, kind="ExternalOutput")

    from concourse.tile import add_dep_helper

    with tile.TileContext(nc) as tc:
        with tc.tile_pool(name="cst", bufs=1) as cst, \
             tc.tile_pool(name="glob", bufs=1) as glob:

            # ---- padded conv-input buffers ----
            def padded(tagname):
                ts = [glob.tile([128, PD, PD], BF16, tag=f"{tagname}{i}",
                                name=f"{tagname}{i}") for i in range(2)]
                for t in ts:
                    nc.gpsimd.memset(t[:, 0, :], 0.0)
                    nc.gpsimd.memset(t[:, PD - 1, :], 0.0)
                    nc.gpsimd.memset(t[:, 1:PD - 1, 0], 0.0)
                    nc.gpsimd.memset(t[:, 1:PD - 1, PD - 1], 0.0)
                return ts

            P1 = padded("P1")   # msam y  (ms conv input)
            P2 = padded("P2")   # sa      (fq / c3 / fuse-add input)
            P3 = padded("P3")   # fup_out + sa (fuse conv input)

            def interior(P, nt=None):
                if nt is None:
                    return P[:, 1:1 + SZ, 1:1 + SZ]
                return P[:, 1 + nt * RT:1 + (nt + 1) * RT, 1:1 + SZ]

            # small tensors that cross the early/late phase boundary
            def gt(shape, dt, nm):
                return glob.tile(list(shape), dt, tag=nm, name=nm)

            proxy_b = [gt((128, NCLS), BF16, f"proxy{i}") for i in range(2)]
            k_b = gt((128, NCLS), BF16, "k_b")
            vT_b = gt((NCLS, 128), BF16, "vT_b")
            avg_b = [gt((128, 1), BF16, f"ab{i}") for i in range(2)]
            max_b = [gt((128, 1), BF16, f"mb{i}") for i in range(2)]

            relu6 = lambda ap: nc.vector.tensor_scalar_min(ap, ap, 6.0)

            # ============ early phase: stream inputs, MSAM -> P1 ============
            W = {}

            def load_const(names):
                for n, s, dt in specs:
                    if n in ("x", "skip") or n in W or n not in names:
                        continue
                    t = cst.tile(list(s), dt, tag=n, name=f"c_{n}")
                    nc.sync.dma_start(out=t, in_=dram[n].ap())
                    W[n] = t

            with tc.tile_pool(name="early", bufs=1) as early:
                xx = [early.tile([128, HW], F32, tag=f"xx{i}", name=f"xx{i}")
                      for i in range(2)]
                xw = [early.tile([128, SZ], F32, tag=f"xw{i}", name=f"xw{i}")
                      for i in range(2)]
                xhs = [early.tile([128, SZ], F32, tag=f"xhs{i}", name=f"xhs{i}")
                       for i in range(2)]
                xhp_all = [early.tile([128, SZ, NT], F32, tag=f"xhp{i}", name=f"xhp{i}")
                           for i in range(2)]
                xmaxp = [early.tile([128, NT], F32, tag=f"xmaxp{i}", name=f"xmaxp{i}")
                         for i in range(2)]

                def load_block(ib, dep=None):
                    x3 = xx[ib][:].rearrange("p (h w) -> p h w", w=SZ)
                    for c in range(NT):
                        sl = slice(c * 512, (c + 1) * 512)
                        xt = early.tile([128, 512], F32, tag="xt", bufs=6,
                                        name=f"xt{ib}_{c}")
                        st = early.tile([128, 512], F32, tag="st", bufs=6,
                                        name=f"st{ib}_{c}")
                        nc.sync.dma_start(out=xt, in_=dram["x"].ap()[ib * 128:(ib + 1) * 128, sl])
                        nc.sync.dma_start(out=st, in_=dram["skip"].ap()[ib * 128:(ib + 1) * 128, sl])
                        nc.gpsimd.tensor_add(xx[ib][:, sl], xt, st)
                        ch3 = x3[:, c * RT:(c + 1) * RT, :]
                        i1 = nc.vector.tensor_reduce(out=xw[ib][:, c * RT:(c + 1) * RT],
                                                     in_=ch3, axis=mybir.AxisListType.X,
                                                     op=ALU.add)
                        # column-sum partial for x_h: reduce the chunk's 8 rows
                        ch3t = xx[ib][:, sl].rearrange("p (h w) -> p w h", w=SZ)
                        i2 = nc.vector.tensor_reduce(out=xhp_all[ib][:, :, c], in_=ch3t,
                                                     axis=mybir.AxisListType.X, op=ALU.add)
                        if dep is not None:
                            add_dep_helper(i1.ins, dep.ins, sync=False,
                                           reason="keep b1 partials off the b0 softmax chain")
                            add_dep_helper(i2.ins, dep.ins, sync=False,
                                           reason="keep b1 partials off the b0 softmax chain")

                def msam_block(ib):
                    x3 = xx[ib][:].rearrange("p (h w) -> p h w", w=SZ)
                    xh = xhs[ib]
                    xh_inst = nc.vector.tensor_reduce(out=xh, in_=xhp_all[ib][:],
                                                      axis=mybir.AxisListType.X, op=ALU.add)
                    qk = early.tile([128, SZ, SZ], BF16, tag="qk", name=f"qk{ib}")
                    Ee = early.tile([128, SZ, SZ], F32, tag="Ee", name=f"Ee{ib}")
                    sqh = [early.tile([128, 1], F32, tag=f"sqh{h}", name=f"sq{ib}_{h}")
                           for h in range(2)]
                    qk_insts = []
                    for h in range(2):
                        rows = slice(h * (SZ // 2), (h + 1) * (SZ // 2))
                        qk_insts.append(nc.vector.scalar_tensor_tensor(
                            out=qk[:, rows, :],
                            in0=xw[ib][:, rows].unsqueeze(2).broadcast_to([128, SZ // 2, SZ]),
                            scalar=1.0 / (HW * 1.0),
                            in1=xh[:].unsqueeze(1).broadcast_to([128, SZ // 2, SZ]),
                            op0=ALU.mult, op1=ALU.mult))
                        nc.scalar.activation(out=Ee[:, rows, :], in_=qk[:, rows, :],
                                             func=AF.Exp, bias=0.0, scale=1.0,
                                             accum_out=sqh[h][:, 0:1])
                    sq = early.tile([128, 1], F32, tag=f"sq{ib}", name=f"sq{ib}")
                    nc.vector.tensor_add(sq, sqh[0], sqh[1])
                    rq = early.tile([128, 1], F32, tag=f"rq{ib}", name=f"rq{ib}")
                    nc.vector.reciprocal(rq, sq)
                    for c in range(NT):
                        Ech = Ee[:, c * RT:(c + 1) * RT, :]
                        xch = x3[:, c * RT:(c + 1) * RT, :]
                        # Ech <- xx*attn, then P1 <- Ech + xx (both on DVE so the
                        # y path never waits behind Pool's block-1 adds)
                        nc.vector.scalar_tensor_tensor(
                            out=Ech, in0=Ech, scalar=rq[:, 0:1], in1=xch,
                            op0=ALU.mult, op1=ALU.mult)
                        nc.vector.tensor_tensor(out=interior(P1[ib], c), in0=Ech,
                                                in1=xch, op=ALU.add)
                    return xh_inst, qk_insts[-1]

                def stats_block(ib, dep=None):
                    x3 = xx[ib][:].rearrange("p (h w) -> p h w", w=SZ)
                    for c in range(NT):
                        i1 = nc.vector.tensor_reduce(out=xmaxp[ib][:, c:c + 1],
                                                     in_=x3[:, c * RT:(c + 1) * RT, :],
                                                     axis=mybir.AxisListType.XY, op=ALU.max)
                        if dep is not None:
                            add_dep_helper(i1.ins, dep.ins, sync=False,
                                           reason="stats off the softmax chain")
                    ssum = early.tile([128, 1], F32, tag=f"ssum{ib}", name=f"ssum{ib}")
                    smax = early.tile([128, 1], F32, tag=f"smax{ib}", name=f"smax{ib}")
                    nc.vector.tensor_reduce(out=ssum, in_=xw[ib][:],
                                            axis=mybir.AxisListType.X, op=ALU.add)
                    nc.vector.tensor_reduce(out=smax, in_=xmaxp[ib][:],
                                            axis=mybir.AxisListType.X, op=ALU.max)
                    nc.scalar.activation(out=avg_b[ib], in_=ssum, func=AF.Copy,
                                         scale=1.0 / HW)
                    nc.scalar.activation(out=max_b[ib], in_=smax, func=AF.Copy)

                load_block(0)
                # ms-conv weights: after block-0's stream, before block-1's
                load_const({"wms", "tms"})
                xh0_inst, qk0_last = msam_block(0)
                load_block(1, dep=xh0_inst)
                _, qk1_last = msam_block(1)
                stats_block(0, dep=qk1_last)
                stats_block(1, dep=qk1_last)

                # remaining constants (queued behind the input loads)
                load_const({n for n, _, _ in specs})

            # =============== late phase: convs + mcam + attention ===========
            with tc.tile_pool(name="late", bufs=1) as late, \
                 tc.tile_pool(name="stg", bufs=1) as stg, \
                 tc.tile_pool(name="psp", bufs=8, space="PSUM") as psp:

                def pst(name):
                    return psp.tile([128, 512], F32, tag="ps", name=name)

                def psq(name, shape, dt=None):
                    return psp.tile(list(shape), dt or F32, tag="ps", name=name)

                TAPS_OUTER = bool(int(os.environ.get("KERNEL_TAPS_OUTER", "0")))

                def conv3x3(Pin, wname, n_ob, emit, ib_split=False, filler=None,
                            obs=None):
                    for ob in (range(n_ob) if obs is None else obs):
                        pslist = [pst(f"ps_{wname}_{ob}_{nt}") for nt in range(NT)]
                        ib_order = ((0, 1),) if not ib_split else ((0,), (1,))
                        if TAPS_OUTER:
                            # same stationary weight across all 8 n-tiles:
                            # 8x fewer LDWEIGHTS switches on the PE
                            for ibs in ib_order:
                                for ib in ibs:
                                    for t9 in range(9):
                                        ky, kx = divmod(t9, 3)
                                        lhsT = W[wname][:, ib, t9, ob * 128:(ob + 1) * 128]
                                        for nt in range(NT):
                                            y0 = nt * RT
                                            nc.tensor.matmul(
                                                pslist[nt][:], lhsT=lhsT,
                                                rhs=Pin[ib][:, y0 + ky:y0 + ky + RT, kx:kx + SZ],
                                                start=(ib == 0 and t9 == 0),
                                                stop=(ib == 1 and t9 == 8))
                                            if ib == 1 and t9 == 8:
                                                emit(ob, nt, pslist[nt])
                                    if ib == 1 and filler is not None:
                                        for step in (filler.pop(0) for _ in range(3) if filler):
                                            step()
                        else:
                            for ibs in ib_order:
                                for nt in range(NT):
                                    y0 = nt * RT
                                    for ib in ibs:
                                        for t9 in range(9):
                                            ky, kx = divmod(t9, 3)
                                            nc.tensor.matmul(
                                                pslist[nt][:],
                                                lhsT=W[wname][:, ib, t9, ob * 128:(ob + 1) * 128],
                                                rhs=Pin[ib][:, y0 + ky:y0 + ky + RT, kx:kx + SZ],
                                                start=(ib == 0 and t9 == 0),
                                                stop=(ib == 1 and t9 == 8))
                                    if ibs[-1] == 1:
                                        emit(ob, nt, pslist[nt])
                                    if filler is not None:
                                        for step in (filler.pop(0) for _ in range(2) if filler):
                                            step()

                # flat fp8 copy of sa (+zero margins) for the DoubleRow fq conv
                MARG = 80
                q8 = late.tile([128, 2, MARG + HW + MARG], mybir.dt.float8e4,
                               tag="q8", name="q8")
                nc.gpsimd.memset(q8[:, :, 0:MARG], 0.0)
                nc.gpsimd.memset(q8[:, :, MARG + HW:], 0.0)

                def emit_ms(ob, nt, ps):
                    pv = interior(P2[ob], nt)
                    nc.scalar.activation(out=pv, in_=ps[:], func=AF.Relu,
                                         bias=W["tms"][:, ob:ob + 1], scale=1.0)
                    relu6(pv)
                    nc.vector.tensor_copy(
                        q8[:, ob, MARG + nt * 512:MARG + (nt + 1) * 512], pv)
                conv3x3(P1, "wms", 2, emit_ms, ib_split=True)

                # ---- MCAM chain as filler steps (interleaved into fq conv) ----
                vblocks = [avg_b[0], avg_b[1], max_b[0], max_b[1]]
                g_colb = [late.tile([128, 1], BF16, tag=f"gc{i}", name=f"gc{i}")
                          for i in range(2)]
                g_row = late.tile([1, DIM], F32, tag="g_row", name="g_row")
                h_b = late.tile([64, 1], BF16, tag="h_b", name="h_b")
                y1c = late.tile([NCLS, 1], BF16, tag="y1c", name="y1c")
                f1c = late.tile([NCLS, 1], BF16, tag="f1c", name="f1c")
                rowt = {nm: late.tile([1, NCLS], F32, tag=f"row_{nm}", name=f"row_{nm}")
                        for nm in ("f1", "y1")}
                cfr = late.tile([1, NCLS], BF16, tag="cfr", name="cfr")
                cfc = late.tile([NCLS, 1], F32, tag="cfc", name="cfc")
                g19 = late.tile([NCLS, DIM], F32, tag="g19", name="g19")
                cmT = late.tile([NCLS, DIM], F32, tag="cmT", name="cmT")
                cme = late.tile([NCLS, DIM], BF16, tag="cme", name="cme")
                v_b = late.tile([128, NCLS], BF16, tag="v_b", name="v_b")
                steps = []

                def s_g_mms():
                    for ob in range(2):
                        pg = psq(f"psg{ob}", (128, 1))
                        for j in range(4):
                            nc.tensor.matmul(pg[:], lhsT=W["wg"][:, j, ob * 128:(ob + 1) * 128],
                                             rhs=vblocks[j], start=(j == 0), stop=(j == 3))
                        nc.scalar.activation(out=g_colb[ob], in_=pg[:], func=AF.Sigmoid,
                                             bias=W["dg"][:, ob:ob + 1], scale=1.0)

                def s_g_rows():
                    for ob in range(2):
                        pr = psq(f"psgr{ob}", (1, 128), BF16)
                        nc.tensor.transpose(pr[:], g_colb[ob][:], W["ident"][:, :])
                        nc.scalar.activation(out=g_row[:, ob * 128:(ob + 1) * 128],
                                             in_=pr[:], func=AF.Copy)
                steps += [s_g_mms, s_g_rows]

                def s_h():
                    ph = psq("psh", (64, 1))
                    for ib in range(2):
                        nc.tensor.matmul(ph[:], lhsT=W["wcse1"][:, ib, :], rhs=g_colb[ib],
                                         start=(ib == 0), stop=(ib == 1))
                    nc.scalar.activation(out=h_b, in_=ph[:], func=AF.Relu,
                                         bias=W["bcse1"][:, 0:1], scale=1.0)
                steps.append(s_h)

                def s_y1():
                    py = psq("psy", (NCLS, 1))
                    nc.tensor.matmul(py[:], lhsT=W["wcse2"][:, :], rhs=h_b,
                                     start=True, stop=True)
                    nc.scalar.activation(out=y1c, in_=py[:], func=AF.Sigmoid,
                                         bias=W["bcse2"][:, 0:1], scale=1.0)
                steps.append(s_y1)

                def s_f1():
                    pf = psq("psf", (NCLS, 1))
                    for ib in range(2):
                        nc.tensor.matmul(pf[:], lhsT=W["wfc1"][:, ib, :], rhs=g_colb[ib],
                                         start=(ib == 0), stop=(ib == 1))
                    nc.scalar.activation(out=f1c, in_=pf[:], func=AF.Identity,
                                         bias=W["bfc1"][:, 0:1], scale=1.0)
                steps.append(s_f1)

                def s_row(nm, src_t):
                    def f():
                        pr = psq(f"pr_{nm}", (1, NCLS), BF16)
                        nc.tensor.transpose(pr[:], src_t[:], W["ident"][0:NCLS, 0:NCLS])
                        nc.scalar.activation(out=rowt[nm], in_=pr[:], func=AF.Copy)
                    return f
                steps += [s_row("f1", f1c), s_row("y1", y1c)]

                def s_sm1():
                    f1r, y1r = rowt["f1"], rowt["y1"]
                    s1 = late.tile([1, 1], F32, tag="s1", name="s1")
                    e1 = late.tile([1, NCLS], F32, tag="e1", name="e1")
                    nc.scalar.activation(out=e1, in_=f1r[:], func=AF.Exp,
                                         bias=0.0, scale=1.0, accum_out=s1[:, 0:1])
                    r1 = late.tile([1, 1], F32, tag="r1", name="r1")
                    nc.vector.reciprocal(r1, s1)
                    nc.vector.tensor_scalar(out=e1, in0=e1, scalar1=r1[:, 0:1],
                                            scalar2=None, op0=ALU.mult)
                    nc.vector.tensor_add(cfr, e1, y1r)
                steps.append(s_sm1)

                def s_cfc():
                    pcx = psq("pc_cf", (NCLS, 1), BF16)
                    nc.tensor.transpose(pcx[:], cfr[:], W["ident"][0:1, 0:1])
                    nc.scalar.activation(out=cfc, in_=pcx[:], func=AF.Copy)
                    nc.gpsimd.partition_broadcast(g19[:], g_row[:])
                steps.append(s_cfc)

                def s_cm():
                    nc.vector.tensor_scalar(out=cmT, in0=g19, scalar1=cfc[:, 0:1],
                                            scalar2=None, op0=ALU.mult)
                    s2 = late.tile([NCLS, 1], F32, tag="s2", name="s2")
                    nc.scalar.activation(out=cme, in_=cmT[:], func=AF.Exp,
                                         bias=0.0, scale=1.0, accum_out=s2[:, 0:1])
                    r2 = late.tile([NCLS, 1], F32, tag="r2", name="r2")
                    nc.vector.reciprocal(r2, s2)
                    nc.vector.tensor_scalar(out=cme, in0=cme, scalar1=r2[:, 0:1],
                                            scalar2=None, op0=ALU.mult)
                steps.append(s_cm)

                def s_half(ib):
                    def f():
                        pt = psq(f"pt{ib}", (128, NCLS), BF16)
                        nc.tensor.transpose(pt[:], cme[:, ib * 128:(ib + 1) * 128],
                                            W["ident"][0:NCLS, 0:NCLS])
                        pp = late.tile([128, NCLS], F32, tag=f"pp{ib}", name=f"pp{ib}")
                        nc.scalar.activation(out=pp, in_=pt[:], func=AF.Copy)
                        s3 = late.tile([128, 1], F32, tag=f"s3_{ib}", name=f"s3_{ib}")
                        nc.scalar.activation(out=proxy_b[ib], in_=pp[:], func=AF.Exp,
                                             bias=0.0, scale=1.0, accum_out=s3[:, 0:1])
                        r3 = late.tile([128, 1], F32, tag=f"r3_{ib}", name=f"r3_{ib}")
                        nc.vector.reciprocal(r3, s3)
                        nc.vector.tensor_scalar(out=proxy_b[ib], in0=proxy_b[ib],
                                                scalar1=r3[:, 0:1], scalar2=None, op0=ALU.mult)
                    return f
                steps += [s_half(0), s_half(1)]

                def s_kv(wname, tname, dst):
                    def f():
                        pkv = psq(f"pkv_{wname}", (128, NCLS))
                        for ib in range(2):
                            nc.tensor.matmul(pkv[:], lhsT=W[wname][:, ib, :],
                                             rhs=proxy_b[ib], start=(ib == 0), stop=(ib == 1))
                        nc.scalar.activation(out=dst, in_=pkv[:], func=AF.Relu,
                                             bias=W[tname][:, 0:1], scale=1.0)
                        relu6(dst[:])
                    return f
                steps += [s_kv("wfk", "tfk", k_b), s_kv("wfv", "tfv", v_b)]

                def s_vT():
                    pvT = psq("pvT", (NCLS, 128), BF16)
                    nc.tensor.transpose(pvT[:], v_b[:], W["ident"][:, :])
                    nc.scalar.activation(out=vT_b, in_=pvT[:], func=AF.Copy)
                steps.append(s_vT)

                # ---- fq conv: P2 -> q ----
                q_b = late.tile([128, HW], BF16, tag="q_b", name="q_b")
                def emit_fq(ob, nt, ps):
                    pv = q_b[:, nt * 512:(nt + 1) * 512]
                    nc.scalar.activation(out=pv, in_=ps[:], func=AF.Relu,
                                         bias=W["tfq"][:, 0:1], scale=1.0)
                    relu6(pv)
                steps.pop(0)()          # g matmuls only; transposes wait out
                                        # the sigmoid latency under the fq conv
                for nt in range(NT):
                    ps = pst(f"ps_fq_{nt}")
                    for t9 in range(9):
                        ky, kx = divmod(t9, 3)
                        off = (ky - 1) * SZ + (kx - 1)
                        base = MARG + nt * 512 + off
                        nc.tensor.matmul(
                            ps[:], lhsT=W["wfq8"][:, t9, :, :],
                            rhs=q8[:, :, base:base + 512],
                            start=(t9 == 0), stop=(t9 == 8),
                            perf_mode=mybir.MatmulPerfMode.DoubleRow)
                    emit_fq(0, nt, ps)

                # c3 conv block 0: covers the tail of the MCAM chain
                sp = [late.tile([128, HW], BF16, tag=f"sp{i}", name=f"sp{i}")
                      for i in range(2)]
                def emit_c3(ob, nt, ps):
                    pv = sp[ob][:, nt * 512:(nt + 1) * 512]
                    nc.scalar.activation(out=pv, in_=ps[:], func=AF.Relu,
                                         bias=W["tc3"][:, ob:ob + 1], scale=1.0)
                    relu6(pv)
                conv3x3(P2, "wc3", 2, emit_c3, obs=(0,), filler=steps)
                while steps:
                    steps.pop(0)()

                # attention mm1: (19, 4096) = k^T q
                mm_b = late.tile([NCLS, HW], BF16, tag="mm_b", name="mm_b")
                for nt in range(NT):
                    pm = psp.tile([NCLS, 512], F32, tag="ps", name=f"pmm{nt}")
                    nc.tensor.matmul(pm[:], lhsT=k_b[:], rhs=q_b[:, nt * 512:(nt + 1) * 512],
                                     start=True, stop=True)
                    nc.scalar.activation(out=mm_b[:, nt * 512:(nt + 1) * 512],
                                         in_=pm[:], func=AF.Copy)

                # c3 conv block 1: covers the attention softmax + mcam tail
                conv3x3(P2, "wc3", 2, emit_c3, obs=(1,), filler=steps)
                while steps:
                    steps.pop(0)()

                # softmax over hw rows of mm (inputs are bounded: no max-sub needed)
                sa_s = late.tile([NCLS, 1], F32, tag="sa_s", name="sa_s")
                A_b = late.tile([NCLS, HW], BF16, tag="A_b", name="A_b")
                nc.scalar.activation(out=A_b, in_=mm_b[:], func=AF.Exp,
                                     bias=0.0, scale=S_ATT, accum_out=sa_s[:, 0:1])
                ra = late.tile([NCLS, 1], F32, tag="ra", name="ra")
                nc.vector.reciprocal(ra, sa_s)
                nc.vector.tensor_scalar(out=A_b, in0=A_b, scalar1=ra[:, 0:1],
                                        scalar2=None, op0=ALU.mult)

                # mm2 + fup + add sa -> P3 (fup trails mm2 by one chunk so the
                # PE never waits on the psum->SBUF evacuation)
                ctx_b = late.tile([128, HW], BF16, tag="ctx_b", name="ctx_b")
                def fup_pair(nt):
                    for ob in range(2):
                        pu = pst(f"pfup{ob}_{nt}")
                        nc.tensor.matmul(pu[:], lhsT=W["wfup"][:, ob * 128:(ob + 1) * 128],
                                         rhs=ctx_b[:, nt * 512:(nt + 1) * 512],
                                         start=True, stop=True)
                        fs = stg.tile([128, 512], BF16, tag="fs", bufs=3,
                                      name=f"fs{ob}_{nt}")
                        nc.scalar.activation(out=fs, in_=pu[:], func=AF.Relu,
                                             bias=W["tfup"][:, ob:ob + 1], scale=1.0)
                        nc.vector.scalar_tensor_tensor(
                            out=interior(P3[ob], nt), in0=fs, scalar=6.0,
                            in1=interior(P2[ob], nt), op0=ALU.min, op1=ALU.add)
                for nt in range(NT):
                    pc2 = pst(f"pctx{nt}")
                    nc.tensor.matmul(pc2[:], lhsT=vT_b[:], rhs=A_b[:, nt * 512:(nt + 1) * 512],
                                     start=True, stop=True)
                    nc.vector.tensor_copy(ctx_b[:, nt * 512:(nt + 1) * 512], pc2[:])
                for nt in range(NT):
                    fup_pair(nt)

                # fuse conv: P3 -> ctxf
                ctxf = [late.tile([128, HW], BF16, tag=f"ctxf{i}", name=f"ctxf{i}")
                        for i in range(2)]
                def emit_fuse(ob, nt, ps):
                    pv = ctxf[ob][:, nt * 512:(nt + 1) * 512]
                    nc.scalar.activation(out=pv, in_=ps[:], func=AF.Relu,
                                         bias=W["tfuse"][:, ob:ob + 1], scale=1.0)
                    relu6(pv)
                conv3x3(P3, "wfuse", 2, emit_fuse)

                # c1 1x1 over concat([ctxf, sp]) -> out
                cat = [ctxf[0], ctxf[1], sp[0], sp[1]]
                for ob in range(2):
                    for nt in range(NT):
                        ps = pst(f"pc1_{ob}_{nt}")
                        for j in range(4):
                            nc.tensor.matmul(ps[:], lhsT=W["wc1"][:, j, ob * 128:(ob + 1) * 128],
                                             rhs=cat[j][:, nt * 512:(nt + 1) * 512],
                                             start=(j == 0), stop=(j == 3))
                        og = stg.tile([128, 512], F32, tag="og", bufs=4,
                                      name=f"og{ob}_{nt}")
                        nc.scalar.activation(out=og, in_=ps[:], func=AF.Relu,
                                             bias=W["tc1"][:, ob:ob + 1], scale=1.0)
                        relu6(og[:])
                        nc.sync.dma_start(
                            out=out_d.ap()[ob * 128:(ob + 1) * 128, nt * 512:(nt + 1) * 512],
                            in_=og)

    nc.compile()
    return nc


# ------------------------------------------------------------------- wrapper

_CACHE = {}


def kernel(x, skip, params):
    from concourse import bass_utils

    x = np.asarray(x, np.float32)
    skip = np.asarray(skip, np.float32)
    B = x.shape[0]
    packed = _prep(params)

    if "nc" not in _CACHE:
        _CACHE["nc"] = build_program()
    nc = _CACHE["nc"]

    in_maps = []
    for i in range(B):
        m = dict(packed)
        m["x"] = np.ascontiguousarray(x[i].reshape(DIM, HW))
        m["skip"] = np.ascontiguousarray(skip[i].reshape(DIM, HW))
        in_maps.append(m)

    trace = bool(int(os.environ.get("KBENCH_TRACE", "0")))
    try:
        res = bass_utils.run_bass_kernel_spmd(
            nc, in_maps, core_ids=list(range(B)), trace=trace)
    except ModuleNotFoundError:
        # axon NTFF profiling hook unavailable in this environment
        os.environ["BASS_NEVER_TRACE"] = "1"
        res = bass_utils.run_bass_kernel_spmd(
            nc, in_maps, core_ids=list(range(B)), trace=False)
    _CACHE["last_result"] = res
    out = np.stack([r["out"].reshape(DIM, SZ, SZ) for r in res.results])
    return out.astype(np.float32)


# revision 44
# speedup vs baseline: 19.4429x; 1.0152x over previous
"""Trainium2 Bass kernel for nn_Attention (MCAM + MSAM + CIAFM block).

Sharding: pure data parallelism — B=8 samples across 8 NeuronCores.
Per core: x,skip (256, 64, 64) f32 -> out (256, 64, 64) f32.

Heavy compute = four 3x3 convs (ms, fq, fuse, c3) done as 9-tap
PSUM-accumulated bf16 matmuls over zero-padded (128, 66, 66) SBUF tiles,
plus 1x1 convs (fup, c1) and a tiny NC=19 cross-attention.

Host-side preprocessing folds:
  - BN scales into conv weights (cbr -> relu6(conv(x, W*s) + t))
  - the entire MCAM front end (4 ECA conv1ds + k=3 mixer + FC) into one
    linear map  g_pre = M @ [avgpool; maxpool] + d   (M: 256x512)

Schedule: inputs stream in per 512-column chunk (adds on GPSIMD, partial
pool stats on DVE during the DMA window); MSAM softmax+modulation feeds
the ms conv per chunk; the ms conv is split by input channel block so its
block-0 matmuls start before block-1's modulated input is finished. The
serial MCAM chain (tiny) is traced between ms and fq so it never blocks
the PE FIFO; c3 is traced before the attention softmax for the same
reason.
"""

import os
import numpy as np
import ml_dtypes

BF = ml_dtypes.bfloat16

DIM, NCLS, SZ = 256, 19, 64
KC = 128
HW = SZ * SZ          # 4096
PD = SZ + 2           # 66
NT = 8                # 512-wide output column tiles
RT = SZ // NT         # 8 rows per tile
S_ATT = float(KC) ** -0.5


# ------------------------------------------------------------------ host prep

def _toeplitz(w, n):
    """Dense matrix of 'same'-padded 1-D cross-correlation with kernel w."""
    w = np.asarray(w, np.float64).reshape(-1)
    k = len(w)
    pad = (k - 1) // 2
    T = np.zeros((n, n), np.float64)
    for j in range(k):
        d = j - pad
        lo, hi = max(0, -d), min(n, n - d)
        idx = np.arange(lo, hi)
        T[idx, idx + d] += w[j]
    return T


def _pack_conv(w, scale=None):
    """(O, I, kh, kw) -> (128, I//128, kh*kw, O): lhsT tiles per (in-block, tap)."""
    w = np.asarray(w, np.float64)
    if scale is not None:
        w = w * np.asarray(scale, np.float64)[:, None, None, None]
    O, I, kh, kw = w.shape
    t = w.reshape(O, I, kh * kw).transpose(1, 2, 0)          # (I, taps, O)
    t = t.reshape(I // 128, 128, kh * kw, O).transpose(1, 0, 2, 3)
    return np.ascontiguousarray(t)


def _bf16(a):
    return np.ascontiguousarray(np.asarray(a, np.float32)).astype(BF)


def _f32(a):
    return np.ascontiguousarray(np.asarray(a, np.float32))


def _cols(v, nb):
    """(nb*128,) bias vector -> (128, nb): column ob = v[ob*128:(ob+1)*128]."""
    return _f32(np.asarray(v, np.float64).reshape(nb, 128).T)


def _prep(params):
    p = {k: np.asarray(v, np.float64) for k, v in params.items()}
    o = {}

    # --- fused MCAM front end: g_pre = M @ [avg; max] + d ---
    T1 = np.zeros((8 * DIM, 2 * DIM))
    b1 = np.zeros(8 * DIM)
    for br in range(2):                       # 0 = avg branch, 1 = max branch
        for kk, nm in enumerate(("mc0", "mc1", "mc2", "mc3")):
            r = (br * 4 + kk) * DIM
            T1[r:r + DIM, br * DIM:(br + 1) * DIM] = _toeplitz(p[nm + "_w"], DIM)
            b1[r:r + DIM] = p[nm + "_b"][0]
    T2 = _toeplitz(p["mcc_w"], 8 * DIM)
    bias2 = T2 @ b1 + p["mcc_b"][0]
    M = p["fc_w"] @ T2 @ T1                   # (256, 512)
    d = p["fc_w"] @ bias2 + p["fc_b"]         # (256,)
    o["wg"] = _bf16(M.T.reshape(4, 128, DIM).transpose(1, 0, 2))   # (128,4,256)
    o["dg"] = _cols(d, 2)                                          # (128,2)

    # --- mcam mid (cse1/cse2/fc1) ---
    o["wcse1"] = _bf16(_pack_conv(p["cse1_w"]))[:, :, 0, :]        # (128,2,64)
    o["bcse1"] = _f32(p["cse1_b"].reshape(64, 1))
    o["wcse2"] = _bf16(p["cse2_w"][:, :, 0, 0].T)                  # (64,19)
    o["bcse2"] = _f32(p["cse2_b"].reshape(NCLS, 1))
    o["wfc1"] = _bf16(_pack_conv(p["fc1_w"], p["fc1_s"]))[:, :, 0, :]  # (128,2,19)
    o["bfc1"] = _f32(p["fc1_t"].reshape(NCLS, 1))

    # --- conv weights, BN scale folded ---
    o["wms"] = _bf16(_pack_conv(p["ms_w"], p["ms_s"]))             # (128,2,9,256)
    o["tms"] = _cols(p["ms_t"], 2)
    # fq weights in fp8 e4m3, DoubleRow layout (Ki=128, tap, Ko=2, M=128):
    # the attention branch is insensitive (8% q noise -> 3e-7 output change)
    wfq = _pack_conv(p["fq_w"], p["fq_s"])                         # (128,2,9,128)
    o["wfq8"] = np.ascontiguousarray(
        wfq.transpose(0, 2, 1, 3)).astype(np.float32).astype(
        ml_dtypes.float8_e4m3)                                     # (128,9,2,128)
    o["tfq"] = _f32(p["fq_t"].reshape(1, 128).T)                   # (128,1)
    o["wfk"] = _bf16(_pack_conv(p["fk_w"], p["fk_s"]))[:, :, 0, :]  # (128,2,128)
    o["tfk"] = _f32(p["fk_t"].reshape(1, 128).T)
    o["wfv"] = _bf16(_pack_conv(p["fv_w"], p["fv_s"]))[:, :, 0, :]
    o["tfv"] = _f32(p["fv_t"].reshape(1, 128).T)
    o["wfup"] = _bf16(_pack_conv(p["fup_w"], p["fup_s"]))[:, 0, 0, :]  # (128,256)
    o["tfup"] = _cols(p["fup_t"], 2)
    o["wfuse"] = _bf16(_pack_conv(p["fuse_w"], p["fuse_s"]))
    o["tfuse"] = _cols(p["fuse_t"], 2)
    o["wc3"] = _bf16(_pack_conv(p["c3_w"], p["c3_s"]))
    o["tc3"] = _cols(p["c3_t"], 2)
    o["wc1"] = _bf16(_pack_conv(p["c1_w"], p["c1_s"]))[:, :, 0, :]  # (128,4,256)
    o["tc1"] = _cols(p["c1_t"], 2)

    o["ident"] = _bf16(np.eye(128))
    return o


# ------------------------------------------------------------- device program

def build_program():
    import concourse.tile as tile
    from concourse import bacc, mybir

    AF = mybir.ActivationFunctionType
    ALU = mybir.AluOpType
    F32 = mybir.dt.float32
    BF16 = mybir.dt.bfloat16

    nc = bacc.Bacc("TRN2", target_bir_lowering=False, debug=False)

    specs = [
        ("x", (DIM, HW), F32), ("skip", (DIM, HW), F32),
        ("wms", (128, 2, 9, DIM), BF16), ("tms", (128, 2), F32),
        ("wfq8", (128, 9, 2, KC), mybir.dt.float8e4), ("tfq", (128, 1), F32),
        ("wg", (128, 4, DIM), BF16), ("dg", (128, 2), F32),
        ("wcse1", (128, 2, 64), BF16), ("bcse1", (64, 1), F32),
        ("wcse2", (64, NCLS), BF16), ("bcse2", (NCLS, 1), F32),
        ("wfc1", (128, 2, NCLS), BF16), ("bfc1", (NCLS, 1), F32),
        ("wfk", (128, 2, KC), BF16), ("tfk", (128, 1), F32),
        ("wfv", (128, 2, KC), BF16), ("tfv", (128, 1), F32),
        ("wfup", (128, DIM), BF16), ("tfup", (128, 2), F32),
        ("wfuse", (128, 2, 9, DIM), BF16), ("tfuse", (128, 2), F32),
        ("wc3", (128, 2, 9, DIM), BF16), ("tc3", (128, 2), F32),
        ("wc1", (128, 4, DIM), BF16), ("tc1", (128, 2), F32),
        ("ident", (128, 128), BF16),
    ]
    dram = {n: nc.dram_tensor(n, list(s), dt, kind="ExternalInput")
            for n, s, dt in specs}
    out_d = nc.dram_tensor("out", [DIM, HW], F32, kind="ExternalOutput")

    from concourse.tile import add_dep_helper

    with tile.TileContext(nc) as tc:
        with tc.tile_pool(name="cst", bufs=1) as cst, \
             tc.tile_pool(name="glob", bufs=1) as glob:

            # ---- padded conv-input buffers ----
            def padded(tagname):
                ts = [glob.tile([128, PD, PD], BF16, tag=f"{tagname}{i}",
                                name=f"{tagname}{i}") for i in range(2)]
                for t in ts:
                    nc.gpsimd.memset(t[:, 0, :], 0.0)
                    nc.gpsimd.memset(t[:, PD - 1, :], 0.0)
                    nc.gpsimd.memset(t[:, 1:PD - 1, 0], 0.0)
                    nc.gpsimd.memset(t[:, 1:PD - 1, PD - 1], 0.0)
                return ts

            P1 = padded("P1")   # msam y  (ms conv input)
            P2 = padded("P2")   # sa      (fq / c3 / fuse-add input)
            P3 = padded("P3")   # fup_out + sa (fuse conv input)

            def interior(P, nt=None):
                if nt is None:
                    return P[:, 1:1 + SZ, 1:1 + SZ]
                return P[:, 1 + nt * RT:1 + (nt + 1) * RT, 1:1 + SZ]

            # small tensors that cross the early/late phase boundary
            def gt(shape, dt, nm):
                return glob.tile(list(shape), dt, tag=nm, name=nm)

            proxy_b = [gt((128, NCLS), BF16, f"proxy{i}") for i in range(2)]
            k_b = gt((128, NCLS), BF16, "k_b")
            avg_b = [gt((128, 1), BF16, f"ab{i}") for i in range(2)]
            max_b = [gt((128, 1), BF16, f"mb{i}") for i in range(2)]

            relu6 = lambda ap: nc.vector.tensor_scalar_min(ap, ap, 6.0)

            # ============ early phase: stream inputs, MSAM -> P1 ============
            W = {}

            def load_const(names):
                for n, s, dt in specs:
                    if n in ("x", "skip") or n in W or n not in names:
                        continue
                    t = cst.tile(list(s), dt, tag=n, name=f"c_{n}")
                    nc.sync.dma_start(out=t, in_=dram[n].ap())
                    W[n] = t

            with tc.tile_pool(name="early", bufs=1) as early:
                xx = [early.tile([128, HW], F32, tag=f"xx{i}", name=f"xx{i}")
                      for i in range(2)]
                xw = [early.tile([128, SZ], F32, tag=f"xw{i}", name=f"xw{i}")
                      for i in range(2)]
                xhs = [early.tile([128, SZ], F32, tag=f"xhs{i}", name=f"xhs{i}")
                       for i in range(2)]
                xhp_all = [early.tile([128, SZ, NT], F32, tag=f"xhp{i}", name=f"xhp{i}")
                           for i in range(2)]
                xmaxp = [early.tile([128, NT], F32, tag=f"xmaxp{i}", name=f"xmaxp{i}")
                         for i in range(2)]

                def load_block(ib, dep=None):
                    x3 = xx[ib][:].rearrange("p (h w) -> p h w", w=SZ)
                    for c in range(NT):
                        sl = slice(c * 512, (c + 1) * 512)
                        xt = early.tile([128, 512], F32, tag="xt", bufs=6,
                                        name=f"xt{ib}_{c}")
                        st = early.tile([128, 512], F32, tag="st", bufs=6,
                                        name=f"st{ib}_{c}")
                        nc.sync.dma_start(out=xt, in_=dram["x"].ap()[ib * 128:(ib + 1) * 128, sl])
                        nc.sync.dma_start(out=st, in_=dram["skip"].ap()[ib * 128:(ib + 1) * 128, sl])
                        nc.gpsimd.tensor_add(xx[ib][:, sl], xt, st)
                        ch3 = x3[:, c * RT:(c + 1) * RT, :]
                        i1 = nc.vector.tensor_reduce(out=xw[ib][:, c * RT:(c + 1) * RT],
                                                     in_=ch3, axis=mybir.AxisListType.X,
                                                     op=ALU.add)
                        # column-sum partial for x_h: reduce the chunk's 8 rows
                        ch3t = xx[ib][:, sl].rearrange("p (h w) -> p w h", w=SZ)
                        i2 = nc.vector.tensor_reduce(out=xhp_all[ib][:, :, c], in_=ch3t,
                                                     axis=mybir.AxisListType.X, op=ALU.add)
                        if dep is not None:
                            add_dep_helper(i1.ins, dep.ins, sync=False,
                                           reason="keep b1 partials off the b0 softmax chain")
                            add_dep_helper(i2.ins, dep.ins, sync=False,
                                           reason="keep b1 partials off the b0 softmax chain")

                def msam_block(ib):
                    x3 = xx[ib][:].rearrange("p (h w) -> p h w", w=SZ)
                    xh = xhs[ib]
                    xh_inst = nc.vector.tensor_reduce(out=xh, in_=xhp_all[ib][:],
                                                      axis=mybir.AxisListType.X, op=ALU.add)
                    qk = early.tile([128, SZ, SZ], BF16, tag="qk", name=f"qk{ib}")
                    Ee = early.tile([128, SZ, SZ], F32, tag="Ee", name=f"Ee{ib}")
                    sqh = [early.tile([128, 1], F32, tag=f"sqh{h}", name=f"sq{ib}_{h}")
                           for h in range(2)]
                    qk_insts = []
                    for h in range(2):
                        rows = slice(h * (SZ // 2), (h + 1) * (SZ // 2))
                        qk_insts.append(nc.vector.scalar_tensor_tensor(
                            out=qk[:, rows, :],
                            in0=xw[ib][:, rows].unsqueeze(2).broadcast_to([128, SZ // 2, SZ]),
                            scalar=1.0 / (HW * 1.0),
                            in1=xh[:].unsqueeze(1).broadcast_to([128, SZ // 2, SZ]),
                            op0=ALU.mult, op1=ALU.mult))
                        nc.scalar.activation(out=Ee[:, rows, :], in_=qk[:, rows, :],
                                             func=AF.Exp, bias=0.0, scale=1.0,
                                             accum_out=sqh[h][:, 0:1])
                    sq = early.tile([128, 1], F32, tag=f"sq{ib}", name=f"sq{ib}")
                    nc.vector.tensor_add(sq, sqh[0], sqh[1])
                    rq = early.tile([128, 1], F32, tag=f"rq{ib}", name=f"rq{ib}")
                    nc.vector.reciprocal(rq, sq)
                    for c in range(NT):
                        Ech = Ee[:, c * RT:(c + 1) * RT, :]
                        xch = x3[:, c * RT:(c + 1) * RT, :]
                        # Ech <- xx*attn, then P1 <- Ech + xx (both on DVE so the
                        # y path never waits behind Pool's block-1 adds)
                        nc.vector.scalar_tensor_tensor(
                            out=Ech, in0=Ech, scalar=rq[:, 0:1], in1=xch,
                            op0=ALU.mult, op1=ALU.mult)
                        nc.vector.tensor_tensor(out=interior(P1[ib], c), in0=Ech,
                                                in1=xch, op=ALU.add)
                    return xh_inst, qk_insts[-1]

                def stats_block(ib, dep=None):
                    x3 = xx[ib][:].rearrange("p (h w) -> p h w", w=SZ)
                    for c in range(NT):
                        i1 = nc.vector.tensor_reduce(out=xmaxp[ib][:, c:c + 1],
                                                     in_=x3[:, c * RT:(c + 1) * RT, :],
                                                     axis=mybir.AxisListType.XY, op=ALU.max)
                        if dep is not None:
                            add_dep_helper(i1.ins, dep.ins, sync=False,
                                           reason="stats off the softmax chain")
                    ssum = early.tile([128, 1], F32, tag=f"ssum{ib}", name=f"ssum{ib}")
                    smax = early.tile([128, 1], F32, tag=f"smax{ib}", name=f"smax{ib}")
                    nc.vector.tensor_reduce(out=ssum, in_=xw[ib][:],
                                            axis=mybir.AxisListType.X, op=ALU.add)
                    nc.vector.tensor_reduce(out=smax, in_=xmaxp[ib][:],
                                            axis=mybir.AxisListType.X, op=ALU.max)
                    nc.scalar.activation(out=avg_b[ib], in_=ssum, func=AF.Copy,
                                         scale=1.0 / HW)
                    nc.scalar.activation(out=max_b[ib], in_=smax, func=AF.Copy)

                load_block(0)
                # ms-conv weights: after block-0's stream, before block-1's
                load_const({"wms", "tms"})
                xh0_inst, qk0_last = msam_block(0)
                load_block(1, dep=xh0_inst)
                _, qk1_last = msam_block(1)
                stats_block(0, dep=qk1_last)
                stats_block(1, dep=qk1_last)

                # remaining constants (queued behind the input loads)
                load_const({n for n, _, _ in specs})

            # =============== late phase: convs + mcam + attention ===========
            with tc.tile_pool(name="late", bufs=1) as late, \
                 tc.tile_pool(name="stg", bufs=1) as stg, \
                 tc.tile_pool(name="psp", bufs=8, space="PSUM") as psp:

                def pst(name):
                    return psp.tile([128, 512], F32, tag="ps", name=name)

                def psq(name, shape, dt=None):
                    return psp.tile(list(shape), dt or F32, tag="ps", name=name)

                TAPS_OUTER = bool(int(os.environ.get("KERNEL_TAPS_OUTER", "0")))

                def conv3x3(Pin, wname, n_ob, emit, ib_split=False, filler=None,
                            obs=None):
                    for ob in (range(n_ob) if obs is None else obs):
                        pslist = [pst(f"ps_{wname}_{ob}_{nt}") for nt in range(NT)]
                        ib_order = ((0, 1),) if not ib_split else ((0,), (1,))
                        if TAPS_OUTER:
                            # same stationary weight across all 8 n-tiles:
                            # 8x fewer LDWEIGHTS switches on the PE
                            for ibs in ib_order:
                                for ib in ibs:
                                    for t9 in range(9):
                                        ky, kx = divmod(t9, 3)
                                        lhsT = W[wname][:, ib, t9, ob * 128:(ob + 1) * 128]
                                        for nt in range(NT):
                                            y0 = nt * RT
                                            nc.tensor.matmul(
                                                pslist[nt][:], lhsT=lhsT,
                                                rhs=Pin[ib][:, y0 + ky:y0 + ky + RT, kx:kx + SZ],
                                                start=(ib == 0 and t9 == 0),
                                                stop=(ib == 1 and t9 == 8))
                                            if ib == 1 and t9 == 8:
                                                emit(ob, nt, pslist[nt])
                                    if ib == 1 and filler is not None:
                                        for step in (filler.pop(0) for _ in range(3) if filler):
                                            step()
                        else:
                            for ibs in ib_order:
                                for nt in range(NT):
                                    y0 = nt * RT
                                    for ib in ibs:
                                        for t9 in range(9):
                                            ky, kx = divmod(t9, 3)
                                            nc.tensor.matmul(
                                                pslist[nt][:],
                                                lhsT=W[wname][:, ib, t9, ob * 128:(ob + 1) * 128],
                                                rhs=Pin[ib][:, y0 + ky:y0 + ky + RT, kx:kx + SZ],
                                                start=(ib == 0 and t9 == 0),
                                                stop=(ib == 1 and t9 == 8))
                                    if ibs[-1] == 1:
                                        emit(ob, nt, pslist[nt])
                                    if filler is not None:
                                        for step in (filler.pop(0) for _ in range(2) if filler):
                                            step()

                # flat fp8 copy of sa (+zero margins) for the DoubleRow fq conv
                MARG = 80
                q8 = late.tile([128, 2, MARG + HW + MARG], mybir.dt.float8e4,
                               tag="q8", name="q8")
                nc.gpsimd.memset(q8[:, :, 0:MARG], 0.0)
                nc.gpsimd.memset(q8[:, :, MARG + HW:], 0.0)

                def emit_ms(ob, nt, ps):
                    pv = interior(P2[ob], nt)
                    nc.scalar.activation(out=pv, in_=ps[:], func=AF.Relu,
                                         bias=W["tms"][:, ob:ob + 1], scale=1.0)
                    relu6(pv)
                    nc.vector.tensor_copy(
                        q8[:, ob, MARG + nt * 512:MARG + (nt + 1) * 512], pv)
                conv3x3(P1, "wms", 2, emit_ms, ib_split=True)

                # ---- MCAM chain as filler steps (interleaved into fq conv) ----
                vblocks = [avg_b[0], avg_b[1], max_b[0], max_b[1]]
                g_colb = [late.tile([128, 1], BF16, tag=f"gc{i}", name=f"gc{i}")
                          for i in range(2)]
                g_row = late.tile([1, DIM], F32, tag="g_row", name="g_row")
                h_b = late.tile([64, 1], BF16, tag="h_b", name="h_b")
                y1c = late.tile([NCLS, 1], BF16, tag="y1c", name="y1c")
                f1c = late.tile([NCLS, 1], BF16, tag="f1c", name="f1c")
                rowt = {nm: late.tile([1, NCLS], F32, tag=f"row_{nm}", name=f"row_{nm}")
                        for nm in ("f1", "y1")}
                cfr = late.tile([1, NCLS], BF16, tag="cfr", name="cfr")
                cfc = late.tile([NCLS, 1], F32, tag="cfc", name="cfc")
                g19 = late.tile([NCLS, DIM], F32, tag="g19", name="g19")
                cmT = late.tile([NCLS, DIM], F32, tag="cmT", name="cmT")
                cme = late.tile([NCLS, DIM], BF16, tag="cme", name="cme")
                v_b = late.tile([128, NCLS], BF16, tag="v_b", name="v_b")
                steps = []

                def s_g_mms():
                    for ob in range(2):
                        pg = psq(f"psg{ob}", (128, 1))
                        for j in range(4):
                            nc.tensor.matmul(pg[:], lhsT=W["wg"][:, j, ob * 128:(ob + 1) * 128],
                                             rhs=vblocks[j], start=(j == 0), stop=(j == 3))
                        nc.scalar.activation(out=g_colb[ob], in_=pg[:], func=AF.Sigmoid,
                                             bias=W["dg"][:, ob:ob + 1], scale=1.0)

                def s_g_rows():
                    for ob in range(2):
                        pr = psq(f"psgr{ob}", (1, 128), BF16)
                        nc.tensor.transpose(pr[:], g_colb[ob][:], W["ident"][:, :])
                        nc.scalar.activation(out=g_row[:, ob * 128:(ob + 1) * 128],
                                             in_=pr[:], func=AF.Copy)
                steps += [s_g_mms, s_g_rows]

                def s_h():
                    ph = psq("psh", (64, 1))
                    for ib in range(2):
                        nc.tensor.matmul(ph[:], lhsT=W["wcse1"][:, ib, :], rhs=g_colb[ib],
                                         start=(ib == 0), stop=(ib == 1))
                    nc.scalar.activation(out=h_b, in_=ph[:], func=AF.Relu,
                                         bias=W["bcse1"][:, 0:1], scale=1.0)
                steps.append(s_h)

                def s_y1():
                    py = psq("psy", (NCLS, 1))
                    nc.tensor.matmul(py[:], lhsT=W["wcse2"][:, :], rhs=h_b,
                                     start=True, stop=True)
                    nc.scalar.activation(out=y1c, in_=py[:], func=AF.Sigmoid,
                                         bias=W["bcse2"][:, 0:1], scale=1.0)
                steps.append(s_y1)

                def s_f1():
                    pf = psq("psf", (NCLS, 1))
                    for ib in range(2):
                        nc.tensor.matmul(pf[:], lhsT=W["wfc1"][:, ib, :], rhs=g_colb[ib],
                                         start=(ib == 0), stop=(ib == 1))
                    nc.scalar.activation(out=f1c, in_=pf[:], func=AF.Identity,
                                         bias=W["bfc1"][:, 0:1], scale=1.0)
                steps.append(s_f1)

                def s_row(nm, src_t):
                    def f():
                        pr = psq(f"pr_{nm}", (1, NCLS), BF16)
                        nc.tensor.transpose(pr[:], src_t[:], W["ident"][0:NCLS, 0:NCLS])
                        nc.scalar.activation(out=rowt[nm], in_=pr[:], func=AF.Copy)
                    return f
                steps += [s_row("f1", f1c), s_row("y1", y1c)]

                def s_sm1():
                    f1r, y1r = rowt["f1"], rowt["y1"]
                    s1 = late.tile([1, 1], F32, tag="s1", name="s1")
                    e1 = late.tile([1, NCLS], F32, tag="e1", name="e1")
                    nc.scalar.activation(out=e1, in_=f1r[:], func=AF.Exp,
                                         bias=0.0, scale=1.0, accum_out=s1[:, 0:1])
                    r1 = late.tile([1, 1], F32, tag="r1", name="r1")
                    nc.vector.reciprocal(r1, s1)
                    nc.vector.tensor_scalar(out=e1, in0=e1, scalar1=r1[:, 0:1],
                                            scalar2=None, op0=ALU.mult)
                    nc.vector.tensor_add(cfr, e1, y1r)
                steps.append(s_sm1)

                def s_cfc():
                    pcx = psq("pc_cf", (NCLS, 1), BF16)
                    nc.tensor.transpose(pcx[:], cfr[:], W["ident"][0:1, 0:1])
                    nc.scalar.activation(out=cfc, in_=pcx[:], func=AF.Copy)
                    nc.gpsimd.partition_broadcast(g19[:], g_row[:])
                steps.append(s_cfc)

                def s_cm():
                    nc.vector.tensor_scalar(out=cmT, in0=g19, scalar1=cfc[:, 0:1],
                                            scalar2=None, op0=ALU.mult)
                    s2 = late.tile([NCLS, 1], F32, tag="s2", name="s2")
                    nc.scalar.activation(out=cme, in_=cmT[:], func=AF.Exp,
                                         bias=0.0, scale=1.0, accum_out=s2[:, 0:1])
                    r2 = late.tile([NCLS, 1], F32, tag="r2", name="r2")
                    nc.vector.reciprocal(r2, s2)
                    nc.vector.tensor_scalar(out=cme, in0=cme, scalar1=r2[:, 0:1],
                                            scalar2=None, op0=ALU.mult)
                steps.append(s_cm)

                def s_half(ib):
                    def f():
                        pt = psq(f"pt{ib}", (128, NCLS), BF16)
                        nc.tensor.transpose(pt[:], cme[:, ib * 128:(ib + 1) * 128],
                                            W["ident"][0:NCLS, 0:NCLS])
                        pp = late.tile([128, NCLS], F32, tag=f"pp{ib}", name=f"pp{ib}")
                        nc.scalar.activation(out=pp, in_=pt[:], func=AF.Copy)
                        s3 = late.tile([128, 1], F32, tag=f"s3_{ib}", name=f"s3_{ib}")
                        nc.scalar.activation(out=proxy_b[ib], in_=pp[:], func=AF.Exp,
                                             bias=0.0, scale=1.0, accum_out=s3[:, 0:1])
                        r3 = late.tile([128, 1], F32, tag=f"r3_{ib}", name=f"r3_{ib}")
                        nc.vector.reciprocal(r3, s3)
                        nc.vector.tensor_scalar(out=proxy_b[ib], in0=proxy_b[ib],
                                                scalar1=r3[:, 0:1], scalar2=None, op0=ALU.mult)
                    return f
                steps += [s_half(0), s_half(1)]

                def s_kv(wname, tname, dst):
                    def f():
                        pkv = psq(f"pkv_{wname}", (128, NCLS))
                        for ib in range(2):
                            nc.tensor.matmul(pkv[:], lhsT=W[wname][:, ib, :],
                                             rhs=proxy_b[ib], start=(ib == 0), stop=(ib == 1))
                        nc.scalar.activation(out=dst, in_=pkv[:], func=AF.Relu,
                                             bias=W[tname][:, 0:1], scale=1.0)
                        relu6(dst[:])
                    return f
                steps += [s_kv("wfk", "tfk", k_b), s_kv("wfv", "tfv", v_b)]

                M2_b = late.tile([NCLS, DIM], BF16, tag="M2_b", name="M2_b")
                def s_M2():
                    # fup folded into the attention: M2 = v_b @ W_fup, so
                    # fup_out = M2.T @ A directly (no ctx round-trip)
                    pM2 = psq("pM2", (NCLS, DIM))
                    nc.tensor.matmul(pM2[:], lhsT=v_b[:], rhs=W["wfup"][:, :],
                                     start=True, stop=True)
                    nc.scalar.activation(out=M2_b, in_=pM2[:], func=AF.Copy)
                steps.append(s_M2)

                # ---- fq conv: P2 -> q ----
                q_b = late.tile([128, HW], BF16, tag="q_b", name="q_b")
                def emit_fq(ob, nt, ps):
                    pv = q_b[:, nt * 512:(nt + 1) * 512]
                    nc.scalar.activation(out=pv, in_=ps[:], func=AF.Relu,
                                         bias=W["tfq"][:, 0:1], scale=1.0)
                    relu6(pv)
                steps.pop(0)()          # g matmuls only; transposes wait out
                                        # the sigmoid latency under the fq conv
                for nt in range(NT):
                    ps = pst(f"ps_fq_{nt}")
                    for t9 in range(9):
                        ky, kx = divmod(t9, 3)
                        off = (ky - 1) * SZ + (kx - 1)
                        base = MARG + nt * 512 + off
                        nc.tensor.matmul(
                            ps[:], lhsT=W["wfq8"][:, t9, :, :],
                            rhs=q8[:, :, base:base + 512],
                            start=(t9 == 0), stop=(t9 == 8),
                            perf_mode=mybir.MatmulPerfMode.DoubleRow)
                    emit_fq(0, nt, ps)

                # c3 conv block 0: covers the tail of the MCAM chain
                sp = [late.tile([128, HW], BF16, tag=f"sp{i}", name=f"sp{i}")
                      for i in range(2)]
                def emit_c3(ob, nt, ps):
                    pv = sp[ob][:, nt * 512:(nt + 1) * 512]
                    nc.scalar.activation(out=pv, in_=ps[:], func=AF.Relu,
                                         bias=W["tc3"][:, ob:ob + 1], scale=1.0)
                    relu6(pv)
                conv3x3(P2, "wc3", 2, emit_c3, obs=(0,), filler=steps)
                while steps:
                    steps.pop(0)()

                # attention mm1: (19, 4096) = k^T q
                mm_b = late.tile([NCLS, HW], BF16, tag="mm_b", name="mm_b")
                for nt in range(NT):
                    pm = psp.tile([NCLS, 512], F32, tag="ps", name=f"pmm{nt}")
                    nc.tensor.matmul(pm[:], lhsT=k_b[:], rhs=q_b[:, nt * 512:(nt + 1) * 512],
                                     start=True, stop=True)
                    nc.scalar.activation(out=mm_b[:, nt * 512:(nt + 1) * 512],
                                         in_=pm[:], func=AF.Copy)

                # c3 conv block 1: covers the attention softmax + mcam tail
                conv3x3(P2, "wc3", 2, emit_c3, obs=(1,), filler=steps)
                while steps:
                    steps.pop(0)()

                # softmax over hw rows of mm (inputs are bounded: no max-sub needed)
                sa_s = late.tile([NCLS, 1], F32, tag="sa_s", name="sa_s")
                A_b = late.tile([NCLS, HW], BF16, tag="A_b", name="A_b")
                nc.scalar.activation(out=A_b, in_=mm_b[:], func=AF.Exp,
                                     bias=0.0, scale=S_ATT, accum_out=sa_s[:, 0:1])
                ra = late.tile([NCLS, 1], F32, tag="ra", name="ra")
                nc.vector.reciprocal(ra, sa_s)
                nc.vector.tensor_scalar(out=A_b, in0=A_b, scalar1=ra[:, 0:1],
                                        scalar2=None, op0=ALU.mult)

                # fup folded into mm2: fup_out = M2.T @ A per (ob, nt)
                for nt in range(NT):
                    for ob in range(2):
                        pu = pst(f"pfup{ob}_{nt}")
                        nc.tensor.matmul(pu[:], lhsT=M2_b[:, ob * 128:(ob + 1) * 128],
                                         rhs=A_b[:, nt * 512:(nt + 1) * 512],
                                         start=True, stop=True)
                        fs = stg.tile([128, 512], BF16, tag="fs", bufs=3,
                                      name=f"fs{ob}_{nt}")
                        nc.scalar.activation(out=fs, in_=pu[:], func=AF.Relu,
                                             bias=W["tfup"][:, ob:ob + 1], scale=1.0)
                        nc.vector.scalar_tensor_tensor(
                            out=interior(P3[ob], nt), in0=fs, scalar=6.0,
                            in1=interior(P2[ob], nt), op0=ALU.min, op1=ALU.add)

                # fuse conv: P3 -> ctxf
                ctxf = [late.tile([128, HW], BF16, tag=f"ctxf{i}", name=f"ctxf{i}")
                        for i in range(2)]
                def emit_fuse(ob, nt, ps):
                    pv = ctxf[ob][:, nt * 512:(nt + 1) * 512]
                    nc.scalar.activation(out=pv, in_=ps[:], func=AF.Relu,
                                         bias=W["tfuse"][:, ob:ob + 1], scale=1.0)
                    relu6(pv)
                conv3x3(P3, "wfuse", 2, emit_fuse)

                # c1 1x1 over concat([ctxf, sp]) -> out
                cat = [ctxf[0], ctxf[1], sp[0], sp[1]]
                for ob in range(2):
                    for nt in range(NT):
                        ps = pst(f"pc1_{ob}_{nt}")
                        for j in range(4):
                            nc.tensor.matmul(ps[:], lhsT=W["wc1"][:, j, ob * 128:(ob + 1) * 128],
                                             rhs=cat[j][:, nt * 512:(nt + 1) * 512],
                                             start=(j == 0), stop=(j == 3))
                        og = stg.tile([128, 512], F32, tag="og", bufs=4,
                                      name=f"og{ob}_{nt}")
                        nc.scalar.activation(out=og, in_=ps[:], func=AF.Relu,
                                             bias=W["tc1"][:, ob:ob + 1], scale=1.0)
                        relu6(og[:])
                        nc.sync.dma_start(
                            out=out_d.ap()[ob * 128:(ob + 1) * 128, nt * 512:(nt + 1) * 512],
                            in_=og)

    nc.compile()
    return nc


# ------------------------------------------------------------------- wrapper

_CACHE = {}


def kernel(x, skip, params):
    from concourse import bass_utils

    x = np.asarray(x, np.float32)
    skip = np.asarray(skip, np.float32)
    B = x.shape[0]
    packed = _prep(params)

    if "nc" not in _CACHE:
        _CACHE["nc"] = build_program()
    nc = _CACHE["nc"]

    in_maps = []
    for i in range(B):
        m = dict(packed)
        m["x"] = np.ascontiguousarray(x[i].reshape(DIM, HW))
        m["skip"] = np.ascontiguousarray(skip[i].reshape(DIM, HW))
        in_maps.append(m)

    trace = bool(int(os.environ.get("KBENCH_TRACE", "0")))
    try:
        res = bass_utils.run_bass_kernel_spmd(
            nc, in_maps, core_ids=list(range(B)), trace=trace)
    except ModuleNotFoundError:
        # axon NTFF profiling hook unavailable in this environment
        os.environ["BASS_NEVER_TRACE"] = "1"
        res = bass_utils.run_bass_kernel_spmd(
            nc, in_maps, core_ids=list(range(B)), trace=False)
    _CACHE["last_result"] = res
    out = np.stack([r["out"].reshape(DIM, SZ, SZ) for r in res.results])
    return out.astype(np.float32)
